# revision 10
# baseline (speedup 1.0000x reference)
"""Trainium2 Bass kernel for nn_DeepERA (GNN + CNN + GCN + MLP head), 8-core SPMD.

Self-contained: hardcodes shapes/sharding. Host does index gathers, weight
packing and layout prep; all dense compute runs on the 8 NeuronCores in two
SPMD launches (phase 1: entity embeddings; phase 2: pair MLPs + head).

Layouts (phase 1, per core):
  GNN: xs kept in "d-layout" [128, 5040] bf16 — partition p<64 -> (group 0,
  din=p), p>=64 -> (group 1, din=p-64); column = compound-in-group*40 + atom.
  Layer updates are never materialized: xs_{i+1} = xs0 + sum(deltas) is kept
  as separate delta tensors and accumulated in PSUM by the next matmul.
  CNN: per-protein doubled storage [128, 522] bf16 — partitions 0:64 hold
  x^T with left-pad 5, partitions 64:128 hold x^T with left-pad 4, so one
  [128, 512] rhs read at col offset 2p yields shifts (2p, 2p+1) stacked on
  the contraction dim. 11x11 conv = 6 such K=128 matmuls vs banded-pair
  weight matrices; two proteins run concurrently on PE col-groups via
  tile_position (0,0)/(0,64).
"""
import numpy as np
import ml_dtypes

import concourse.bass as bass
import concourse.bacc as bacc
import concourse.tile as tile
import concourse.mybir as mybir
from concourse.bass_utils import run_bass_kernel_spmd

BF16 = ml_dtypes.bfloat16
F32 = np.float32

DIM = 64
N_C = 2000
N_P = 1500
N_P_PAD = 1504           # 8 * 188
N_ATOMS = 40
L = 512
WIN = 5
B = 4096
NCORES = 8
CPC = N_C // NCORES      # 250 compounds / core
PPC = N_P_PAD // NCORES  # 188 proteins / core
BPC = B // NCORES        # 512 pairs / core
G = CPC // 2             # 125 compounds per partition-group
CHUNK = 42               # 3-compound chunks per group
GCP = CHUNK * 3 * N_ATOMS  # 5040 padded cols per group (5000 real)
NJ = 10
JW = GCP // NJ           # 504
NPAIR = PPC // 2         # 94 protein pairs / core

dt = mybir.dt
AFT = mybir.ActivationFunctionType


def _bands(K):
    """11 banded matrices Band_a[din, dout] = K[a, din - dout + 5]."""
    i, j = np.indices((DIM, DIM))
    bsel = i - j + WIN
    mask = (bsel >= 0) & (bsel < 11)
    out = np.zeros((11, DIM, DIM), np.float32)
    for a in range(11):
        out[a][mask] = K[a][bsel[mask]]
    return out


# ---------------------------------------------------------------- phase 1 ----
def build_phase1():
    nc = bacc.Bacc()
    bf, f32 = dt.bfloat16, dt.float32

    xw0_d = nc.dram_tensor("xw0", [128, GCP], bf, kind="ExternalInput")
    adjb_d = nc.dram_tensor("adjb", [120, 2 * CHUNK * 120], bf, kind="ExternalInput")
    wg_d = nc.dram_tensor("wg", [128, 3 * 128], bf, kind="ExternalInput")
    bg_d = nc.dram_tensor("bg", [128, 3], f32, kind="ExternalInput")
    idn_d = nc.dram_tensor("idn", [128, 128], bf, kind="ExternalInput")
    xp_d = nc.dram_tensor("xp", [PPC, 128, 522], bf, kind="ExternalInput")
    wc_d = nc.dram_tensor("wc", [128, 18 * 64], bf, kind="ExternalInput")
    bc_d = nc.dram_tensor("bc", [128, 3], f32, kind="ExternalInput")
    acf_d = nc.dram_tensor("acf", [16, 128, N_C], bf, kind="ExternalInput")
    acs_d = nc.dram_tensor("acs", [16, 128, CPC], bf, kind="ExternalInput")
    xsc_d = nc.dram_tensor("xsc", [128, 16 * 64], bf, kind="ExternalInput")
    wgd_d = nc.dram_tensor("wgd", [64, 128], bf, kind="ExternalInput")
    bgd_d = nc.dram_tensor("bgd", [64, 2], f32, kind="ExternalInput")
    apf_d = nc.dram_tensor("apf", [12, 128, N_P_PAD], bf, kind="ExternalInput")
    aps_d = nc.dram_tensor("aps", [12, 128, PPC], bf, kind="ExternalInput")
    xsp_d = nc.dram_tensor("xsp", [128, 12 * 64], bf, kind="ExternalInput")
    wgp_d = nc.dram_tensor("wgp", [64, 128], bf, kind="ExternalInput")
    bgp_d = nc.dram_tensor("bgp", [64, 2], f32, kind="ExternalInput")

    csum_d = nc.dram_tensor("csum", [128, 3 * CHUNK], f32, kind="ExternalOutput")
    pacc_d = nc.dram_tensor("pacc", [128, PPC], f32, kind="ExternalOutput")
    xc2_d = nc.dram_tensor("xc2", [64, CPC], bf, kind="ExternalOutput")
    xp2_d = nc.dram_tensor("xp2", [64, PPC], bf, kind="ExternalOutput")

    with tile.TileContext(nc) as tc:
        with tc.tile_pool(name="data", bufs=1) as data:
            # ---- persistent tiles + loads
            xw0 = data.tile([128, GCP], bf, name="xw0", tag="xw0")
            adjb = data.tile([120, 2 * CHUNK * 120], bf, name="adjb", tag="adjb")
            wg = data.tile([128, 3 * 128], bf, name="wg", tag="wg")
            bg = data.tile([128, 3], f32, name="bg", tag="bg")
            idn = data.tile([128, 128], bf, name="idn", tag="idn")
            wc = data.tile([128, 18 * 64], bf, name="wc", tag="wc")
            bc = data.tile([128, 3], f32, name="bc", tag="bc")
            for t, d in [(xw0, xw0_d), (adjb, adjb_d), (wg, wg_d), (bg, bg_d),
                         (idn, idn_d), (wc, wc_d), (bc, bc_d)]:
                nc.sync.dma_start(t[:], d[:])

            hsT = data.tile([128, GCP], bf, name="hsT", tag="hsT")
            dx = [data.tile([128, GCP], bf, name=f"dx{i}", tag=f"dx{i}") for i in range(3)]
            pracc = data.tile([128, PPC], f32, name="pracc", tag="pracc")

            # =================== GNN ===================
            with (
                tc.tile_pool(name="ps_h", bufs=2, space=bass.MemorySpace.PSUM) as ps_h,
                tc.tile_pool(name="ps_t", bufs=2, space=bass.MemorySpace.PSUM) as ps_t,
                tc.tile_pool(name="ps_d", bufs=2, space=bass.MemorySpace.PSUM) as ps_d,
                tc.tile_pool(name="ha_p", bufs=3) as ha_p,
            ):
                for ly in range(3):
                    srcs = [xw0] + dx[:ly]
                    for j in range(NJ):
                        ph = ps_h.tile([128, JW], f32, name="ph", tag="ph")
                        for si, s in enumerate(srcs):
                            nc.tensor.matmul(
                                ph[:], wg[:, ly * 128:(ly + 1) * 128],
                                s[:, j * JW:(j + 1) * JW],
                                start=(si == 0), stop=(si == len(srcs) - 1))
                        nc.scalar.activation(hsT[:, j * JW:(j + 1) * JW], ph[:],
                                             AFT.Relu, bias=bg[:, ly:ly + 1])
                    for c in range(CHUNK):
                        pt = ps_t.tile([120, 128], f32, name="pt", tag="pt")
                        nc.tensor.matmul(pt[:], hsT[:, c * 120:(c + 1) * 120],
                                         idn[:], start=True, stop=True)
                        ha = ha_p.tile([120, 128], bf, name="ha", tag="ha")
                        nc.scalar.copy(ha[:], pt[:])
                        pd = ps_d.tile([128, 120], f32, name="pd", tag="pd")
                        nc.tensor.matmul(
                            pd[0:64, :], ha[:, 0:64],
                            adjb[:, c * 120:(c + 1) * 120],
                            start=True, stop=True, tile_position=(0, 0))
                        nc.tensor.matmul(
                            pd[64:128, :], ha[:, 64:128],
                            adjb[:, (CHUNK + c) * 120:(CHUNK + c + 1) * 120],
                            start=True, stop=True, tile_position=(0, 64))
                        nc.vector.tensor_copy(dx[ly][:, c * 120:(c + 1) * 120], pd[:])

                # comp_int raw sums: reduce over atoms for xs0 + d1 + d2 + d3
                crs = []
                for si, s in enumerate([xw0] + dx):
                    cr = ha_p.tile([128, 3 * CHUNK], f32, name=f"cr{si}", tag=f"cr{si}")
                    nc.vector.reduce_sum(
                        cr[:], s[:].rearrange("p (c a) -> p c a", a=N_ATOMS),
                        axis=mybir.AxisListType.X)
                    crs.append(cr)
                nc.vector.tensor_add(crs[0][:], crs[0][:], crs[1][:])
                nc.vector.tensor_add(crs[2][:], crs[2][:], crs[3][:])
                nc.vector.tensor_add(crs[0][:], crs[0][:], crs[2][:])
                nc.sync.dma_start(csum_d[:], crs[0][:])

            # =================== CNN ===================
            with (
                tc.tile_pool(name="xb", bufs=1) as xb_pool,
                tc.tile_pool(name="ps_c", bufs=3, space=bass.MemorySpace.PSUM) as ps_c,
                tc.tile_pool(name="scr", bufs=2) as scr_p,
            ):
                xb = [xb_pool.tile([128, 522], bf, name=f"xb{i}", tag=f"xb{i}") for i in range(8)]
                for t in xb:
                    nc.gpsimd.memset(t[:], 0.0)
                for pr in range(NPAIR):
                    base = (pr % 2) * 4
                    cur0, nxt0 = xb[base + 0], xb[base + 1]
                    cur1, nxt1 = xb[base + 2], xb[base + 3]
                    nc.sync.dma_start(cur0[:], xp_d[2 * pr])
                    nc.sync.dma_start(cur1[:], xp_d[2 * pr + 1])
                    for ly in range(3):
                        pc = ps_c.tile([128, 512], f32, name="pc", tag="pc")
                        for p in range(6):
                            w = wc[:, (ly * 6 + p) * 64:(ly * 6 + p + 1) * 64]
                            nc.tensor.matmul(pc[0:64, :], w,
                                             cur0[:, 2 * p:2 * p + 512],
                                             start=(p == 0), stop=(p == 5),
                                             tile_position=(0, 0))
                            nc.tensor.matmul(pc[64:128, :], w,
                                             cur1[:, 2 * p:2 * p + 512],
                                             start=(p == 0), stop=(p == 5),
                                             tile_position=(0, 64))
                        if ly < 2:
                            nc.scalar.activation(nxt0[0:64, 5:517], pc[0:64, :],
                                                 AFT.Relu, bias=bc[0:64, ly:ly + 1])
                            nc.scalar.activation(nxt1[64:128, 4:516], pc[64:128, :],
                                                 AFT.Relu, bias=bc[64:128, ly:ly + 1])
                            nc.sync.dma_start(nxt0[64:128, 4:516], nxt0[0:64, 5:517])
                            nc.sync.dma_start(nxt1[0:64, 5:517], nxt1[64:128, 4:516])
                            cur0, nxt0 = nxt0, cur0
                            cur1, nxt1 = nxt1, cur1
                        else:
                            s = scr_p.tile([128, 512], bf, name="scr", tag="scr")
                            nc.scalar.activation(
                                s[0:64, :], pc[0:64, :], AFT.Relu,
                                bias=bc[0:64, 2:3],
                                accum_out=pracc[0:64, 2 * pr:2 * pr + 1])
                            nc.scalar.activation(
                                s[64:128, :], pc[64:128, :], AFT.Relu,
                                bias=bc[64:128, 2:3],
                                accum_out=pracc[64:128, 2 * pr + 1:2 * pr + 2])
                nc.sync.dma_start(pacc_d[:], pracc[:])

            # =================== GCN ===================
            def gcn(nk, nfull, jn, jw, nshard, af_d, as_d, xs_d, w_d, b_d, out_d):
                with (
                    tc.tile_pool(name="gd", bufs=1) as gd,
                    tc.tile_pool(name="ga", bufs=6) as ga,
                    tc.tile_pool(name="gt", bufs=3) as gt,
                    tc.tile_pool(name="ps_g", bufs=2, space=bass.MemorySpace.PSUM) as ps_g,
                    tc.tile_pool(name="ps_g2", bufs=2, space=bass.MemorySpace.PSUM) as ps_g2,
                    tc.tile_pool(name="ps_t2", bufs=2, space=bass.MemorySpace.PSUM) as ps_t2,
                ):
                    xs = gd.tile([128, nk * 64], dt.bfloat16, name="xs", tag="xs")
                    w = gd.tile([64, 128], dt.bfloat16, name="w", tag="w")
                    b = gd.tile([64, 2], dt.float32, name="b", tag="b")
                    x1T = gd.tile([64, nk * 128], dt.bfloat16, name="x1T", tag="x1T")
                    x1n = gd.tile([128, nk * 64], dt.bfloat16, name="x1n", tag="x1n")
                    nc.sync.dma_start(xs[:], xs_d[:])
                    nc.sync.dma_start(w[:], w_d[:])
                    nc.sync.dma_start(b[:], b_d[:])
                    if nk * 128 > nfull:
                        nc.gpsimd.memset(x1T[:, nfull:nk * 128], 0.0)
                    # layer 1 (full, redundant on all cores)
                    for j in range(jn):
                        pg = ps_g.tile([64, jw], dt.float32, name="pg", tag="pg")
                        for k in range(nk):
                            at = ga.tile([128, jw], dt.bfloat16, name="at", tag="at")
                            nc.sync.dma_start(at[:], af_d[k][:, j * jw:(j + 1) * jw])
                            nc.tensor.matmul(pg[:], xs[:, k * 64:(k + 1) * 64], at[:],
                                             start=(k == 0), stop=(k == nk - 1))
                        p1 = gt.tile([64, jw], dt.bfloat16, name="p1", tag="p1")
                        nc.scalar.copy(p1[:], pg[:])
                        pg2 = ps_g2.tile([64, jw], dt.float32, name="pg2", tag="pg2")
                        nc.tensor.matmul(pg2[:], w[:, 0:64], p1[:],
                                         start=True, stop=True)
                        nc.scalar.activation(x1T[:, j * jw:(j + 1) * jw], pg2[:],
                                             AFT.Relu, bias=b[:, 0:1])
                    # transpose x1T -> x1n (normal layout chunks)
                    for k in range(nk):
                        pt2 = ps_t2.tile([128, 64], dt.float32, name="pt2", tag="pt2")
                        nc.tensor.matmul(pt2[:], x1T[:, k * 128:(k + 1) * 128],
                                         idn[0:64, 0:64], start=True, stop=True)
                        nc.scalar.copy(x1n[:, k * 64:(k + 1) * 64], pt2[:])
                    # layer 2 (only this core's shard of rows)
                    pg = ps_g.tile([64, nshard], dt.float32, name="pgs", tag="pg")
                    for k in range(nk):
                        at = ga.tile([128, nshard], dt.bfloat16, name="ats", tag="ats")
                        nc.sync.dma_start(at[:], as_d[k][:])
                        nc.tensor.matmul(pg[:], x1n[:, k * 64:(k + 1) * 64], at[:],
                                         start=(k == 0), stop=(k == nk - 1))
                    p2 = gt.tile([64, nshard], dt.bfloat16, name="p2", tag="p2")
                    nc.scalar.copy(p2[:], pg[:])
                    pg2 = ps_g2.tile([64, nshard], dt.float32, name="pg2s", tag="pg2")
                    nc.tensor.matmul(pg2[:], w[:, 64:128], p2[:],
                                     start=True, stop=True)
                    x2T = gt.tile([64, nshard], dt.bfloat16, name="x2T", tag="x2T")
                    nc.scalar.activation(x2T[:], pg2[:], AFT.Relu, bias=b[:, 1:2])
                    nc.sync.dma_start(out_d[:], x2T[:])

            gcn(16, N_C, 4, 500, CPC, acf_d, acs_d, xsc_d, wgd_d, bgd_d, xc2_d)
            gcn(12, N_P_PAD, 4, 376, PPC, apf_d, aps_d, xsp_d, wgp_d, bgp_d, xp2_d)

    nc.compile()
    return nc


# ---------------------------------------------------------------- phase 2 ----
def build_phase2():
    nc = bacc.Bacc()
    bf, f32 = dt.bfloat16, dt.float32

    ent_d = nc.dram_tensor("ent", [256, BPC], f32, kind="ExternalInput")
    df_d = nc.dram_tensor("df", [8, 128, BPC], f32, kind="ExternalInput")
    pf_d = nc.dram_tensor("pf", [8, 128, BPC], f32, kind="ExternalInput")
    wd1_d = nc.dram_tensor("wd1", [128, 1024], f32, kind="ExternalInput")
    wd23_d = nc.dram_tensor("wd23", [128, 128], f32, kind="ExternalInput")
    wp1_d = nc.dram_tensor("wp1", [128, 1024], f32, kind="ExternalInput")
    wp23_d = nc.dram_tensor("wp23", [128, 128], f32, kind="ExternalInput")
    wo1_d = nc.dram_tensor("wo1", [128, 768], f32, kind="ExternalInput")
    wo2_d = nc.dram_tensor("wo2", [128, 512], f32, kind="ExternalInput")
    wo3_d = nc.dram_tensor("wo3", [128, 512], f32, kind="ExternalInput")
    wi_d = nc.dram_tensor("wi", [128, 4], f32, kind="ExternalInput")
    bia_d = nc.dram_tensor("bia", [128, 16], f32, kind="ExternalInput")
    # bia columns: 0 bd1, 1 bd2(0:64), 2 bd3(0:64), 3 bp1, 4 bp2, 5 bp3,
    #              6 bo1_m0, 7 bo1_m1, 8 bo2_m0, 9 bo2_m1, 10 bo3_m0, 11 bo3_m1,
    #              12 bint(0:2)
    out_d = nc.dram_tensor("out2", [2, BPC], f32, kind="ExternalOutput")

    with tile.TileContext(nc) as tc:
        with (
            tc.tile_pool(name="data", bufs=1) as data,
            tc.tile_pool(name="ps_a", bufs=2, space=bass.MemorySpace.PSUM) as ps_a,
            tc.tile_pool(name="ps_b", bufs=2, space=bass.MemorySpace.PSUM) as ps_b,
        ):
            wd1 = data.tile([128, 1024], f32, name="wd1", tag="wd1")
            wd23 = data.tile([128, 128], f32, name="wd23", tag="wd23")
            wp1 = data.tile([128, 1024], f32, name="wp1", tag="wp1")
            wp23 = data.tile([128, 128], f32, name="wp23", tag="wp23")
            wo1 = data.tile([128, 768], f32, name="wo1", tag="wo1")
            wo2 = data.tile([128, 512], f32, name="wo2", tag="wo2")
            wo3 = data.tile([128, 512], f32, name="wo3", tag="wo3")
            wi = data.tile([128, 4], f32, name="wi", tag="wi")
            bia = data.tile([128, 16], f32, name="bia", tag="bia")
            for t, d in [(wd1, wd1_d), (wd23, wd23_d), (wp1, wp1_d),
                         (wp23, wp23_d), (wo1, wo1_d), (wo2, wo2_d),
                         (wo3, wo3_d), (wi, wi_d), (bia, bia_d)]:
                nc.sync.dma_start(t[:], d[:])
            C = [data.tile([128, BPC], f32, name=f"C{k}", tag=f"C{k}") for k in range(3)]
            nc.sync.dma_start(C[0][:], ent_d[0:128, :])
            nc.sync.dma_start(C[1][64:128, :], ent_d[128:192, :])
            nc.sync.dma_start(C[2][0:64, :], ent_d[192:256, :])

            def mlp(src_d, w1, w23, bcol, outtile, outslice, tpos):
                f1p = ps_a.tile([128, BPC], f32, name="f1p", tag="psa")
                for k in range(8):
                    xt = data.tile([128, BPC], f32, name=f"xt{bcol}_{k}", tag=f"xt{bcol}_{k}")
                    nc.sync.dma_start(xt[:], src_d[k])
                    nc.tensor.matmul(f1p[:], w1[:, k * 128:(k + 1) * 128], xt[:],
                                     start=(k == 0), stop=(k == 7))
                f1 = data.tile([128, BPC], f32, name=f"f1_{bcol}", tag=f"f1_{bcol}")
                nc.scalar.activation(f1[:], f1p[:], AFT.Relu, bias=bia[:, bcol:bcol + 1])
                f2p = ps_b.tile([128, BPC], f32, name="f2p", tag="psb")
                nc.tensor.matmul(f2p[0:64, :], w23[:, 0:64], f1[:],
                                 start=True, stop=True)
                f2 = data.tile([64, BPC], f32, name=f"f2_{bcol}", tag=f"f2_{bcol}")
                nc.scalar.activation(f2[:], f2p[0:64, :], AFT.Relu,
                                     bias=bia[0:64, bcol + 1:bcol + 2])
                f3p = ps_b.tile([128, BPC], f32, name="f3p", tag="psb")
                lo, hi = (0, 64) if tpos == (0, 0) else (64, 128)
                nc.tensor.matmul(f3p[lo:hi, :], w23[0:64, 64:128], f2[:],
                                 start=True, stop=True, tile_position=tpos)
                nc.scalar.activation(outtile[outslice, :], f3p[lo:hi, :], AFT.Relu,
                                     bias=bia[lo:hi, bcol + 2:bcol + 3])

            mlp(df_d, wd1, wd23, 0, C[1], slice(0, 64), (0, 0))
            mlp(pf_d, wp1, wp23, 3, C[2], slice(64, 128), (0, 64))

            h = C
            for li, (wt, nk, bcol) in enumerate([(wo1, 3, 6), (wo2, 2, 8),
                                                 (wo3, 2, 10)]):
                hn = []
                for m in range(2):
                    hp = ps_a.tile([128, BPC], f32, name=f"hp{li}{m}", tag="psa")
                    for k in range(nk):
                        nc.tensor.matmul(hp[:], wt[:, (k * 2 + m) * 128:
                                                    (k * 2 + m + 1) * 128],
                                         h[k][:], start=(k == 0), stop=(k == nk - 1))
                    ht = data.tile([128, BPC], f32, name=f"h{li}{m}", tag=f"h{li}{m}")
                    nc.scalar.activation(ht[:], hp[:], AFT.Relu,
                                         bias=bia[:, bcol + m:bcol + m + 1])
                    hn.append(ht)
                h = hn
            zp = ps_b.tile([2, BPC], f32, name="zp", tag="psb")
            for k in range(2):
                nc.tensor.matmul(zp[:], wi[:, k * 2:(k + 1) * 2], h[k][:],
                                 start=(k == 0), stop=(k == 1))
            zs = data.tile([2, BPC], f32, name="zs", tag="zs")
            nc.scalar.activation(zs[:], zp[:], AFT.Sigmoid, bias=bia[0:2, 12:13])
            nc.sync.dma_start(out_d[:], zs[:])

    nc.compile()
    return nc


# ------------------------------------------------------------- host prep ----
def _prep_phase1_inputs(I):
    """Returns list of 8 per-core input dicts for phase 1."""
    bf = BF16
    emb_fp = np.asarray(I["embed_fp"], F32)
    compounds = np.asarray(I["compounds"])
    adj = np.asarray(I["adjacencies"], F32)
    W_gnn = np.asarray(I["W_gnn"], F32)
    b_gnn = np.asarray(I["b_gnn"], F32)
    emb_w = np.asarray(I["embed_word"], F32).astype(bf)
    proteins = np.asarray(I["proteins"])
    K_cnn = np.asarray(I["K_cnn"], F32)
    b_cnn = np.asarray(I["b_cnn"], F32)

    # GNN packing
    xg = emb_fp[compounds]                       # [N_C, 40, 64] f32
    wg = np.zeros((128, 3 * 128), F32)
    bg = np.zeros((128, 3), F32)
    for i in range(3):
        wg[0:64, i * 128:i * 128 + 64] = W_gnn[i]
        wg[64:128, i * 128 + 64:i * 128 + 128] = W_gnn[i]
        bg[0:64, i] = b_gnn[i]
        bg[64:128, i] = b_gnn[i]
    wg = wg.astype(bf)
    idn = np.eye(128, dtype=F32).astype(bf)

    # CNN packing
    bands = np.stack([_bands(K_cnn[i]) for i in range(3)])   # [3, 11, 64, 64]
    wcp = np.zeros((128, 18 * 64), F32)
    for i in range(3):
        for p in range(6):
            cb = (i * 6 + p) * 64
            wcp[0:64, cb:cb + 64] = bands[i, 2 * p]
            if 2 * p + 1 < 11:
                wcp[64:128, cb:cb + 64] = bands[i, 2 * p + 1]
    wcp = wcp.astype(bf)
    bcp = np.zeros((128, 3), F32)
    bcp[:, :] = b_cnn[None, :]

    prot_pad = np.zeros((N_P_PAD,) + proteins.shape[1:], proteins.dtype)
    prot_pad[:N_P] = proteins
    xw_all = emb_w[prot_pad]                     # [1504, 512, 64] bf16
    xT = np.ascontiguousarray(xw_all.transpose(0, 2, 1))  # [1504, 64, 512]
    xp_all = np.zeros((N_P_PAD, 128, 522), bf)
    xp_all[:, 0:64, 5:517] = xT
    xp_all[:, 64:128, 4:516] = xT

    # GCN packing
    def gcn_pack(A, Xs, Wl, bl, nk, nfull, npad, shard_lo, shard_n):
        Ap = np.zeros((nk * 128, npad), F32)
        Ap[:nfull, :nfull] = A[:nfull, :nfull]
        Xp = np.zeros((nk * 128, 64), F32)
        Xp[:nfull] = Xs[:nfull]
        af = np.ascontiguousarray(
            Ap.reshape(nk, 128, npad)).astype(bf)
        asd = np.ascontiguousarray(
            Ap[:, shard_lo:shard_lo + shard_n].reshape(nk, 128, shard_n)).astype(bf)
        xs = np.ascontiguousarray(
            Xp.reshape(nk, 128, 64).transpose(1, 0, 2).reshape(128, nk * 64)).astype(bf)
        w = np.concatenate([Wl[0], Wl[1]], axis=1).astype(bf)   # [64, 128]
        b = np.stack([bl[0], bl[1]], axis=1).astype(F32)        # [64, 2]
        return af, asd, xs, w, b

    A_c = np.asarray(I["A_c"], F32)
    A_p = np.asarray(I["A_p"], F32)
    Ap_pad = np.zeros((N_P_PAD, N_P_PAD), F32)
    Ap_pad[:N_P, :N_P] = A_p
    Xs_c = np.asarray(I["Xs_c"], F32)
    Xs_p = np.asarray(I["Xs_p"], F32)
    Xsp_pad = np.zeros((N_P_PAD, 64), F32)
    Xsp_pad[:N_P] = Xs_p

    in_maps = []
    for c in range(NCORES):
        m = {}
        # GNN per-core
        xs_c = xg[c * CPC:(c + 1) * CPC]          # [250, 40, 64]
        xw0 = np.zeros((128, GCP), F32)
        for g in range(2):
            blk = xs_c[g * G:(g + 1) * G].reshape(G * N_ATOMS, 64).T  # [64, 5000]
            xw0[g * 64:(g + 1) * 64, :G * N_ATOMS] = blk
        m["xw0"] = xw0.astype(bf)
        adjb = np.zeros((120, 2 * CHUNK * 120), F32)
        for g in range(2):
            for ch in range(CHUNK):
                for k3 in range(3):
                    ci = g * G + ch * 3 + k3
                    if ch * 3 + k3 < G:
                        cb = (g * CHUNK + ch) * 120 + k3 * 40
                        adjb[k3 * 40:(k3 + 1) * 40, cb:cb + 40] = \
                            adj[c * CPC + ci]
        m["adjb"] = adjb.astype(bf)
        m["wg"] = wg
        m["bg"] = bg
        m["idn"] = idn
        # CNN per-core
        m["xp"] = xp_all[c * PPC:(c + 1) * PPC]
        m["wc"] = wcp
        m["bc"] = bcp
        # GCN per-core
        acf, acs, xsc, wgd, bgd = gcn_pack(
            A_c, Xs_c, np.asarray(I["W_gcn_d"], F32), np.asarray(I["b_gcn_d"], F32),
            16, N_C, N_C, c * CPC, CPC)
        m["acf"], m["acs"], m["xsc"], m["wgd"], m["bgd"] = acf, acs, xsc, wgd, bgd
        apf, aps, xsp, wgp, bgp = gcn_pack(
            Ap_pad, Xsp_pad, np.asarray(I["W_gcn_p"], F32), np.asarray(I["b_gcn_p"], F32),
            12, N_P_PAD, N_P_PAD, c * PPC, PPC)
        m["apf"], m["aps"], m["xsp"], m["wgp"], m["bgp"] = apf, aps, xsp, wgp, bgp
        in_maps.append(m)
    return in_maps


def _prep_phase2_inputs(I, comp_intT, Xc2T, prot_intT, Xp2T):
    bf = BF16
    idx_c = np.asarray(I["idx_c"])
    idx_p = np.asarray(I["idx_p"])
    df = np.asarray(I["drug_feat"], F32)
    pf = np.asarray(I["protein_feat"], F32).astype(F32)

    def pack_w1(W):   # [1024, 128] -> [128, 1024] lhsT chunks
        out = np.zeros((128, 1024), F32)
        for k in range(8):
            out[:, k * 128:(k + 1) * 128] = W[k * 128:(k + 1) * 128, :]
        return out.astype(F32)

    def pack_w23(W2, W3):  # [128, 64], [64, 64] -> [128, 128]
        out = np.zeros((128, 128), F32)
        out[:, 0:64] = W2
        out[0:64, 64:128] = W3
        return out.astype(F32)

    def pack_head(W, nk):  # [nk*128, 256] -> [128, nk*256]
        out = np.zeros((128, nk * 256), F32)
        for k in range(nk):
            for mh in range(2):
                out[:, (k * 2 + mh) * 128:(k * 2 + mh + 1) * 128] = \
                    W[k * 128:(k + 1) * 128, mh * 128:(mh + 1) * 128]
        return out.astype(F32)

    wd1 = pack_w1(np.asarray(I["Wd1"], F32))
    wd23 = pack_w23(np.asarray(I["Wd2"], F32), np.asarray(I["Wd3"], F32))
    wp1 = pack_w1(np.asarray(I["Wp1"], F32))
    wp23 = pack_w23(np.asarray(I["Wp2"], F32), np.asarray(I["Wp3"], F32))
    wo1 = pack_head(np.asarray(I["Wo1"], F32), 3)
    wo2 = pack_head(np.asarray(I["Wo2"], F32), 2)
    wo3 = pack_head(np.asarray(I["Wo3"], F32), 2)
    wi = np.zeros((128, 4), F32)
    Wi = np.asarray(I["W_int"], F32)
    for k in range(2):
        wi[:, k * 2:(k + 1) * 2] = Wi[k * 128:(k + 1) * 128, :]
    wi = wi.astype(F32)
    bia = np.zeros((128, 16), F32)
    bia[:, 0] = np.asarray(I["bd1"], F32)
    bia[0:64, 1] = np.asarray(I["bd2"], F32)
    bia[0:64, 2] = np.asarray(I["bd3"], F32)
    bia[:, 3] = np.asarray(I["bp1"], F32)
    bia[0:64, 4] = np.asarray(I["bp2"], F32)
    bia[64:128, 5] = np.asarray(I["bp3"], F32)
    bo1 = np.asarray(I["bo1"], F32); bia[:, 6] = bo1[0:128]; bia[:, 7] = bo1[128:256]
    bo2 = np.asarray(I["bo2"], F32); bia[:, 8] = bo2[0:128]; bia[:, 9] = bo2[128:256]
    bo3 = np.asarray(I["bo3"], F32); bia[:, 10] = bo3[0:128]; bia[:, 11] = bo3[128:256]
    bia[0:2, 12] = np.asarray(I["b_int"], F32)

    in_maps = []
    for c in range(NCORES):
        ic = idx_c[c * BPC:(c + 1) * BPC]
        ip = idx_p[c * BPC:(c + 1) * BPC]
        ent = np.concatenate([comp_intT[:, ic], Xc2T[:, ic],
                              prot_intT[:, ip], Xp2T[:, ip]], axis=0).astype(F32)
        dfT = np.ascontiguousarray(df[ic].T).reshape(8, 128, BPC)
        pfT = np.ascontiguousarray(pf[ip].T).reshape(8, 128, BPC)
        m = dict(ent=ent, df=dfT, pf=pfT, wd1=wd1, wd23=wd23, wp1=wp1,
                 wp23=wp23, wo1=wo1, wo2=wo2, wo3=wo3, wi=wi, bia=bia)
        in_maps.append(m)
    return in_maps


_CACHE = {}


def _get_kernels():
    if "p1" not in _CACHE:
        _CACHE["p1"] = build_phase1()
        _CACHE["p2"] = build_phase2()
    return _CACHE["p1"], _CACHE["p2"]


def run(inputs, trace=False):
    """Full pipeline. Returns (output [4096, 2] f32, info dict)."""
    I = inputs
    nc1, nc2 = _get_kernels()
    info = {}

    in1 = _prep_phase1_inputs(I)
    r1 = run_bass_kernel_spmd(nc1, in1, core_ids=list(range(NCORES)), trace=trace)
    res1 = r1.results
    if trace:
        info["p1_exec_ns"] = r1.exec_time_ns

    comp_sumT = np.zeros((64, N_C), F32)
    Xc2T = np.zeros((64, N_C), F32)
    prot_sumT = np.zeros((64, N_P_PAD), F32)
    Xp2T = np.zeros((64, N_P_PAD), F32)
    for c in range(NCORES):
        cs = np.asarray(res1[c]["csum"], F32)       # [128, 126]
        comp_sumT[:, c * CPC:c * CPC + G] = cs[0:64, 0:G]
        comp_sumT[:, c * CPC + G:(c + 1) * CPC] = cs[64:128, 0:G]
        Xc2T[:, c * CPC:(c + 1) * CPC] = np.asarray(res1[c]["xc2"], F32)
        pa = np.asarray(res1[c]["pacc"], F32)       # [128, 188]
        ev = np.arange(0, PPC, 2)
        prot_sumT[:, c * PPC + ev] = pa[0:64, ev]
        prot_sumT[:, c * PPC + ev + 1] = pa[64:128, ev + 1]
        Xp2T[:, c * PPC:(c + 1) * PPC] = np.asarray(res1[c]["xp2"], F32)
    comp_intT = comp_sumT / N_ATOMS
    prot_intT = prot_sumT / L

    in2 = _prep_phase2_inputs(I, comp_intT, Xc2T, prot_intT, Xp2T)
    r2 = run_bass_kernel_spmd(nc2, in2, core_ids=list(range(NCORES)), trace=trace)
    res2 = r2.results
    if trace:
        info["p2_exec_ns"] = r2.exec_time_ns

    out = np.zeros((B, 2), F32)
    for c in range(NCORES):
        out[c * BPC:(c + 1) * BPC] = np.asarray(res2[c]["out2"], F32).T
    return out, info


def kernel(**inputs):
    out, _ = run(inputs)
    return out


# revision 11
# speedup vs baseline: 1.0239x; 1.0239x over previous
"""Trainium2 Bass kernel for nn_DeepERA (GNN + CNN + GCN + MLP head), 8-core SPMD.

Self-contained: hardcodes shapes/sharding. Host does index gathers, weight
packing and layout prep; all dense compute runs on the 8 NeuronCores in two
SPMD launches (phase 1: entity embeddings; phase 2: pair MLPs + head).

Layouts (phase 1, per core):
  GNN: xs kept in "d-layout" [128, 5040] bf16 — partition p<64 -> (group 0,
  din=p), p>=64 -> (group 1, din=p-64); column = compound-in-group*40 + atom.
  Layer updates are never materialized: xs_{i+1} = xs0 + sum(deltas) is kept
  as separate delta tensors and accumulated in PSUM by the next matmul.
  CNN: per-protein doubled storage [128, 522] bf16 — partitions 0:64 hold
  x^T with left-pad 5, partitions 64:128 hold x^T with left-pad 4, so one
  [128, 512] rhs read at col offset 2p yields shifts (2p, 2p+1) stacked on
  the contraction dim. 11x11 conv = 6 such K=128 matmuls vs banded-pair
  weight matrices; two proteins run concurrently on PE col-groups via
  tile_position (0,0)/(0,64).
"""
import numpy as np
import ml_dtypes

import concourse.bass as bass
import concourse.bacc as bacc
import concourse.tile as tile
import concourse.mybir as mybir
from concourse.bass_utils import run_bass_kernel_spmd

BF16 = ml_dtypes.bfloat16
F32 = np.float32

DIM = 64
N_C = 2000
N_P = 1500
N_P_PAD = 1504           # 8 * 188
N_ATOMS = 40
L = 512
WIN = 5
B = 4096
NCORES = 8
CPC = N_C // NCORES      # 250 compounds / core
PPC = N_P_PAD // NCORES  # 188 proteins / core
BPC = B // NCORES        # 512 pairs / core
G = CPC // 2             # 125 compounds per partition-group
CHUNK = 42               # 3-compound chunks per group
GCP = CHUNK * 3 * N_ATOMS  # 5040 padded cols per group (5000 real)
NJ = 10
JW = GCP // NJ           # 504
NPAIR = PPC // 2         # 94 protein pairs / core

dt = mybir.dt
AFT = mybir.ActivationFunctionType


def _bands(K):
    """11 banded matrices Band_a[din, dout] = K[a, din - dout + 5]."""
    i, j = np.indices((DIM, DIM))
    bsel = i - j + WIN
    mask = (bsel >= 0) & (bsel < 11)
    out = np.zeros((11, DIM, DIM), np.float32)
    for a in range(11):
        out[a][mask] = K[a][bsel[mask]]
    return out


# ---------------------------------------------------------------- phase 1 ----
def build_phase1():
    nc = bacc.Bacc()
    bf, f32 = dt.bfloat16, dt.float32

    xw0_d = nc.dram_tensor("xw0", [128, GCP], bf, kind="ExternalInput")
    adjb_d = nc.dram_tensor("adjb", [120, 2 * CHUNK * 120], bf, kind="ExternalInput")
    wg_d = nc.dram_tensor("wg", [128, 3 * 128], bf, kind="ExternalInput")
    bg_d = nc.dram_tensor("bg", [128, 3], f32, kind="ExternalInput")
    idn_d = nc.dram_tensor("idn", [128, 128], bf, kind="ExternalInput")
    xp_d = nc.dram_tensor("xp", [PPC, 128, 522], bf, kind="ExternalInput")
    wc_d = nc.dram_tensor("wc", [128, 18 * 64], bf, kind="ExternalInput")
    bc_d = nc.dram_tensor("bc", [128, 3], f32, kind="ExternalInput")
    acf_d = nc.dram_tensor("acf", [16, 128, N_C], bf, kind="ExternalInput")
    acs_d = nc.dram_tensor("acs", [16, 128, CPC], bf, kind="ExternalInput")
    xsc_d = nc.dram_tensor("xsc", [128, 16 * 64], bf, kind="ExternalInput")
    wgd_d = nc.dram_tensor("wgd", [64, 128], bf, kind="ExternalInput")
    bgd_d = nc.dram_tensor("bgd", [64, 2], f32, kind="ExternalInput")
    apf_d = nc.dram_tensor("apf", [12, 128, N_P_PAD], bf, kind="ExternalInput")
    aps_d = nc.dram_tensor("aps", [12, 128, PPC], bf, kind="ExternalInput")
    xsp_d = nc.dram_tensor("xsp", [128, 12 * 64], bf, kind="ExternalInput")
    wgp_d = nc.dram_tensor("wgp", [64, 128], bf, kind="ExternalInput")
    bgp_d = nc.dram_tensor("bgp", [64, 2], f32, kind="ExternalInput")

    csum_d = nc.dram_tensor("csum", [128, 3 * CHUNK], f32, kind="ExternalOutput")
    pacc_d = nc.dram_tensor("pacc", [128, PPC], f32, kind="ExternalOutput")
    xc2_d = nc.dram_tensor("xc2", [64, CPC], bf, kind="ExternalOutput")
    xp2_d = nc.dram_tensor("xp2", [64, PPC], bf, kind="ExternalOutput")

    with tile.TileContext(nc) as tc:
        with tc.tile_pool(name="data", bufs=1) as data:
            # ---- persistent tiles + loads
            xw0 = data.tile([128, GCP], bf, name="xw0", tag="xw0")
            adjb = data.tile([120, 2 * CHUNK * 120], bf, name="adjb", tag="adjb")
            wg = data.tile([128, 3 * 128], bf, name="wg", tag="wg")
            bg = data.tile([128, 3], f32, name="bg", tag="bg")
            idn = data.tile([128, 128], bf, name="idn", tag="idn")
            wc = data.tile([128, 18 * 64], bf, name="wc", tag="wc")
            bc = data.tile([128, 3], f32, name="bc", tag="bc")
            for t, d in [(xw0, xw0_d), (adjb, adjb_d), (wg, wg_d), (bg, bg_d),
                         (idn, idn_d), (wc, wc_d), (bc, bc_d)]:
                nc.sync.dma_start(t[:], d[:])

            hsT = data.tile([128, GCP], bf, name="hsT", tag="hsT")
            dx = [data.tile([128, GCP], bf, name=f"dx{i}", tag=f"dx{i}") for i in range(3)]
            pracc = data.tile([128, PPC], f32, name="pracc", tag="pracc")

            # =================== GNN ===================
            with (
                tc.tile_pool(name="ps_h", bufs=2, space=bass.MemorySpace.PSUM) as ps_h,
                tc.tile_pool(name="ps_t", bufs=3, space=bass.MemorySpace.PSUM) as ps_t,
                tc.tile_pool(name="ps_d", bufs=3, space=bass.MemorySpace.PSUM) as ps_d,
                tc.tile_pool(name="ha_p", bufs=4) as ha_p,
            ):
                for ly in range(3):
                    srcs = [xw0] + dx[:ly]
                    for j in range(NJ):
                        ph = ps_h.tile([128, JW], f32, name="ph", tag="ph")
                        for si, s in enumerate(srcs):
                            nc.tensor.matmul(
                                ph[:], wg[:, ly * 128:(ly + 1) * 128],
                                s[:, j * JW:(j + 1) * JW],
                                start=(si == 0), stop=(si == len(srcs) - 1))
                        nc.scalar.activation(hsT[:, j * JW:(j + 1) * JW], ph[:],
                                             AFT.Relu, bias=bg[:, ly:ly + 1])
                    for c in range(CHUNK):
                        pt = ps_t.tile([120, 128], f32, name="pt", tag="pt")
                        nc.tensor.matmul(pt[:], hsT[:, c * 120:(c + 1) * 120],
                                         idn[:], start=True, stop=True)
                        ha = ha_p.tile([120, 128], bf, name="ha", tag="ha")
                        nc.scalar.copy(ha[:], pt[:])
                        pd = ps_d.tile([128, 120], f32, name="pd", tag="pd")
                        nc.tensor.matmul(
                            pd[0:64, :], ha[:, 0:64],
                            adjb[:, c * 120:(c + 1) * 120],
                            start=True, stop=True, tile_position=(0, 0))
                        nc.tensor.matmul(
                            pd[64:128, :], ha[:, 64:128],
                            adjb[:, (CHUNK + c) * 120:(CHUNK + c + 1) * 120],
                            start=True, stop=True, tile_position=(0, 64))
                        nc.vector.tensor_copy(dx[ly][:, c * 120:(c + 1) * 120], pd[:])

                # comp_int raw sums: reduce over atoms for xs0 + d1 + d2 + d3
                crs = []
                for si, s in enumerate([xw0] + dx):
                    cr = ha_p.tile([128, 3 * CHUNK], f32, name=f"cr{si}", tag=f"cr{si}")
                    nc.vector.reduce_sum(
                        cr[:], s[:].rearrange("p (c a) -> p c a", a=N_ATOMS),
                        axis=mybir.AxisListType.X)
                    crs.append(cr)
                nc.vector.tensor_add(crs[0][:], crs[0][:], crs[1][:])
                nc.vector.tensor_add(crs[2][:], crs[2][:], crs[3][:])
                nc.vector.tensor_add(crs[0][:], crs[0][:], crs[2][:])
                nc.sync.dma_start(csum_d[:], crs[0][:])

            # =================== CNN ===================
            with (
                tc.tile_pool(name="xb", bufs=1) as xb_pool,
                tc.tile_pool(name="ps_c", bufs=6, space=bass.MemorySpace.PSUM) as ps_c,
                tc.tile_pool(name="scr", bufs=4) as scr_p,
            ):
                xb = [xb_pool.tile([128, 522], bf, name=f"xb{i}", tag=f"xb{i}") for i in range(16)]
                for t in xb:
                    nc.gpsimd.memset(t[:], 0.0)
                for pr in range(NPAIR):
                    base = (pr % 4) * 4
                    cur0, nxt0 = xb[base + 0], xb[base + 1]
                    cur1, nxt1 = xb[base + 2], xb[base + 3]
                    nc.sync.dma_start(cur0[:], xp_d[2 * pr])
                    nc.sync.dma_start(cur1[:], xp_d[2 * pr + 1])
                    for ly in range(3):
                        pc = ps_c.tile([128, 512], f32, name="pc", tag="pc")
                        for p in range(6):
                            w = wc[:, (ly * 6 + p) * 64:(ly * 6 + p + 1) * 64]
                            nc.tensor.matmul(pc[0:64, :], w,
                                             cur0[:, 2 * p:2 * p + 512],
                                             start=(p == 0), stop=(p == 5),
                                             tile_position=(0, 0))
                            nc.tensor.matmul(pc[64:128, :], w,
                                             cur1[:, 2 * p:2 * p + 512],
                                             start=(p == 0), stop=(p == 5),
                                             tile_position=(0, 64))
                        if ly < 2:
                            nc.scalar.activation(nxt0[0:64, 5:517], pc[0:64, :],
                                                 AFT.Relu, bias=bc[0:64, ly:ly + 1])
                            nc.scalar.activation(nxt1[64:128, 4:516], pc[64:128, :],
                                                 AFT.Relu, bias=bc[64:128, ly:ly + 1])
                            nc.sync.dma_start(nxt0[64:128, 4:516], nxt0[0:64, 5:517])
                            nc.sync.dma_start(nxt1[0:64, 5:517], nxt1[64:128, 4:516])
                            cur0, nxt0 = nxt0, cur0
                            cur1, nxt1 = nxt1, cur1
                        else:
                            s = scr_p.tile([128, 512], bf, name="scr", tag="scr")
                            nc.scalar.activation(
                                s[0:64, :], pc[0:64, :], AFT.Relu,
                                bias=bc[0:64, 2:3],
                                accum_out=pracc[0:64, 2 * pr:2 * pr + 1])
                            nc.scalar.activation(
                                s[64:128, :], pc[64:128, :], AFT.Relu,
                                bias=bc[64:128, 2:3],
                                accum_out=pracc[64:128, 2 * pr + 1:2 * pr + 2])
                nc.sync.dma_start(pacc_d[:], pracc[:])

            # =================== GCN ===================
            def gcn(nk, nfull, jn, jw, nshard, af_d, as_d, xs_d, w_d, b_d, out_d):
                with (
                    tc.tile_pool(name="gd", bufs=1) as gd,
                    tc.tile_pool(name="ga", bufs=10) as ga,
                    tc.tile_pool(name="gt", bufs=3) as gt,
                    tc.tile_pool(name="ps_g", bufs=2, space=bass.MemorySpace.PSUM) as ps_g,
                    tc.tile_pool(name="ps_g2", bufs=2, space=bass.MemorySpace.PSUM) as ps_g2,
                    tc.tile_pool(name="ps_t2", bufs=2, space=bass.MemorySpace.PSUM) as ps_t2,
                ):
                    xs = gd.tile([128, nk * 64], dt.bfloat16, name="xs", tag="xs")
                    w = gd.tile([64, 128], dt.bfloat16, name="w", tag="w")
                    b = gd.tile([64, 2], dt.float32, name="b", tag="b")
                    x1T = gd.tile([64, nk * 128], dt.bfloat16, name="x1T", tag="x1T")
                    x1n = gd.tile([128, nk * 64], dt.bfloat16, name="x1n", tag="x1n")
                    nc.sync.dma_start(xs[:], xs_d[:])
                    nc.sync.dma_start(w[:], w_d[:])
                    nc.sync.dma_start(b[:], b_d[:])
                    if nk * 128 > nfull:
                        nc.gpsimd.memset(x1T[:, nfull:nk * 128], 0.0)
                    # layer 1 (full, redundant on all cores)
                    for j in range(jn):
                        pg = ps_g.tile([64, jw], dt.float32, name="pg", tag="pg")
                        for k in range(nk):
                            at = ga.tile([128, jw], dt.bfloat16, name="at", tag="at")
                            nc.sync.dma_start(at[:], af_d[k][:, j * jw:(j + 1) * jw])
                            nc.tensor.matmul(pg[:], xs[:, k * 64:(k + 1) * 64], at[:],
                                             start=(k == 0), stop=(k == nk - 1))
                        p1 = gt.tile([64, jw], dt.bfloat16, name="p1", tag="p1")
                        nc.scalar.copy(p1[:], pg[:])
                        pg2 = ps_g2.tile([64, jw], dt.float32, name="pg2", tag="pg2")
                        nc.tensor.matmul(pg2[:], w[:, 0:64], p1[:],
                                         start=True, stop=True)
                        nc.scalar.activation(x1T[:, j * jw:(j + 1) * jw], pg2[:],
                                             AFT.Relu, bias=b[:, 0:1])
                    # transpose x1T -> x1n (normal layout chunks)
                    for k in range(nk):
                        pt2 = ps_t2.tile([128, 64], dt.float32, name="pt2", tag="pt2")
                        nc.tensor.matmul(pt2[:], x1T[:, k * 128:(k + 1) * 128],
                                         idn[0:64, 0:64], start=True, stop=True)
                        nc.scalar.copy(x1n[:, k * 64:(k + 1) * 64], pt2[:])
                    # layer 2 (only this core's shard of rows)
                    pg = ps_g.tile([64, nshard], dt.float32, name="pgs", tag="pg")
                    for k in range(nk):
                        at = ga.tile([128, nshard], dt.bfloat16, name="ats", tag="ats")
                        nc.sync.dma_start(at[:], as_d[k][:])
                        nc.tensor.matmul(pg[:], x1n[:, k * 64:(k + 1) * 64], at[:],
                                         start=(k == 0), stop=(k == nk - 1))
                    p2 = gt.tile([64, nshard], dt.bfloat16, name="p2", tag="p2")
                    nc.scalar.copy(p2[:], pg[:])
                    pg2 = ps_g2.tile([64, nshard], dt.float32, name="pg2s", tag="pg2")
                    nc.tensor.matmul(pg2[:], w[:, 64:128], p2[:],
                                     start=True, stop=True)
                    x2T = gt.tile([64, nshard], dt.bfloat16, name="x2T", tag="x2T")
                    nc.scalar.activation(x2T[:], pg2[:], AFT.Relu, bias=b[:, 1:2])
                    nc.sync.dma_start(out_d[:], x2T[:])

            gcn(16, N_C, 4, 500, CPC, acf_d, acs_d, xsc_d, wgd_d, bgd_d, xc2_d)
            gcn(12, N_P_PAD, 4, 376, PPC, apf_d, aps_d, xsp_d, wgp_d, bgp_d, xp2_d)

    nc.compile()
    return nc


# ---------------------------------------------------------------- phase 2 ----
def build_phase2():
    nc = bacc.Bacc()
    bf, f32 = dt.bfloat16, dt.float32

    ent_d = nc.dram_tensor("ent", [256, BPC], f32, kind="ExternalInput")
    df_d = nc.dram_tensor("df", [8, 128, BPC], f32, kind="ExternalInput")
    pf_d = nc.dram_tensor("pf", [8, 128, BPC], f32, kind="ExternalInput")
    wd1_d = nc.dram_tensor("wd1", [128, 1024], f32, kind="ExternalInput")
    wd23_d = nc.dram_tensor("wd23", [128, 128], f32, kind="ExternalInput")
    wp1_d = nc.dram_tensor("wp1", [128, 1024], f32, kind="ExternalInput")
    wp23_d = nc.dram_tensor("wp23", [128, 128], f32, kind="ExternalInput")
    wo1_d = nc.dram_tensor("wo1", [128, 768], f32, kind="ExternalInput")
    wo2_d = nc.dram_tensor("wo2", [128, 512], f32, kind="ExternalInput")
    wo3_d = nc.dram_tensor("wo3", [128, 512], f32, kind="ExternalInput")
    wi_d = nc.dram_tensor("wi", [128, 4], f32, kind="ExternalInput")
    bia_d = nc.dram_tensor("bia", [128, 16], f32, kind="ExternalInput")
    # bia columns: 0 bd1, 1 bd2(0:64), 2 bd3(0:64), 3 bp1, 4 bp2, 5 bp3,
    #              6 bo1_m0, 7 bo1_m1, 8 bo2_m0, 9 bo2_m1, 10 bo3_m0, 11 bo3_m1,
    #              12 bint(0:2)
    out_d = nc.dram_tensor("out2", [2, BPC], f32, kind="ExternalOutput")

    with tile.TileContext(nc) as tc:
        with (
            tc.tile_pool(name="data", bufs=1) as data,
            tc.tile_pool(name="ps_a", bufs=2, space=bass.MemorySpace.PSUM) as ps_a,
            tc.tile_pool(name="ps_b", bufs=2, space=bass.MemorySpace.PSUM) as ps_b,
        ):
            wd1 = data.tile([128, 1024], f32, name="wd1", tag="wd1")
            wd23 = data.tile([128, 128], f32, name="wd23", tag="wd23")
            wp1 = data.tile([128, 1024], f32, name="wp1", tag="wp1")
            wp23 = data.tile([128, 128], f32, name="wp23", tag="wp23")
            wo1 = data.tile([128, 768], f32, name="wo1", tag="wo1")
            wo2 = data.tile([128, 512], f32, name="wo2", tag="wo2")
            wo3 = data.tile([128, 512], f32, name="wo3", tag="wo3")
            wi = data.tile([128, 4], f32, name="wi", tag="wi")
            bia = data.tile([128, 16], f32, name="bia", tag="bia")
            for t, d in [(wd1, wd1_d), (wd23, wd23_d), (wp1, wp1_d),
                         (wp23, wp23_d), (wo1, wo1_d), (wo2, wo2_d),
                         (wo3, wo3_d), (wi, wi_d), (bia, bia_d)]:
                nc.sync.dma_start(t[:], d[:])
            C = [data.tile([128, BPC], f32, name=f"C{k}", tag=f"C{k}") for k in range(3)]
            nc.sync.dma_start(C[0][:], ent_d[0:128, :])
            nc.sync.dma_start(C[1][64:128, :], ent_d[128:192, :])
            nc.sync.dma_start(C[2][0:64, :], ent_d[192:256, :])

            def mlp(src_d, w1, w23, bcol, outtile, outslice, tpos):
                f1p = ps_a.tile([128, BPC], f32, name="f1p", tag="psa")
                for k in range(8):
                    xt = data.tile([128, BPC], f32, name=f"xt{bcol}_{k}", tag=f"xt{bcol}_{k}")
                    nc.sync.dma_start(xt[:], src_d[k])
                    nc.tensor.matmul(f1p[:], w1[:, k * 128:(k + 1) * 128], xt[:],
                                     start=(k == 0), stop=(k == 7))
                f1 = data.tile([128, BPC], f32, name=f"f1_{bcol}", tag=f"f1_{bcol}")
                nc.scalar.activation(f1[:], f1p[:], AFT.Relu, bias=bia[:, bcol:bcol + 1])
                f2p = ps_b.tile([128, BPC], f32, name="f2p", tag="psb")
                nc.tensor.matmul(f2p[0:64, :], w23[:, 0:64], f1[:],
                                 start=True, stop=True)
                f2 = data.tile([64, BPC], f32, name=f"f2_{bcol}", tag=f"f2_{bcol}")
                nc.scalar.activation(f2[:], f2p[0:64, :], AFT.Relu,
                                     bias=bia[0:64, bcol + 1:bcol + 2])
                f3p = ps_b.tile([128, BPC], f32, name="f3p", tag="psb")
                lo, hi = (0, 64) if tpos == (0, 0) else (64, 128)
                nc.tensor.matmul(f3p[lo:hi, :], w23[0:64, 64:128], f2[:],
                                 start=True, stop=True, tile_position=tpos)
                nc.scalar.activation(outtile[outslice, :], f3p[lo:hi, :], AFT.Relu,
                                     bias=bia[lo:hi, bcol + 2:bcol + 3])

            mlp(df_d, wd1, wd23, 0, C[1], slice(0, 64), (0, 0))
            mlp(pf_d, wp1, wp23, 3, C[2], slice(64, 128), (0, 64))

            h = C
            for li, (wt, nk, bcol) in enumerate([(wo1, 3, 6), (wo2, 2, 8),
                                                 (wo3, 2, 10)]):
                hn = []
                for m in range(2):
                    hp = ps_a.tile([128, BPC], f32, name=f"hp{li}{m}", tag="psa")
                    for k in range(nk):
                        nc.tensor.matmul(hp[:], wt[:, (k * 2 + m) * 128:
                                                    (k * 2 + m + 1) * 128],
                                         h[k][:], start=(k == 0), stop=(k == nk - 1))
                    ht = data.tile([128, BPC], f32, name=f"h{li}{m}", tag=f"h{li}{m}")
                    nc.scalar.activation(ht[:], hp[:], AFT.Relu,
                                         bias=bia[:, bcol + m:bcol + m + 1])
                    hn.append(ht)
                h = hn
            zp = ps_b.tile([2, BPC], f32, name="zp", tag="psb")
            for k in range(2):
                nc.tensor.matmul(zp[:], wi[:, k * 2:(k + 1) * 2], h[k][:],
                                 start=(k == 0), stop=(k == 1))
            zs = data.tile([2, BPC], f32, name="zs", tag="zs")
            nc.scalar.activation(zs[:], zp[:], AFT.Sigmoid, bias=bia[0:2, 12:13])
            nc.sync.dma_start(out_d[:], zs[:])

    nc.compile()
    return nc


# ------------------------------------------------------------- host prep ----
def _prep_phase1_inputs(I):
    """Returns list of 8 per-core input dicts for phase 1."""
    bf = BF16
    emb_fp = np.asarray(I["embed_fp"], F32)
    compounds = np.asarray(I["compounds"])
    adj = np.asarray(I["adjacencies"], F32)
    W_gnn = np.asarray(I["W_gnn"], F32)
    b_gnn = np.asarray(I["b_gnn"], F32)
    emb_w = np.asarray(I["embed_word"], F32).astype(bf)
    proteins = np.asarray(I["proteins"])
    K_cnn = np.asarray(I["K_cnn"], F32)
    b_cnn = np.asarray(I["b_cnn"], F32)

    # GNN packing
    xg = emb_fp[compounds]                       # [N_C, 40, 64] f32
    wg = np.zeros((128, 3 * 128), F32)
    bg = np.zeros((128, 3), F32)
    for i in range(3):
        wg[0:64, i * 128:i * 128 + 64] = W_gnn[i]
        wg[64:128, i * 128 + 64:i * 128 + 128] = W_gnn[i]
        bg[0:64, i] = b_gnn[i]
        bg[64:128, i] = b_gnn[i]
    wg = wg.astype(bf)
    idn = np.eye(128, dtype=F32).astype(bf)

    # CNN packing
    bands = np.stack([_bands(K_cnn[i]) for i in range(3)])   # [3, 11, 64, 64]
    wcp = np.zeros((128, 18 * 64), F32)
    for i in range(3):
        for p in range(6):
            cb = (i * 6 + p) * 64
            wcp[0:64, cb:cb + 64] = bands[i, 2 * p]
            if 2 * p + 1 < 11:
                wcp[64:128, cb:cb + 64] = bands[i, 2 * p + 1]
    wcp = wcp.astype(bf)
    bcp = np.zeros((128, 3), F32)
    bcp[:, :] = b_cnn[None, :]

    prot_pad = np.zeros((N_P_PAD,) + proteins.shape[1:], proteins.dtype)
    prot_pad[:N_P] = proteins
    xw_all = emb_w[prot_pad]                     # [1504, 512, 64] bf16
    xT = np.ascontiguousarray(xw_all.transpose(0, 2, 1))  # [1504, 64, 512]
    xp_all = np.zeros((N_P_PAD, 128, 522), bf)
    xp_all[:, 0:64, 5:517] = xT
    xp_all[:, 64:128, 4:516] = xT

    # GCN packing
    def gcn_pack(A, Xs, Wl, bl, nk, nfull, npad, shard_lo, shard_n):
        Ap = np.zeros((nk * 128, npad), F32)
        Ap[:nfull, :nfull] = A[:nfull, :nfull]
        Xp = np.zeros((nk * 128, 64), F32)
        Xp[:nfull] = Xs[:nfull]
        af = np.ascontiguousarray(
            Ap.reshape(nk, 128, npad)).astype(bf)
        asd = np.ascontiguousarray(
            Ap[:, shard_lo:shard_lo + shard_n].reshape(nk, 128, shard_n)).astype(bf)
        xs = np.ascontiguousarray(
            Xp.reshape(nk, 128, 64).transpose(1, 0, 2).reshape(128, nk * 64)).astype(bf)
        w = np.concatenate([Wl[0], Wl[1]], axis=1).astype(bf)   # [64, 128]
        b = np.stack([bl[0], bl[1]], axis=1).astype(F32)        # [64, 2]
        return af, asd, xs, w, b

    A_c = np.asarray(I["A_c"], F32)
    A_p = np.asarray(I["A_p"], F32)
    Ap_pad = np.zeros((N_P_PAD, N_P_PAD), F32)
    Ap_pad[:N_P, :N_P] = A_p
    Xs_c = np.asarray(I["Xs_c"], F32)
    Xs_p = np.asarray(I["Xs_p"], F32)
    Xsp_pad = np.zeros((N_P_PAD, 64), F32)
    Xsp_pad[:N_P] = Xs_p

    in_maps = []
    for c in range(NCORES):
        m = {}
        # GNN per-core
        xs_c = xg[c * CPC:(c + 1) * CPC]          # [250, 40, 64]
        xw0 = np.zeros((128, GCP), F32)
        for g in range(2):
            blk = xs_c[g * G:(g + 1) * G].reshape(G * N_ATOMS, 64).T  # [64, 5000]
            xw0[g * 64:(g + 1) * 64, :G * N_ATOMS] = blk
        m["xw0"] = xw0.astype(bf)
        adjb = np.zeros((120, 2 * CHUNK * 120), F32)
        for g in range(2):
            for ch in range(CHUNK):
                for k3 in range(3):
                    ci = g * G + ch * 3 + k3
                    if ch * 3 + k3 < G:
                        cb = (g * CHUNK + ch) * 120 + k3 * 40
                        adjb[k3 * 40:(k3 + 1) * 40, cb:cb + 40] = \
                            adj[c * CPC + ci]
        m["adjb"] = adjb.astype(bf)
        m["wg"] = wg
        m["bg"] = bg
        m["idn"] = idn
        # CNN per-core
        m["xp"] = xp_all[c * PPC:(c + 1) * PPC]
        m["wc"] = wcp
        m["bc"] = bcp
        # GCN per-core
        acf, acs, xsc, wgd, bgd = gcn_pack(
            A_c, Xs_c, np.asarray(I["W_gcn_d"], F32), np.asarray(I["b_gcn_d"], F32),
            16, N_C, N_C, c * CPC, CPC)
        m["acf"], m["acs"], m["xsc"], m["wgd"], m["bgd"] = acf, acs, xsc, wgd, bgd
        apf, aps, xsp, wgp, bgp = gcn_pack(
            Ap_pad, Xsp_pad, np.asarray(I["W_gcn_p"], F32), np.asarray(I["b_gcn_p"], F32),
            12, N_P_PAD, N_P_PAD, c * PPC, PPC)
        m["apf"], m["aps"], m["xsp"], m["wgp"], m["bgp"] = apf, aps, xsp, wgp, bgp
        in_maps.append(m)
    return in_maps


def _prep_phase2_inputs(I, comp_intT, Xc2T, prot_intT, Xp2T):
    bf = BF16
    idx_c = np.asarray(I["idx_c"])
    idx_p = np.asarray(I["idx_p"])
    df = np.asarray(I["drug_feat"], F32)
    pf = np.asarray(I["protein_feat"], F32).astype(F32)

    def pack_w1(W):   # [1024, 128] -> [128, 1024] lhsT chunks
        out = np.zeros((128, 1024), F32)
        for k in range(8):
            out[:, k * 128:(k + 1) * 128] = W[k * 128:(k + 1) * 128, :]
        return out.astype(F32)

    def pack_w23(W2, W3):  # [128, 64], [64, 64] -> [128, 128]
        out = np.zeros((128, 128), F32)
        out[:, 0:64] = W2
        out[0:64, 64:128] = W3
        return out.astype(F32)

    def pack_head(W, nk):  # [nk*128, 256] -> [128, nk*256]
        out = np.zeros((128, nk * 256), F32)
        for k in range(nk):
            for mh in range(2):
                out[:, (k * 2 + mh) * 128:(k * 2 + mh + 1) * 128] = \
                    W[k * 128:(k + 1) * 128, mh * 128:(mh + 1) * 128]
        return out.astype(F32)

    wd1 = pack_w1(np.asarray(I["Wd1"], F32))
    wd23 = pack_w23(np.asarray(I["Wd2"], F32), np.asarray(I["Wd3"], F32))
    wp1 = pack_w1(np.asarray(I["Wp1"], F32))
    wp23 = pack_w23(np.asarray(I["Wp2"], F32), np.asarray(I["Wp3"], F32))
    wo1 = pack_head(np.asarray(I["Wo1"], F32), 3)
    wo2 = pack_head(np.asarray(I["Wo2"], F32), 2)
    wo3 = pack_head(np.asarray(I["Wo3"], F32), 2)
    wi = np.zeros((128, 4), F32)
    Wi = np.asarray(I["W_int"], F32)
    for k in range(2):
        wi[:, k * 2:(k + 1) * 2] = Wi[k * 128:(k + 1) * 128, :]
    wi = wi.astype(F32)
    bia = np.zeros((128, 16), F32)
    bia[:, 0] = np.asarray(I["bd1"], F32)
    bia[0:64, 1] = np.asarray(I["bd2"], F32)
    bia[0:64, 2] = np.asarray(I["bd3"], F32)
    bia[:, 3] = np.asarray(I["bp1"], F32)
    bia[0:64, 4] = np.asarray(I["bp2"], F32)
    bia[64:128, 5] = np.asarray(I["bp3"], F32)
    bo1 = np.asarray(I["bo1"], F32); bia[:, 6] = bo1[0:128]; bia[:, 7] = bo1[128:256]
    bo2 = np.asarray(I["bo2"], F32); bia[:, 8] = bo2[0:128]; bia[:, 9] = bo2[128:256]
    bo3 = np.asarray(I["bo3"], F32); bia[:, 10] = bo3[0:128]; bia[:, 11] = bo3[128:256]
    bia[0:2, 12] = np.asarray(I["b_int"], F32)

    in_maps = []
    for c in range(NCORES):
        ic = idx_c[c * BPC:(c + 1) * BPC]
        ip = idx_p[c * BPC:(c + 1) * BPC]
        ent = np.concatenate([comp_intT[:, ic], Xc2T[:, ic],
                              prot_intT[:, ip], Xp2T[:, ip]], axis=0).astype(F32)
        dfT = np.ascontiguousarray(df[ic].T).reshape(8, 128, BPC)
        pfT = np.ascontiguousarray(pf[ip].T).reshape(8, 128, BPC)
        m = dict(ent=ent, df=dfT, pf=pfT, wd1=wd1, wd23=wd23, wp1=wp1,
                 wp23=wp23, wo1=wo1, wo2=wo2, wo3=wo3, wi=wi, bia=bia)
        in_maps.append(m)
    return in_maps


_CACHE = {}


def _get_kernels():
    if "p1" not in _CACHE:
        _CACHE["p1"] = build_phase1()
        _CACHE["p2"] = build_phase2()
    return _CACHE["p1"], _CACHE["p2"]


def run(inputs, trace=False):
    """Full pipeline. Returns (output [4096, 2] f32, info dict)."""
    I = inputs
    nc1, nc2 = _get_kernels()
    info = {}

    in1 = _prep_phase1_inputs(I)
    r1 = run_bass_kernel_spmd(nc1, in1, core_ids=list(range(NCORES)), trace=trace)
    res1 = r1.results
    if trace:
        info["p1_exec_ns"] = r1.exec_time_ns

    comp_sumT = np.zeros((64, N_C), F32)
    Xc2T = np.zeros((64, N_C), F32)
    prot_sumT = np.zeros((64, N_P_PAD), F32)
    Xp2T = np.zeros((64, N_P_PAD), F32)
    for c in range(NCORES):
        cs = np.asarray(res1[c]["csum"], F32)       # [128, 126]
        comp_sumT[:, c * CPC:c * CPC + G] = cs[0:64, 0:G]
        comp_sumT[:, c * CPC + G:(c + 1) * CPC] = cs[64:128, 0:G]
        Xc2T[:, c * CPC:(c + 1) * CPC] = np.asarray(res1[c]["xc2"], F32)
        pa = np.asarray(res1[c]["pacc"], F32)       # [128, 188]
        ev = np.arange(0, PPC, 2)
        prot_sumT[:, c * PPC + ev] = pa[0:64, ev]
        prot_sumT[:, c * PPC + ev + 1] = pa[64:128, ev + 1]
        Xp2T[:, c * PPC:(c + 1) * PPC] = np.asarray(res1[c]["xp2"], F32)
    comp_intT = comp_sumT / N_ATOMS
    prot_intT = prot_sumT / L

    in2 = _prep_phase2_inputs(I, comp_intT, Xc2T, prot_intT, Xp2T)
    r2 = run_bass_kernel_spmd(nc2, in2, core_ids=list(range(NCORES)), trace=trace)
    res2 = r2.results
    if trace:
        info["p2_exec_ns"] = r2.exec_time_ns

    out = np.zeros((B, 2), F32)
    for c in range(NCORES):
        out[c * BPC:(c + 1) * BPC] = np.asarray(res2[c]["out2"], F32).T
    return out, info


def kernel(**inputs):
    out, _ = run(inputs)
    return out


# revision 12
# speedup vs baseline: 2.2192x; 2.1674x over previous
"""Trainium2 Bass kernel for nn_DeepERA (GNN + CNN + GCN + MLP head), 8-core SPMD.

Self-contained: hardcodes shapes/sharding. Host does index gathers, weight
packing and layout prep; all dense compute runs on the 8 NeuronCores in two
SPMD launches (phase 1: entity embeddings; phase 2: pair MLPs + head).

Layouts (phase 1, per core):
  GNN: xs kept in "d-layout" [128, 5040] bf16 — partition p<64 -> (group 0,
  din=p), p>=64 -> (group 1, din=p-64); column = compound-in-group*40 + atom.
  Layer updates are never materialized: xs_{i+1} = xs0 + sum(deltas) is kept
  as separate delta tensors and accumulated in PSUM by the next matmul.
  CNN: per-protein doubled storage [128, 522] bf16 — partitions 0:64 hold
  x^T with left-pad 5, partitions 64:128 hold x^T with left-pad 4, so one
  [128, 512] rhs read at col offset 2p yields shifts (2p, 2p+1) stacked on
  the contraction dim. 11x11 conv = 6 such K=128 matmuls vs banded-pair
  weight matrices; two proteins run concurrently on PE col-groups via
  tile_position (0,0)/(0,64).
"""
import numpy as np
import ml_dtypes

import concourse.bass as bass
import concourse.bacc as bacc
import concourse.tile as tile
import concourse.mybir as mybir
from concourse.bass_utils import run_bass_kernel_spmd

BF16 = ml_dtypes.bfloat16
F32 = np.float32

DIM = 64
N_C = 2000
N_P = 1500
N_P_PAD = 1504           # 8 * 188
N_ATOMS = 40
L = 512
WIN = 5
B = 4096
NCORES = 8
CPC = N_C // NCORES      # 250 compounds / core
PPC = N_P_PAD // NCORES  # 188 proteins / core
BPC = B // NCORES        # 512 pairs / core
G = CPC // 2             # 125 compounds per partition-group
CHUNK = 42               # 3-compound chunks per group
GCP = CHUNK * 3 * N_ATOMS  # 5040 padded cols per group (5000 real)
NJ = 10
JW = GCP // NJ           # 504
NPAIR = PPC // 2         # 94 protein pairs / core

dt = mybir.dt
AFT = mybir.ActivationFunctionType


def _bands(K):
    """11 banded matrices Band_a[din, dout] = K[a, din - dout + 5]."""
    i, j = np.indices((DIM, DIM))
    bsel = i - j + WIN
    mask = (bsel >= 0) & (bsel < 11)
    out = np.zeros((11, DIM, DIM), np.float32)
    for a in range(11):
        out[a][mask] = K[a][bsel[mask]]
    return out


# ---------------------------------------------------------------- phase 1 ----
def build_phase1():
    nc = bacc.Bacc()
    bf, f32 = dt.bfloat16, dt.float32

    xw0_d = nc.dram_tensor("xw0", [128, GCP], bf, kind="ExternalInput")
    adjb_d = nc.dram_tensor("adjb", [120, 2 * CHUNK * 120], bf, kind="ExternalInput")
    wg_d = nc.dram_tensor("wg", [128, 3 * 128], bf, kind="ExternalInput")
    bg_d = nc.dram_tensor("bg", [128, 3], f32, kind="ExternalInput")
    idn_d = nc.dram_tensor("idn", [128, 128], bf, kind="ExternalInput")
    xp_d = nc.dram_tensor("xp", [PPC, 128, 522], bf, kind="ExternalInput")
    wc_d = nc.dram_tensor("wc", [128, 18 * 64], bf, kind="ExternalInput")
    bc_d = nc.dram_tensor("bc", [128, 3], f32, kind="ExternalInput")
    acf_d = nc.dram_tensor("acf", [16, 128, N_C], bf, kind="ExternalInput")
    acs_d = nc.dram_tensor("acs", [16, 128, CPC], bf, kind="ExternalInput")
    xsc_d = nc.dram_tensor("xsc", [128, 16 * 64], bf, kind="ExternalInput")
    wgd_d = nc.dram_tensor("wgd", [64, 128], bf, kind="ExternalInput")
    bgd_d = nc.dram_tensor("bgd", [64, 2], f32, kind="ExternalInput")
    apf_d = nc.dram_tensor("apf", [12, 128, N_P_PAD], bf, kind="ExternalInput")
    aps_d = nc.dram_tensor("aps", [12, 128, PPC], bf, kind="ExternalInput")
    xsp_d = nc.dram_tensor("xsp", [128, 12 * 64], bf, kind="ExternalInput")
    wgp_d = nc.dram_tensor("wgp", [64, 128], bf, kind="ExternalInput")
    bgp_d = nc.dram_tensor("bgp", [64, 2], f32, kind="ExternalInput")

    csum_d = nc.dram_tensor("csum", [128, 3 * CHUNK], f32, kind="ExternalOutput")
    pacc_d = nc.dram_tensor("pacc", [128, PPC], f32, kind="ExternalOutput")
    xc2_d = nc.dram_tensor("xc2", [64, CPC], bf, kind="ExternalOutput")
    xp2_d = nc.dram_tensor("xp2", [64, PPC], bf, kind="ExternalOutput")

    with tile.TileContext(nc) as tc:
        with tc.tile_pool(name="data", bufs=1) as data:
            # ---- persistent tiles + loads
            xw0 = data.tile([128, GCP], bf, name="xw0", tag="xw0")
            adjb = data.tile([120, 2 * CHUNK * 120], bf, name="adjb", tag="adjb")
            wg = data.tile([128, 3 * 128], bf, name="wg", tag="wg")
            bg = data.tile([128, 3], f32, name="bg", tag="bg")
            idn = data.tile([128, 128], bf, name="idn", tag="idn")
            wc = data.tile([128, 18 * 64], bf, name="wc", tag="wc")
            bc = data.tile([128, 3], f32, name="bc", tag="bc")
            for t, d in [(xw0, xw0_d), (adjb, adjb_d), (wg, wg_d), (bg, bg_d),
                         (idn, idn_d), (wc, wc_d), (bc, bc_d)]:
                nc.sync.dma_start(t[:], d[:])

            hsT = data.tile([128, GCP], bf, name="hsT", tag="hsT")
            dx = [data.tile([128, GCP], bf, name=f"dx{i}", tag=f"dx{i}") for i in range(3)]
            pracc = data.tile([128, PPC], f32, name="pracc", tag="pracc")

            # =================== GNN ===================
            with (
                tc.tile_pool(name="ps_h", bufs=2, space=bass.MemorySpace.PSUM) as ps_h,
                tc.tile_pool(name="ps_t", bufs=3, space=bass.MemorySpace.PSUM) as ps_t,
                tc.tile_pool(name="ps_d", bufs=3, space=bass.MemorySpace.PSUM) as ps_d,
                tc.tile_pool(name="ha_p", bufs=4) as ha_p,
            ):
                for ly in range(3):
                    srcs = [xw0] + dx[:ly]
                    for j in range(NJ):
                        ph = ps_h.tile([128, JW], f32, name="ph", tag="ph")
                        for si, s in enumerate(srcs):
                            nc.tensor.matmul(
                                ph[:], wg[:, ly * 128:(ly + 1) * 128],
                                s[:, j * JW:(j + 1) * JW],
                                start=(si == 0), stop=(si == len(srcs) - 1))
                        nc.scalar.activation(hsT[:, j * JW:(j + 1) * JW], ph[:],
                                             AFT.Relu, bias=bg[:, ly:ly + 1])
                    for c in range(CHUNK):
                        pt = ps_t.tile([120, 128], f32, name="pt", tag="pt")
                        nc.tensor.matmul(pt[:], hsT[:, c * 120:(c + 1) * 120],
                                         idn[:], start=True, stop=True)
                        ha = ha_p.tile([120, 128], bf, name="ha", tag="ha")
                        nc.scalar.copy(ha[:], pt[:])
                        pd = ps_d.tile([128, 120], f32, name="pd", tag="pd")
                        nc.tensor.matmul(
                            pd[0:64, :], ha[:, 0:64],
                            adjb[:, c * 120:(c + 1) * 120],
                            start=True, stop=True, tile_position=(0, 0))
                        nc.tensor.matmul(
                            pd[64:128, :], ha[:, 64:128],
                            adjb[:, (CHUNK + c) * 120:(CHUNK + c + 1) * 120],
                            start=True, stop=True, tile_position=(0, 64))
                        nc.vector.tensor_copy(dx[ly][:, c * 120:(c + 1) * 120], pd[:])

                # comp_int raw sums: reduce over atoms for xs0 + d1 + d2 + d3
                crs = []
                for si, s in enumerate([xw0] + dx):
                    cr = ha_p.tile([128, 3 * CHUNK], f32, name=f"cr{si}", tag=f"cr{si}")
                    nc.vector.reduce_sum(
                        cr[:], s[:].rearrange("p (c a) -> p c a", a=N_ATOMS),
                        axis=mybir.AxisListType.X)
                    crs.append(cr)
                nc.vector.tensor_add(crs[0][:], crs[0][:], crs[1][:])
                nc.vector.tensor_add(crs[2][:], crs[2][:], crs[3][:])
                nc.vector.tensor_add(crs[0][:], crs[0][:], crs[2][:])
                nc.sync.dma_start(csum_d[:], crs[0][:])

            # =================== CNN ===================
            with (
                tc.tile_pool(name="xb", bufs=1) as xb_pool,
                tc.tile_pool(name="ps_c", bufs=6, space=bass.MemorySpace.PSUM) as ps_c,
                tc.tile_pool(name="scr", bufs=4) as scr_p,
            ):
                xb = [xb_pool.tile([128, 522], bf, name=f"xb{i}", tag=f"xb{i}") for i in range(64)]
                for t in xb:
                    nc.gpsimd.memset(t[:], 0.0)
                BLK = 8
                for b0 in range(0, NPAIR, BLK):
                    prs = list(range(b0, min(b0 + BLK, NPAIR)))
                    for pr in prs:
                        s4 = (pr % 16) * 4
                        nc.sync.dma_start(xb[s4][:], xp_d[2 * pr])
                        nc.sync.dma_start(xb[s4 + 2][:], xp_d[2 * pr + 1])
                    for ly in range(3):
                        for pr in prs:
                            s4 = (pr % 16) * 4
                            cur0 = xb[s4 + (ly % 2)]
                            cur1 = xb[s4 + 2 + (ly % 2)]
                            nxt0 = xb[s4 + 1 - (ly % 2)]
                            nxt1 = xb[s4 + 3 - (ly % 2)]
                            pc = ps_c.tile([128, 512], f32, name="pc", tag="pc")
                            for p in range(6):
                                w = wc[:, (ly * 6 + p) * 64:(ly * 6 + p + 1) * 64]
                                nc.tensor.matmul(pc[0:64, :], w,
                                                 cur0[:, 2 * p:2 * p + 512],
                                                 start=(p == 0), stop=(p == 5),
                                                 tile_position=(0, 0))
                                nc.tensor.matmul(pc[64:128, :], w,
                                                 cur1[:, 2 * p:2 * p + 512],
                                                 start=(p == 0), stop=(p == 5),
                                                 tile_position=(0, 64))
                            if ly < 2:
                                nc.scalar.activation(nxt0[0:64, 5:517], pc[0:64, :],
                                                     AFT.Relu, bias=bc[0:64, ly:ly + 1])
                                nc.scalar.activation(nxt1[64:128, 4:516], pc[64:128, :],
                                                     AFT.Relu, bias=bc[64:128, ly:ly + 1])
                                nc.gpsimd.dma_start(nxt0[64:128, 4:516], nxt0[0:64, 5:517])
                                nc.gpsimd.dma_start(nxt1[0:64, 5:517], nxt1[64:128, 4:516])
                            else:
                                s = scr_p.tile([128, 512], bf, name="scr", tag="scr")
                                nc.scalar.activation(
                                    s[0:64, :], pc[0:64, :], AFT.Relu,
                                    bias=bc[0:64, 2:3],
                                    accum_out=pracc[0:64, 2 * pr:2 * pr + 1])
                                nc.scalar.activation(
                                    s[64:128, :], pc[64:128, :], AFT.Relu,
                                    bias=bc[64:128, 2:3],
                                    accum_out=pracc[64:128, 2 * pr + 1:2 * pr + 2])
                nc.sync.dma_start(pacc_d[:], pracc[:])

            # =================== GCN ===================
            def gcn(nk, nfull, jn, jw, nshard, af_d, as_d, xs_d, w_d, b_d, out_d):
                with (
                    tc.tile_pool(name="gd", bufs=1) as gd,
                    tc.tile_pool(name="ga", bufs=10) as ga,
                    tc.tile_pool(name="gt", bufs=3) as gt,
                    tc.tile_pool(name="ps_g", bufs=2, space=bass.MemorySpace.PSUM) as ps_g,
                    tc.tile_pool(name="ps_g2", bufs=2, space=bass.MemorySpace.PSUM) as ps_g2,
                    tc.tile_pool(name="ps_t2", bufs=2, space=bass.MemorySpace.PSUM) as ps_t2,
                ):
                    xs = gd.tile([128, nk * 64], dt.bfloat16, name="xs", tag="xs")
                    w = gd.tile([64, 128], dt.bfloat16, name="w", tag="w")
                    b = gd.tile([64, 2], dt.float32, name="b", tag="b")
                    x1T = gd.tile([64, nk * 128], dt.bfloat16, name="x1T", tag="x1T")
                    x1n = gd.tile([128, nk * 64], dt.bfloat16, name="x1n", tag="x1n")
                    nc.sync.dma_start(xs[:], xs_d[:])
                    nc.sync.dma_start(w[:], w_d[:])
                    nc.sync.dma_start(b[:], b_d[:])
                    if nk * 128 > nfull:
                        nc.gpsimd.memset(x1T[:, nfull:nk * 128], 0.0)
                    # layer 1 (full, redundant on all cores)
                    for j in range(jn):
                        pg = ps_g.tile([64, jw], dt.float32, name="pg", tag="pg")
                        for k in range(nk):
                            at = ga.tile([128, jw], dt.bfloat16, name="at", tag="at")
                            nc.sync.dma_start(at[:], af_d[k][:, j * jw:(j + 1) * jw])
                            nc.tensor.matmul(pg[:], xs[:, k * 64:(k + 1) * 64], at[:],
                                             start=(k == 0), stop=(k == nk - 1))
                        p1 = gt.tile([64, jw], dt.bfloat16, name="p1", tag="p1")
                        nc.scalar.copy(p1[:], pg[:])
                        pg2 = ps_g2.tile([64, jw], dt.float32, name="pg2", tag="pg2")
                        nc.tensor.matmul(pg2[:], w[:, 0:64], p1[:],
                                         start=True, stop=True)
                        nc.scalar.activation(x1T[:, j * jw:(j + 1) * jw], pg2[:],
                                             AFT.Relu, bias=b[:, 0:1])
                    # transpose x1T -> x1n (normal layout chunks)
                    for k in range(nk):
                        pt2 = ps_t2.tile([128, 64], dt.float32, name="pt2", tag="pt2")
                        nc.tensor.matmul(pt2[:], x1T[:, k * 128:(k + 1) * 128],
                                         idn[0:64, 0:64], start=True, stop=True)
                        nc.scalar.copy(x1n[:, k * 64:(k + 1) * 64], pt2[:])
                    # layer 2 (only this core's shard of rows)
                    pg = ps_g.tile([64, nshard], dt.float32, name="pgs", tag="pg")
                    for k in range(nk):
                        at = ga.tile([128, nshard], dt.bfloat16, name="ats", tag="ats")
                        nc.sync.dma_start(at[:], as_d[k][:])
                        nc.tensor.matmul(pg[:], x1n[:, k * 64:(k + 1) * 64], at[:],
                                         start=(k == 0), stop=(k == nk - 1))
                    p2 = gt.tile([64, nshard], dt.bfloat16, name="p2", tag="p2")
                    nc.scalar.copy(p2[:], pg[:])
                    pg2 = ps_g2.tile([64, nshard], dt.float32, name="pg2s", tag="pg2")
                    nc.tensor.matmul(pg2[:], w[:, 64:128], p2[:],
                                     start=True, stop=True)
                    x2T = gt.tile([64, nshard], dt.bfloat16, name="x2T", tag="x2T")
                    nc.scalar.activation(x2T[:], pg2[:], AFT.Relu, bias=b[:, 1:2])
                    nc.sync.dma_start(out_d[:], x2T[:])

            gcn(16, N_C, 4, 500, CPC, acf_d, acs_d, xsc_d, wgd_d, bgd_d, xc2_d)
            gcn(12, N_P_PAD, 4, 376, PPC, apf_d, aps_d, xsp_d, wgp_d, bgp_d, xp2_d)

    nc.compile()
    return nc


# ---------------------------------------------------------------- phase 2 ----
def build_phase2():
    nc = bacc.Bacc()
    bf, f32 = dt.bfloat16, dt.float32

    ent_d = nc.dram_tensor("ent", [256, BPC], f32, kind="ExternalInput")
    df_d = nc.dram_tensor("df", [8, 128, BPC], f32, kind="ExternalInput")
    pf_d = nc.dram_tensor("pf", [8, 128, BPC], f32, kind="ExternalInput")
    wd1_d = nc.dram_tensor("wd1", [128, 1024], f32, kind="ExternalInput")
    wd23_d = nc.dram_tensor("wd23", [128, 128], f32, kind="ExternalInput")
    wp1_d = nc.dram_tensor("wp1", [128, 1024], f32, kind="ExternalInput")
    wp23_d = nc.dram_tensor("wp23", [128, 128], f32, kind="ExternalInput")
    wo1_d = nc.dram_tensor("wo1", [128, 768], f32, kind="ExternalInput")
    wo2_d = nc.dram_tensor("wo2", [128, 512], f32, kind="ExternalInput")
    wo3_d = nc.dram_tensor("wo3", [128, 512], f32, kind="ExternalInput")
    wi_d = nc.dram_tensor("wi", [128, 4], f32, kind="ExternalInput")
    bia_d = nc.dram_tensor("bia", [128, 16], f32, kind="ExternalInput")
    # bia columns: 0 bd1, 1 bd2(0:64), 2 bd3(0:64), 3 bp1, 4 bp2, 5 bp3,
    #              6 bo1_m0, 7 bo1_m1, 8 bo2_m0, 9 bo2_m1, 10 bo3_m0, 11 bo3_m1,
    #              12 bint(0:2)
    out_d = nc.dram_tensor("out2", [2, BPC], f32, kind="ExternalOutput")

    with tile.TileContext(nc) as tc:
        with (
            tc.tile_pool(name="data", bufs=1) as data,
            tc.tile_pool(name="ps_a", bufs=2, space=bass.MemorySpace.PSUM) as ps_a,
            tc.tile_pool(name="ps_b", bufs=2, space=bass.MemorySpace.PSUM) as ps_b,
        ):
            wd1 = data.tile([128, 1024], f32, name="wd1", tag="wd1")
            wd23 = data.tile([128, 128], f32, name="wd23", tag="wd23")
            wp1 = data.tile([128, 1024], f32, name="wp1", tag="wp1")
            wp23 = data.tile([128, 128], f32, name="wp23", tag="wp23")
            wo1 = data.tile([128, 768], f32, name="wo1", tag="wo1")
            wo2 = data.tile([128, 512], f32, name="wo2", tag="wo2")
            wo3 = data.tile([128, 512], f32, name="wo3", tag="wo3")
            wi = data.tile([128, 4], f32, name="wi", tag="wi")
            bia = data.tile([128, 16], f32, name="bia", tag="bia")
            for t, d in [(wd1, wd1_d), (wd23, wd23_d), (wp1, wp1_d),
                         (wp23, wp23_d), (wo1, wo1_d), (wo2, wo2_d),
                         (wo3, wo3_d), (wi, wi_d), (bia, bia_d)]:
                nc.sync.dma_start(t[:], d[:])
            C = [data.tile([128, BPC], f32, name=f"C{k}", tag=f"C{k}") for k in range(3)]
            nc.sync.dma_start(C[0][:], ent_d[0:128, :])
            nc.sync.dma_start(C[1][64:128, :], ent_d[128:192, :])
            nc.sync.dma_start(C[2][0:64, :], ent_d[192:256, :])

            def mlp(src_d, w1, w23, bcol, outtile, outslice, tpos):
                f1p = ps_a.tile([128, BPC], f32, name="f1p", tag="psa")
                for k in range(8):
                    xt = data.tile([128, BPC], f32, name=f"xt{bcol}_{k}", tag=f"xt{bcol}_{k}")
                    nc.sync.dma_start(xt[:], src_d[k])
                    nc.tensor.matmul(f1p[:], w1[:, k * 128:(k + 1) * 128], xt[:],
                                     start=(k == 0), stop=(k == 7))
                f1 = data.tile([128, BPC], f32, name=f"f1_{bcol}", tag=f"f1_{bcol}")
                nc.scalar.activation(f1[:], f1p[:], AFT.Relu, bias=bia[:, bcol:bcol + 1])
                f2p = ps_b.tile([128, BPC], f32, name="f2p", tag="psb")
                nc.tensor.matmul(f2p[0:64, :], w23[:, 0:64], f1[:],
                                 start=True, stop=True)
                f2 = data.tile([64, BPC], f32, name=f"f2_{bcol}", tag=f"f2_{bcol}")
                nc.scalar.activation(f2[:], f2p[0:64, :], AFT.Relu,
                                     bias=bia[0:64, bcol + 1:bcol + 2])
                f3p = ps_b.tile([128, BPC], f32, name="f3p", tag="psb")
                lo, hi = (0, 64) if tpos == (0, 0) else (64, 128)
                nc.tensor.matmul(f3p[lo:hi, :], w23[0:64, 64:128], f2[:],
                                 start=True, stop=True, tile_position=tpos)
                nc.scalar.activation(outtile[outslice, :], f3p[lo:hi, :], AFT.Relu,
                                     bias=bia[lo:hi, bcol + 2:bcol + 3])

            mlp(df_d, wd1, wd23, 0, C[1], slice(0, 64), (0, 0))
            mlp(pf_d, wp1, wp23, 3, C[2], slice(64, 128), (0, 64))

            h = C
            for li, (wt, nk, bcol) in enumerate([(wo1, 3, 6), (wo2, 2, 8),
                                                 (wo3, 2, 10)]):
                hn = []
                for m in range(2):
                    hp = ps_a.tile([128, BPC], f32, name=f"hp{li}{m}", tag="psa")
                    for k in range(nk):
                        nc.tensor.matmul(hp[:], wt[:, (k * 2 + m) * 128:
                                                    (k * 2 + m + 1) * 128],
                                         h[k][:], start=(k == 0), stop=(k == nk - 1))
                    ht = data.tile([128, BPC], f32, name=f"h{li}{m}", tag=f"h{li}{m}")
                    nc.scalar.activation(ht[:], hp[:], AFT.Relu,
                                         bias=bia[:, bcol + m:bcol + m + 1])
                    hn.append(ht)
                h = hn
            zp = ps_b.tile([2, BPC], f32, name="zp", tag="psb")
            for k in range(2):
                nc.tensor.matmul(zp[:], wi[:, k * 2:(k + 1) * 2], h[k][:],
                                 start=(k == 0), stop=(k == 1))
            zs = data.tile([2, BPC], f32, name="zs", tag="zs")
            nc.scalar.activation(zs[:], zp[:], AFT.Sigmoid, bias=bia[0:2, 12:13])
            nc.sync.dma_start(out_d[:], zs[:])

    nc.compile()
    return nc


# ------------------------------------------------------------- host prep ----
def _prep_phase1_inputs(I):
    """Returns list of 8 per-core input dicts for phase 1."""
    bf = BF16
    emb_fp = np.asarray(I["embed_fp"], F32)
    compounds = np.asarray(I["compounds"])
    adj = np.asarray(I["adjacencies"], F32)
    W_gnn = np.asarray(I["W_gnn"], F32)
    b_gnn = np.asarray(I["b_gnn"], F32)
    emb_w = np.asarray(I["embed_word"], F32).astype(bf)
    proteins = np.asarray(I["proteins"])
    K_cnn = np.asarray(I["K_cnn"], F32)
    b_cnn = np.asarray(I["b_cnn"], F32)

    # GNN packing
    xg = emb_fp[compounds]                       # [N_C, 40, 64] f32
    wg = np.zeros((128, 3 * 128), F32)
    bg = np.zeros((128, 3), F32)
    for i in range(3):
        wg[0:64, i * 128:i * 128 + 64] = W_gnn[i]
        wg[64:128, i * 128 + 64:i * 128 + 128] = W_gnn[i]
        bg[0:64, i] = b_gnn[i]
        bg[64:128, i] = b_gnn[i]
    wg = wg.astype(bf)
    idn = np.eye(128, dtype=F32).astype(bf)

    # CNN packing
    bands = np.stack([_bands(K_cnn[i]) for i in range(3)])   # [3, 11, 64, 64]
    wcp = np.zeros((128, 18 * 64), F32)
    for i in range(3):
        for p in range(6):
            cb = (i * 6 + p) * 64
            wcp[0:64, cb:cb + 64] = bands[i, 2 * p]
            if 2 * p + 1 < 11:
                wcp[64:128, cb:cb + 64] = bands[i, 2 * p + 1]
    wcp = wcp.astype(bf)
    bcp = np.zeros((128, 3), F32)
    bcp[:, :] = b_cnn[None, :]

    prot_pad = np.zeros((N_P_PAD,) + proteins.shape[1:], proteins.dtype)
    prot_pad[:N_P] = proteins
    xw_all = emb_w[prot_pad]                     # [1504, 512, 64] bf16
    xT = np.ascontiguousarray(xw_all.transpose(0, 2, 1))  # [1504, 64, 512]
    xp_all = np.zeros((N_P_PAD, 128, 522), bf)
    xp_all[:, 0:64, 5:517] = xT
    xp_all[:, 64:128, 4:516] = xT

    # GCN packing
    def gcn_pack(A, Xs, Wl, bl, nk, nfull, npad, shard_lo, shard_n):
        Ap = np.zeros((nk * 128, npad), F32)
        Ap[:nfull, :nfull] = A[:nfull, :nfull]
        Xp = np.zeros((nk * 128, 64), F32)
        Xp[:nfull] = Xs[:nfull]
        af = np.ascontiguousarray(
            Ap.reshape(nk, 128, npad)).astype(bf)
        asd = np.ascontiguousarray(
            Ap[:, shard_lo:shard_lo + shard_n].reshape(nk, 128, shard_n)).astype(bf)
        xs = np.ascontiguousarray(
            Xp.reshape(nk, 128, 64).transpose(1, 0, 2).reshape(128, nk * 64)).astype(bf)
        w = np.concatenate([Wl[0], Wl[1]], axis=1).astype(bf)   # [64, 128]
        b = np.stack([bl[0], bl[1]], axis=1).astype(F32)        # [64, 2]
        return af, asd, xs, w, b

    A_c = np.asarray(I["A_c"], F32)
    A_p = np.asarray(I["A_p"], F32)
    Ap_pad = np.zeros((N_P_PAD, N_P_PAD), F32)
    Ap_pad[:N_P, :N_P] = A_p
    Xs_c = np.asarray(I["Xs_c"], F32)
    Xs_p = np.asarray(I["Xs_p"], F32)
    Xsp_pad = np.zeros((N_P_PAD, 64), F32)
    Xsp_pad[:N_P] = Xs_p

    in_maps = []
    for c in range(NCORES):
        m = {}
        # GNN per-core
        xs_c = xg[c * CPC:(c + 1) * CPC]          # [250, 40, 64]
        xw0 = np.zeros((128, GCP), F32)
        for g in range(2):
            blk = xs_c[g * G:(g + 1) * G].reshape(G * N_ATOMS, 64).T  # [64, 5000]
            xw0[g * 64:(g + 1) * 64, :G * N_ATOMS] = blk
        m["xw0"] = xw0.astype(bf)
        adjb = np.zeros((120, 2 * CHUNK * 120), F32)
        for g in range(2):
            for ch in range(CHUNK):
                for k3 in range(3):
                    ci = g * G + ch * 3 + k3
                    if ch * 3 + k3 < G:
                        cb = (g * CHUNK + ch) * 120 + k3 * 40
                        adjb[k3 * 40:(k3 + 1) * 40, cb:cb + 40] = \
                            adj[c * CPC + ci]
        m["adjb"] = adjb.astype(bf)
        m["wg"] = wg
        m["bg"] = bg
        m["idn"] = idn
        # CNN per-core
        m["xp"] = xp_all[c * PPC:(c + 1) * PPC]
        m["wc"] = wcp
        m["bc"] = bcp
        # GCN per-core
        acf, acs, xsc, wgd, bgd = gcn_pack(
            A_c, Xs_c, np.asarray(I["W_gcn_d"], F32), np.asarray(I["b_gcn_d"], F32),
            16, N_C, N_C, c * CPC, CPC)
        m["acf"], m["acs"], m["xsc"], m["wgd"], m["bgd"] = acf, acs, xsc, wgd, bgd
        apf, aps, xsp, wgp, bgp = gcn_pack(
            Ap_pad, Xsp_pad, np.asarray(I["W_gcn_p"], F32), np.asarray(I["b_gcn_p"], F32),
            12, N_P_PAD, N_P_PAD, c * PPC, PPC)
        m["apf"], m["aps"], m["xsp"], m["wgp"], m["bgp"] = apf, aps, xsp, wgp, bgp
        in_maps.append(m)
    return in_maps


def _prep_phase2_inputs(I, comp_intT, Xc2T, prot_intT, Xp2T):
    bf = BF16
    idx_c = np.asarray(I["idx_c"])
    idx_p = np.asarray(I["idx_p"])
    df = np.asarray(I["drug_feat"], F32)
    pf = np.asarray(I["protein_feat"], F32).astype(F32)

    def pack_w1(W):   # [1024, 128] -> [128, 1024] lhsT chunks
        out = np.zeros((128, 1024), F32)
        for k in range(8):
            out[:, k * 128:(k + 1) * 128] = W[k * 128:(k + 1) * 128, :]
        return out.astype(F32)

    def pack_w23(W2, W3):  # [128, 64], [64, 64] -> [128, 128]
        out = np.zeros((128, 128), F32)
        out[:, 0:64] = W2
        out[0:64, 64:128] = W3
        return out.astype(F32)

    def pack_head(W, nk):  # [nk*128, 256] -> [128, nk*256]
        out = np.zeros((128, nk * 256), F32)
        for k in range(nk):
            for mh in range(2):
                out[:, (k * 2 + mh) * 128:(k * 2 + mh + 1) * 128] = \
                    W[k * 128:(k + 1) * 128, mh * 128:(mh + 1) * 128]
        return out.astype(F32)

    wd1 = pack_w1(np.asarray(I["Wd1"], F32))
    wd23 = pack_w23(np.asarray(I["Wd2"], F32), np.asarray(I["Wd3"], F32))
    wp1 = pack_w1(np.asarray(I["Wp1"], F32))
    wp23 = pack_w23(np.asarray(I["Wp2"], F32), np.asarray(I["Wp3"], F32))
    wo1 = pack_head(np.asarray(I["Wo1"], F32), 3)
    wo2 = pack_head(np.asarray(I["Wo2"], F32), 2)
    wo3 = pack_head(np.asarray(I["Wo3"], F32), 2)
    wi = np.zeros((128, 4), F32)
    Wi = np.asarray(I["W_int"], F32)
    for k in range(2):
        wi[:, k * 2:(k + 1) * 2] = Wi[k * 128:(k + 1) * 128, :]
    wi = wi.astype(F32)
    bia = np.zeros((128, 16), F32)
    bia[:, 0] = np.asarray(I["bd1"], F32)
    bia[0:64, 1] = np.asarray(I["bd2"], F32)
    bia[0:64, 2] = np.asarray(I["bd3"], F32)
    bia[:, 3] = np.asarray(I["bp1"], F32)
    bia[0:64, 4] = np.asarray(I["bp2"], F32)
    bia[64:128, 5] = np.asarray(I["bp3"], F32)
    bo1 = np.asarray(I["bo1"], F32); bia[:, 6] = bo1[0:128]; bia[:, 7] = bo1[128:256]
    bo2 = np.asarray(I["bo2"], F32); bia[:, 8] = bo2[0:128]; bia[:, 9] = bo2[128:256]
    bo3 = np.asarray(I["bo3"], F32); bia[:, 10] = bo3[0:128]; bia[:, 11] = bo3[128:256]
    bia[0:2, 12] = np.asarray(I["b_int"], F32)

    in_maps = []
    for c in range(NCORES):
        ic = idx_c[c * BPC:(c + 1) * BPC]
        ip = idx_p[c * BPC:(c + 1) * BPC]
        ent = np.concatenate([comp_intT[:, ic], Xc2T[:, ic],
                              prot_intT[:, ip], Xp2T[:, ip]], axis=0).astype(F32)
        dfT = np.ascontiguousarray(df[ic].T).reshape(8, 128, BPC)
        pfT = np.ascontiguousarray(pf[ip].T).reshape(8, 128, BPC)
        m = dict(ent=ent, df=dfT, pf=pfT, wd1=wd1, wd23=wd23, wp1=wp1,
                 wp23=wp23, wo1=wo1, wo2=wo2, wo3=wo3, wi=wi, bia=bia)
        in_maps.append(m)
    return in_maps


_CACHE = {}


def _get_kernels():
    if "p1" not in _CACHE:
        _CACHE["p1"] = build_phase1()
        _CACHE["p2"] = build_phase2()
    return _CACHE["p1"], _CACHE["p2"]


def run(inputs, trace=False):
    """Full pipeline. Returns (output [4096, 2] f32, info dict)."""
    I = inputs
    nc1, nc2 = _get_kernels()
    info = {}

    in1 = _prep_phase1_inputs(I)
    r1 = run_bass_kernel_spmd(nc1, in1, core_ids=list(range(NCORES)), trace=trace)
    res1 = r1.results
    if trace:
        info["p1_exec_ns"] = r1.exec_time_ns

    comp_sumT = np.zeros((64, N_C), F32)
    Xc2T = np.zeros((64, N_C), F32)
    prot_sumT = np.zeros((64, N_P_PAD), F32)
    Xp2T = np.zeros((64, N_P_PAD), F32)
    for c in range(NCORES):
        cs = np.asarray(res1[c]["csum"], F32)       # [128, 126]
        comp_sumT[:, c * CPC:c * CPC + G] = cs[0:64, 0:G]
        comp_sumT[:, c * CPC + G:(c + 1) * CPC] = cs[64:128, 0:G]
        Xc2T[:, c * CPC:(c + 1) * CPC] = np.asarray(res1[c]["xc2"], F32)
        pa = np.asarray(res1[c]["pacc"], F32)       # [128, 188]
        ev = np.arange(0, PPC, 2)
        prot_sumT[:, c * PPC + ev] = pa[0:64, ev]
        prot_sumT[:, c * PPC + ev + 1] = pa[64:128, ev + 1]
        Xp2T[:, c * PPC:(c + 1) * PPC] = np.asarray(res1[c]["xp2"], F32)
    comp_intT = comp_sumT / N_ATOMS
    prot_intT = prot_sumT / L

    in2 = _prep_phase2_inputs(I, comp_intT, Xc2T, prot_intT, Xp2T)
    r2 = run_bass_kernel_spmd(nc2, in2, core_ids=list(range(NCORES)), trace=trace)
    res2 = r2.results
    if trace:
        info["p2_exec_ns"] = r2.exec_time_ns

    out = np.zeros((B, 2), F32)
    for c in range(NCORES):
        out[c * BPC:(c + 1) * BPC] = np.asarray(res2[c]["out2"], F32).T
    return out, info


def kernel(**inputs):
    out, _ = run(inputs)
    return out


# revision 13
# speedup vs baseline: 2.3777x; 1.0714x over previous
"""Trainium2 Bass kernel for nn_DeepERA (GNN + CNN + GCN + MLP head), 8-core SPMD.

Self-contained: hardcodes shapes/sharding. Host does index gathers, weight
packing and layout prep; all dense compute runs on the 8 NeuronCores in two
SPMD launches (phase 1: entity embeddings; phase 2: pair MLPs + head).

Layouts (phase 1, per core):
  GNN: xs kept in "d-layout" [128, 5040] bf16 — partition p<64 -> (group 0,
  din=p), p>=64 -> (group 1, din=p-64); column = compound-in-group*40 + atom.
  Layer updates are never materialized: xs_{i+1} = xs0 + sum(deltas) is kept
  as separate delta tensors and accumulated in PSUM by the next matmul.
  CNN: per-protein doubled storage [128, 522] bf16 — partitions 0:64 hold
  x^T with left-pad 5, partitions 64:128 hold x^T with left-pad 4, so one
  [128, 512] rhs read at col offset 2p yields shifts (2p, 2p+1) stacked on
  the contraction dim. 11x11 conv = 6 such K=128 matmuls vs banded-pair
  weight matrices; two proteins run concurrently on PE col-groups via
  tile_position (0,0)/(0,64).
"""
import numpy as np
import ml_dtypes

import concourse.bass as bass
import concourse.bacc as bacc
import concourse.tile as tile
import concourse.mybir as mybir
from concourse.bass_utils import run_bass_kernel_spmd

BF16 = ml_dtypes.bfloat16
F32 = np.float32

DIM = 64
N_C = 2000
N_P = 1500
N_P_PAD = 1504           # 8 * 188
N_ATOMS = 40
L = 512
WIN = 5
B = 4096
NCORES = 8
CPC = N_C // NCORES      # 250 compounds / core
PPC = N_P_PAD // NCORES  # 188 proteins / core
BPC = B // NCORES        # 512 pairs / core
G = CPC // 2             # 125 compounds per partition-group
CHUNK = 42               # 3-compound chunks per group
GCP = CHUNK * 3 * N_ATOMS  # 5040 padded cols per group (5000 real)
NJ = 10
JW = GCP // NJ           # 504
NPAIR = PPC // 2         # 94 protein pairs / core

dt = mybir.dt
AFT = mybir.ActivationFunctionType


def _bands(K):
    """11 banded matrices Band_a[din, dout] = K[a, din - dout + 5]."""
    i, j = np.indices((DIM, DIM))
    bsel = i - j + WIN
    mask = (bsel >= 0) & (bsel < 11)
    out = np.zeros((11, DIM, DIM), np.float32)
    for a in range(11):
        out[a][mask] = K[a][bsel[mask]]
    return out


# ---------------------------------------------------------------- phase 1 ----
def build_phase1():
    nc = bacc.Bacc()
    bf, f32 = dt.bfloat16, dt.float32

    xw0_d = nc.dram_tensor("xw0", [128, GCP], bf, kind="ExternalInput")
    adjb_d = nc.dram_tensor("adjb", [120, 2 * CHUNK * 120], bf, kind="ExternalInput")
    wg_d = nc.dram_tensor("wg", [128, 3 * 128], bf, kind="ExternalInput")
    bg_d = nc.dram_tensor("bg", [128, 3], f32, kind="ExternalInput")
    idn_d = nc.dram_tensor("idn", [128, 128], bf, kind="ExternalInput")
    xp_d = nc.dram_tensor("xp", [PPC, 128, 522], bf, kind="ExternalInput")
    wc_d = nc.dram_tensor("wc", [128, 18 * 64], bf, kind="ExternalInput")
    bc_d = nc.dram_tensor("bc", [128, 3], f32, kind="ExternalInput")
    acf_d = nc.dram_tensor("acf", [16, 128, N_C], bf, kind="ExternalInput")
    acs_d = nc.dram_tensor("acs", [16, 128, CPC], bf, kind="ExternalInput")
    xsc_d = nc.dram_tensor("xsc", [128, 16 * 64], bf, kind="ExternalInput")
    wgd_d = nc.dram_tensor("wgd", [64, 128], bf, kind="ExternalInput")
    bgd_d = nc.dram_tensor("bgd", [64, 2], f32, kind="ExternalInput")
    apf_d = nc.dram_tensor("apf", [12, 128, N_P_PAD], bf, kind="ExternalInput")
    aps_d = nc.dram_tensor("aps", [12, 128, PPC], bf, kind="ExternalInput")
    xsp_d = nc.dram_tensor("xsp", [128, 12 * 64], bf, kind="ExternalInput")
    wgp_d = nc.dram_tensor("wgp", [64, 128], bf, kind="ExternalInput")
    bgp_d = nc.dram_tensor("bgp", [64, 2], f32, kind="ExternalInput")

    csum_d = nc.dram_tensor("csum", [128, 3 * CHUNK], f32, kind="ExternalOutput")
    pacc_d = nc.dram_tensor("pacc", [128, PPC], f32, kind="ExternalOutput")
    xc2_d = nc.dram_tensor("xc2", [64, CPC], bf, kind="ExternalOutput")
    xp2_d = nc.dram_tensor("xp2", [64, PPC], bf, kind="ExternalOutput")

    with tile.TileContext(nc) as tc:
        with tc.tile_pool(name="data", bufs=1) as data:
            # ---- persistent tiles + loads
            xw0 = data.tile([128, GCP], bf, name="xw0", tag="xw0")
            adjb = data.tile([120, 2 * CHUNK * 120], bf, name="adjb", tag="adjb")
            wg = data.tile([128, 3 * 128], bf, name="wg", tag="wg")
            bg = data.tile([128, 3], f32, name="bg", tag="bg")
            idn = data.tile([128, 128], bf, name="idn", tag="idn")
            wc = data.tile([128, 18 * 64], bf, name="wc", tag="wc")
            bc = data.tile([128, 3], f32, name="bc", tag="bc")
            for t, d in [(xw0, xw0_d), (adjb, adjb_d), (wg, wg_d), (bg, bg_d),
                         (idn, idn_d), (wc, wc_d), (bc, bc_d)]:
                nc.sync.dma_start(t[:], d[:])

            hsT = data.tile([128, GCP], bf, name="hsT", tag="hsT")
            dx = [data.tile([128, GCP], bf, name=f"dx{i}", tag=f"dx{i}") for i in range(3)]
            pracc = data.tile([128, PPC], f32, name="pracc", tag="pracc")

            # =================== CNN ===================
            with (
                tc.tile_pool(name="xb", bufs=1) as xb_pool,
                tc.tile_pool(name="ps_c", bufs=6, space=bass.MemorySpace.PSUM) as ps_c,
                tc.tile_pool(name="scr", bufs=4) as scr_p,
            ):
                xb = [xb_pool.tile([128, 522], bf, name=f"xb{i}", tag=f"xb{i}") for i in range(64)]
                for t in xb:
                    nc.gpsimd.memset(t[:], 0.0)
                BLK = 8
                for b0 in range(0, NPAIR, BLK):
                    prs = list(range(b0, min(b0 + BLK, NPAIR)))
                    for pr in prs:
                        s4 = (pr % 16) * 4
                        nc.sync.dma_start(xb[s4][:], xp_d[2 * pr])
                        nc.sync.dma_start(xb[s4 + 2][:], xp_d[2 * pr + 1])
                    for ly in range(3):
                        for pr in prs:
                            s4 = (pr % 16) * 4
                            cur0 = xb[s4 + (ly % 2)]
                            cur1 = xb[s4 + 2 + (ly % 2)]
                            nxt0 = xb[s4 + 1 - (ly % 2)]
                            nxt1 = xb[s4 + 3 - (ly % 2)]
                            pc = ps_c.tile([128, 512], f32, name="pc", tag="pc")
                            for p in range(6):
                                w = wc[:, (ly * 6 + p) * 64:(ly * 6 + p + 1) * 64]
                                nc.tensor.matmul(pc[0:64, :], w,
                                                 cur0[:, 2 * p:2 * p + 512],
                                                 start=(p == 0), stop=(p == 5),
                                                 tile_position=(0, 0))
                                nc.tensor.matmul(pc[64:128, :], w,
                                                 cur1[:, 2 * p:2 * p + 512],
                                                 start=(p == 0), stop=(p == 5),
                                                 tile_position=(0, 64))
                            if ly < 2:
                                nc.scalar.activation(nxt0[0:64, 5:517], pc[0:64, :],
                                                     AFT.Relu, bias=bc[0:64, ly:ly + 1])
                                nc.vector.tensor_scalar(
                                    nxt1[64:128, 4:516], pc[64:128, :],
                                    bc[64:128, ly:ly + 1], 0.0,
                                    op0=mybir.AluOpType.add, op1=mybir.AluOpType.max)
                                nc.gpsimd.dma_start(nxt0[64:128, 4:516], nxt0[0:64, 5:517])
                                nc.gpsimd.dma_start(nxt1[0:64, 5:517], nxt1[64:128, 4:516])
                            else:
                                s = scr_p.tile([128, 512], bf, name="scr", tag="scr")
                                nc.scalar.activation(
                                    s[0:64, :], pc[0:64, :], AFT.Relu,
                                    bias=bc[0:64, 2:3],
                                    accum_out=pracc[0:64, 2 * pr:2 * pr + 1])
                                nc.scalar.activation(
                                    s[64:128, :], pc[64:128, :], AFT.Relu,
                                    bias=bc[64:128, 2:3],
                                    accum_out=pracc[64:128, 2 * pr + 1:2 * pr + 2])
                nc.sync.dma_start(pacc_d[:], pracc[:])

            # =================== GNN ===================
            with (
                tc.tile_pool(name="ps_h", bufs=2, space=bass.MemorySpace.PSUM) as ps_h,
                tc.tile_pool(name="ps_t", bufs=3, space=bass.MemorySpace.PSUM) as ps_t,
                tc.tile_pool(name="ps_d", bufs=3, space=bass.MemorySpace.PSUM) as ps_d,
                tc.tile_pool(name="ha_p", bufs=4) as ha_p,
            ):
                for ly in range(3):
                    srcs = [xw0] + dx[:ly]
                    for j in range(NJ):
                        ph = ps_h.tile([128, JW], f32, name="ph", tag="ph")
                        for si, s in enumerate(srcs):
                            nc.tensor.matmul(
                                ph[:], wg[:, ly * 128:(ly + 1) * 128],
                                s[:, j * JW:(j + 1) * JW],
                                start=(si == 0), stop=(si == len(srcs) - 1))
                        nc.scalar.activation(hsT[:, j * JW:(j + 1) * JW], ph[:],
                                             AFT.Relu, bias=bg[:, ly:ly + 1])
                    for c in range(CHUNK):
                        pt = ps_t.tile([120, 128], f32, name="pt", tag="pt")
                        nc.tensor.matmul(pt[:], hsT[:, c * 120:(c + 1) * 120],
                                         idn[:], start=True, stop=True)
                        ha = ha_p.tile([120, 128], bf, name="ha", tag="ha")
                        nc.scalar.copy(ha[:], pt[:])
                        pd = ps_d.tile([128, 120], f32, name="pd", tag="pd")
                        nc.tensor.matmul(
                            pd[0:64, :], ha[:, 0:64],
                            adjb[:, c * 120:(c + 1) * 120],
                            start=True, stop=True, tile_position=(0, 0))
                        nc.tensor.matmul(
                            pd[64:128, :], ha[:, 64:128],
                            adjb[:, (CHUNK + c) * 120:(CHUNK + c + 1) * 120],
                            start=True, stop=True, tile_position=(0, 64))
                        nc.vector.tensor_copy(dx[ly][:, c * 120:(c + 1) * 120], pd[:])

                # comp_int raw sums: reduce over atoms for xs0 + d1 + d2 + d3
                crs = []
                for si, s in enumerate([xw0] + dx):
                    cr = ha_p.tile([128, 3 * CHUNK], f32, name=f"cr{si}", tag=f"cr{si}")
                    nc.vector.reduce_sum(
                        cr[:], s[:].rearrange("p (c a) -> p c a", a=N_ATOMS),
                        axis=mybir.AxisListType.X)
                    crs.append(cr)
                nc.vector.tensor_add(crs[0][:], crs[0][:], crs[1][:])
                nc.vector.tensor_add(crs[2][:], crs[2][:], crs[3][:])
                nc.vector.tensor_add(crs[0][:], crs[0][:], crs[2][:])
                nc.sync.dma_start(csum_d[:], crs[0][:])

            # =================== GCN ===================
            def gcn(nk, nfull, jn, jw, nshard, af_d, as_d, xs_d, w_d, b_d, out_d):
                with (
                    tc.tile_pool(name="gd", bufs=1) as gd,
                    tc.tile_pool(name="ga", bufs=10) as ga,
                    tc.tile_pool(name="gt", bufs=3) as gt,
                    tc.tile_pool(name="ps_g", bufs=2, space=bass.MemorySpace.PSUM) as ps_g,
                    tc.tile_pool(name="ps_g2", bufs=2, space=bass.MemorySpace.PSUM) as ps_g2,
                    tc.tile_pool(name="ps_t2", bufs=2, space=bass.MemorySpace.PSUM) as ps_t2,
                ):
                    xs = gd.tile([128, nk * 64], dt.bfloat16, name="xs", tag="xs")
                    w = gd.tile([64, 128], dt.bfloat16, name="w", tag="w")
                    b = gd.tile([64, 2], dt.float32, name="b", tag="b")
                    x1T = gd.tile([64, nk * 128], dt.bfloat16, name="x1T", tag="x1T")
                    x1n = gd.tile([128, nk * 64], dt.bfloat16, name="x1n", tag="x1n")
                    nc.sync.dma_start(xs[:], xs_d[:])
                    nc.sync.dma_start(w[:], w_d[:])
                    nc.sync.dma_start(b[:], b_d[:])
                    if nk * 128 > nfull:
                        nc.gpsimd.memset(x1T[:, nfull:nk * 128], 0.0)
                    # layer 1 (full, redundant on all cores)
                    for j in range(jn):
                        pg = ps_g.tile([64, jw], dt.float32, name="pg", tag="pg")
                        for k in range(nk):
                            at = ga.tile([128, jw], dt.bfloat16, name="at", tag="at")
                            nc.sync.dma_start(at[:], af_d[k][:, j * jw:(j + 1) * jw])
                            nc.tensor.matmul(pg[:], xs[:, k * 64:(k + 1) * 64], at[:],
                                             start=(k == 0), stop=(k == nk - 1))
                        p1 = gt.tile([64, jw], dt.bfloat16, name="p1", tag="p1")
                        nc.scalar.copy(p1[:], pg[:])
                        pg2 = ps_g2.tile([64, jw], dt.float32, name="pg2", tag="pg2")
                        nc.tensor.matmul(pg2[:], w[:, 0:64], p1[:],
                                         start=True, stop=True)
                        nc.scalar.activation(x1T[:, j * jw:(j + 1) * jw], pg2[:],
                                             AFT.Relu, bias=b[:, 0:1])
                    # transpose x1T -> x1n (normal layout chunks)
                    for k in range(nk):
                        pt2 = ps_t2.tile([128, 64], dt.float32, name="pt2", tag="pt2")
                        nc.tensor.matmul(pt2[:], x1T[:, k * 128:(k + 1) * 128],
                                         idn[0:64, 0:64], start=True, stop=True)
                        nc.scalar.copy(x1n[:, k * 64:(k + 1) * 64], pt2[:])
                    # layer 2 (only this core's shard of rows)
                    pg = ps_g.tile([64, nshard], dt.float32, name="pgs", tag="pg")
                    for k in range(nk):
                        at = ga.tile([128, nshard], dt.bfloat16, name="ats", tag="ats")
                        nc.sync.dma_start(at[:], as_d[k][:])
                        nc.tensor.matmul(pg[:], x1n[:, k * 64:(k + 1) * 64], at[:],
                                         start=(k == 0), stop=(k == nk - 1))
                    p2 = gt.tile([64, nshard], dt.bfloat16, name="p2", tag="p2")
                    nc.scalar.copy(p2[:], pg[:])
                    pg2 = ps_g2.tile([64, nshard], dt.float32, name="pg2s", tag="pg2")
                    nc.tensor.matmul(pg2[:], w[:, 64:128], p2[:],
                                     start=True, stop=True)
                    x2T = gt.tile([64, nshard], dt.bfloat16, name="x2T", tag="x2T")
                    nc.scalar.activation(x2T[:], pg2[:], AFT.Relu, bias=b[:, 1:2])
                    nc.sync.dma_start(out_d[:], x2T[:])

            gcn(16, N_C, 4, 500, CPC, acf_d, acs_d, xsc_d, wgd_d, bgd_d, xc2_d)
            gcn(12, N_P_PAD, 4, 376, PPC, apf_d, aps_d, xsp_d, wgp_d, bgp_d, xp2_d)

    nc.compile()
    return nc


# ---------------------------------------------------------------- phase 2 ----
def build_phase2():
    nc = bacc.Bacc()
    bf, f32 = dt.bfloat16, dt.float32

    ent_d = nc.dram_tensor("ent", [256, BPC], f32, kind="ExternalInput")
    df_d = nc.dram_tensor("df", [8, 128, BPC], f32, kind="ExternalInput")
    pf_d = nc.dram_tensor("pf", [8, 128, BPC], f32, kind="ExternalInput")
    wd1_d = nc.dram_tensor("wd1", [128, 1024], f32, kind="ExternalInput")
    wd23_d = nc.dram_tensor("wd23", [128, 128], f32, kind="ExternalInput")
    wp1_d = nc.dram_tensor("wp1", [128, 1024], f32, kind="ExternalInput")
    wp23_d = nc.dram_tensor("wp23", [128, 128], f32, kind="ExternalInput")
    wo1_d = nc.dram_tensor("wo1", [128, 768], f32, kind="ExternalInput")
    wo2_d = nc.dram_tensor("wo2", [128, 512], f32, kind="ExternalInput")
    wo3_d = nc.dram_tensor("wo3", [128, 512], f32, kind="ExternalInput")
    wi_d = nc.dram_tensor("wi", [128, 4], f32, kind="ExternalInput")
    bia_d = nc.dram_tensor("bia", [128, 16], f32, kind="ExternalInput")
    # bia columns: 0 bd1, 1 bd2(0:64), 2 bd3(0:64), 3 bp1, 4 bp2, 5 bp3,
    #              6 bo1_m0, 7 bo1_m1, 8 bo2_m0, 9 bo2_m1, 10 bo3_m0, 11 bo3_m1,
    #              12 bint(0:2)
    out_d = nc.dram_tensor("out2", [2, BPC], f32, kind="ExternalOutput")

    with tile.TileContext(nc) as tc:
        with (
            tc.tile_pool(name="data", bufs=1) as data,
            tc.tile_pool(name="ps_a", bufs=2, space=bass.MemorySpace.PSUM) as ps_a,
            tc.tile_pool(name="ps_b", bufs=2, space=bass.MemorySpace.PSUM) as ps_b,
        ):
            wd1 = data.tile([128, 1024], f32, name="wd1", tag="wd1")
            wd23 = data.tile([128, 128], f32, name="wd23", tag="wd23")
            wp1 = data.tile([128, 1024], f32, name="wp1", tag="wp1")
            wp23 = data.tile([128, 128], f32, name="wp23", tag="wp23")
            wo1 = data.tile([128, 768], f32, name="wo1", tag="wo1")
            wo2 = data.tile([128, 512], f32, name="wo2", tag="wo2")
            wo3 = data.tile([128, 512], f32, name="wo3", tag="wo3")
            wi = data.tile([128, 4], f32, name="wi", tag="wi")
            bia = data.tile([128, 16], f32, name="bia", tag="bia")
            for t, d in [(wd1, wd1_d), (wd23, wd23_d), (wp1, wp1_d),
                         (wp23, wp23_d), (wo1, wo1_d), (wo2, wo2_d),
                         (wo3, wo3_d), (wi, wi_d), (bia, bia_d)]:
                nc.sync.dma_start(t[:], d[:])
            C = [data.tile([128, BPC], f32, name=f"C{k}", tag=f"C{k}") for k in range(3)]
            nc.sync.dma_start(C[0][:], ent_d[0:128, :])
            nc.sync.dma_start(C[1][64:128, :], ent_d[128:192, :])
            nc.sync.dma_start(C[2][0:64, :], ent_d[192:256, :])

            def mlp(src_d, w1, w23, bcol, outtile, outslice, tpos):
                f1p = ps_a.tile([128, BPC], f32, name="f1p", tag="psa")
                for k in range(8):
                    xt = data.tile([128, BPC], f32, name=f"xt{bcol}_{k}", tag=f"xt{bcol}_{k}")
                    nc.sync.dma_start(xt[:], src_d[k])
                    nc.tensor.matmul(f1p[:], w1[:, k * 128:(k + 1) * 128], xt[:],
                                     start=(k == 0), stop=(k == 7))
                f1 = data.tile([128, BPC], f32, name=f"f1_{bcol}", tag=f"f1_{bcol}")
                nc.scalar.activation(f1[:], f1p[:], AFT.Relu, bias=bia[:, bcol:bcol + 1])
                f2p = ps_b.tile([128, BPC], f32, name="f2p", tag="psb")
                nc.tensor.matmul(f2p[0:64, :], w23[:, 0:64], f1[:],
                                 start=True, stop=True)
                f2 = data.tile([64, BPC], f32, name=f"f2_{bcol}", tag=f"f2_{bcol}")
                nc.scalar.activation(f2[:], f2p[0:64, :], AFT.Relu,
                                     bias=bia[0:64, bcol + 1:bcol + 2])
                f3p = ps_b.tile([128, BPC], f32, name="f3p", tag="psb")
                lo, hi = (0, 64) if tpos == (0, 0) else (64, 128)
                nc.tensor.matmul(f3p[lo:hi, :], w23[0:64, 64:128], f2[:],
                                 start=True, stop=True, tile_position=tpos)
                nc.scalar.activation(outtile[outslice, :], f3p[lo:hi, :], AFT.Relu,
                                     bias=bia[lo:hi, bcol + 2:bcol + 3])

            mlp(df_d, wd1, wd23, 0, C[1], slice(0, 64), (0, 0))
            mlp(pf_d, wp1, wp23, 3, C[2], slice(64, 128), (0, 64))

            h = C
            for li, (wt, nk, bcol) in enumerate([(wo1, 3, 6), (wo2, 2, 8),
                                                 (wo3, 2, 10)]):
                hn = []
                for m in range(2):
                    hp = ps_a.tile([128, BPC], f32, name=f"hp{li}{m}", tag="psa")
                    for k in range(nk):
                        nc.tensor.matmul(hp[:], wt[:, (k * 2 + m) * 128:
                                                    (k * 2 + m + 1) * 128],
                                         h[k][:], start=(k == 0), stop=(k == nk - 1))
                    ht = data.tile([128, BPC], f32, name=f"h{li}{m}", tag=f"h{li}{m}")
                    nc.scalar.activation(ht[:], hp[:], AFT.Relu,
                                         bias=bia[:, bcol + m:bcol + m + 1])
                    hn.append(ht)
                h = hn
            zp = ps_b.tile([2, BPC], f32, name="zp", tag="psb")
            for k in range(2):
                nc.tensor.matmul(zp[:], wi[:, k * 2:(k + 1) * 2], h[k][:],
                                 start=(k == 0), stop=(k == 1))
            zs = data.tile([2, BPC], f32, name="zs", tag="zs")
            nc.scalar.activation(zs[:], zp[:], AFT.Sigmoid, bias=bia[0:2, 12:13])
            nc.sync.dma_start(out_d[:], zs[:])

    nc.compile()
    return nc


# ------------------------------------------------------------- host prep ----
def _prep_phase1_inputs(I):
    """Returns list of 8 per-core input dicts for phase 1."""
    bf = BF16
    emb_fp = np.asarray(I["embed_fp"], F32)
    compounds = np.asarray(I["compounds"])
    adj = np.asarray(I["adjacencies"], F32)
    W_gnn = np.asarray(I["W_gnn"], F32)
    b_gnn = np.asarray(I["b_gnn"], F32)
    emb_w = np.asarray(I["embed_word"], F32).astype(bf)
    proteins = np.asarray(I["proteins"])
    K_cnn = np.asarray(I["K_cnn"], F32)
    b_cnn = np.asarray(I["b_cnn"], F32)

    # GNN packing
    xg = emb_fp[compounds]                       # [N_C, 40, 64] f32
    wg = np.zeros((128, 3 * 128), F32)
    bg = np.zeros((128, 3), F32)
    for i in range(3):
        wg[0:64, i * 128:i * 128 + 64] = W_gnn[i]
        wg[64:128, i * 128 + 64:i * 128 + 128] = W_gnn[i]
        bg[0:64, i] = b_gnn[i]
        bg[64:128, i] = b_gnn[i]
    wg = wg.astype(bf)
    idn = np.eye(128, dtype=F32).astype(bf)

    # CNN packing
    bands = np.stack([_bands(K_cnn[i]) for i in range(3)])   # [3, 11, 64, 64]
    wcp = np.zeros((128, 18 * 64), F32)
    for i in range(3):
        for p in range(6):
            cb = (i * 6 + p) * 64
            wcp[0:64, cb:cb + 64] = bands[i, 2 * p]
            if 2 * p + 1 < 11:
                wcp[64:128, cb:cb + 64] = bands[i, 2 * p + 1]
    wcp = wcp.astype(bf)
    bcp = np.zeros((128, 3), F32)
    bcp[:, :] = b_cnn[None, :]

    prot_pad = np.zeros((N_P_PAD,) + proteins.shape[1:], proteins.dtype)
    prot_pad[:N_P] = proteins
    xw_all = emb_w[prot_pad]                     # [1504, 512, 64] bf16
    xT = np.ascontiguousarray(xw_all.transpose(0, 2, 1))  # [1504, 64, 512]
    xp_all = np.zeros((N_P_PAD, 128, 522), bf)
    xp_all[:, 0:64, 5:517] = xT
    xp_all[:, 64:128, 4:516] = xT

    # GCN packing
    def gcn_pack(A, Xs, Wl, bl, nk, nfull, npad, shard_lo, shard_n):
        Ap = np.zeros((nk * 128, npad), F32)
        Ap[:nfull, :nfull] = A[:nfull, :nfull]
        Xp = np.zeros((nk * 128, 64), F32)
        Xp[:nfull] = Xs[:nfull]
        af = np.ascontiguousarray(
            Ap.reshape(nk, 128, npad)).astype(bf)
        asd = np.ascontiguousarray(
            Ap[:, shard_lo:shard_lo + shard_n].reshape(nk, 128, shard_n)).astype(bf)
        xs = np.ascontiguousarray(
            Xp.reshape(nk, 128, 64).transpose(1, 0, 2).reshape(128, nk * 64)).astype(bf)
        w = np.concatenate([Wl[0], Wl[1]], axis=1).astype(bf)   # [64, 128]
        b = np.stack([bl[0], bl[1]], axis=1).astype(F32)        # [64, 2]
        return af, asd, xs, w, b

    A_c = np.asarray(I["A_c"], F32)
    A_p = np.asarray(I["A_p"], F32)
    Ap_pad = np.zeros((N_P_PAD, N_P_PAD), F32)
    Ap_pad[:N_P, :N_P] = A_p
    Xs_c = np.asarray(I["Xs_c"], F32)
    Xs_p = np.asarray(I["Xs_p"], F32)
    Xsp_pad = np.zeros((N_P_PAD, 64), F32)
    Xsp_pad[:N_P] = Xs_p

    in_maps = []
    for c in range(NCORES):
        m = {}
        # GNN per-core
        xs_c = xg[c * CPC:(c + 1) * CPC]          # [250, 40, 64]
        xw0 = np.zeros((128, GCP), F32)
        for g in range(2):
            blk = xs_c[g * G:(g + 1) * G].reshape(G * N_ATOMS, 64).T  # [64, 5000]
            xw0[g * 64:(g + 1) * 64, :G * N_ATOMS] = blk
        m["xw0"] = xw0.astype(bf)
        adjb = np.zeros((120, 2 * CHUNK * 120), F32)
        for g in range(2):
            for ch in range(CHUNK):
                for k3 in range(3):
                    ci = g * G + ch * 3 + k3
                    if ch * 3 + k3 < G:
                        cb = (g * CHUNK + ch) * 120 + k3 * 40
                        adjb[k3 * 40:(k3 + 1) * 40, cb:cb + 40] = \
                            adj[c * CPC + ci]
        m["adjb"] = adjb.astype(bf)
        m["wg"] = wg
        m["bg"] = bg
        m["idn"] = idn
        # CNN per-core
        m["xp"] = xp_all[c * PPC:(c + 1) * PPC]
        m["wc"] = wcp
        m["bc"] = bcp
        # GCN per-core
        acf, acs, xsc, wgd, bgd = gcn_pack(
            A_c, Xs_c, np.asarray(I["W_gcn_d"], F32), np.asarray(I["b_gcn_d"], F32),
            16, N_C, N_C, c * CPC, CPC)
        m["acf"], m["acs"], m["xsc"], m["wgd"], m["bgd"] = acf, acs, xsc, wgd, bgd
        apf, aps, xsp, wgp, bgp = gcn_pack(
            Ap_pad, Xsp_pad, np.asarray(I["W_gcn_p"], F32), np.asarray(I["b_gcn_p"], F32),
            12, N_P_PAD, N_P_PAD, c * PPC, PPC)
        m["apf"], m["aps"], m["xsp"], m["wgp"], m["bgp"] = apf, aps, xsp, wgp, bgp
        in_maps.append(m)
    return in_maps


def _prep_phase2_inputs(I, comp_intT, Xc2T, prot_intT, Xp2T):
    bf = BF16
    idx_c = np.asarray(I["idx_c"])
    idx_p = np.asarray(I["idx_p"])
    df = np.asarray(I["drug_feat"], F32)
    pf = np.asarray(I["protein_feat"], F32).astype(F32)

    def pack_w1(W):   # [1024, 128] -> [128, 1024] lhsT chunks
        out = np.zeros((128, 1024), F32)
        for k in range(8):
            out[:, k * 128:(k + 1) * 128] = W[k * 128:(k + 1) * 128, :]
        return out.astype(F32)

    def pack_w23(W2, W3):  # [128, 64], [64, 64] -> [128, 128]
        out = np.zeros((128, 128), F32)
        out[:, 0:64] = W2
        out[0:64, 64:128] = W3
        return out.astype(F32)

    def pack_head(W, nk):  # [nk*128, 256] -> [128, nk*256]
        out = np.zeros((128, nk * 256), F32)
        for k in range(nk):
            for mh in range(2):
                out[:, (k * 2 + mh) * 128:(k * 2 + mh + 1) * 128] = \
                    W[k * 128:(k + 1) * 128, mh * 128:(mh + 1) * 128]
        return out.astype(F32)

    wd1 = pack_w1(np.asarray(I["Wd1"], F32))
    wd23 = pack_w23(np.asarray(I["Wd2"], F32), np.asarray(I["Wd3"], F32))
    wp1 = pack_w1(np.asarray(I["Wp1"], F32))
    wp23 = pack_w23(np.asarray(I["Wp2"], F32), np.asarray(I["Wp3"], F32))
    wo1 = pack_head(np.asarray(I["Wo1"], F32), 3)
    wo2 = pack_head(np.asarray(I["Wo2"], F32), 2)
    wo3 = pack_head(np.asarray(I["Wo3"], F32), 2)
    wi = np.zeros((128, 4), F32)
    Wi = np.asarray(I["W_int"], F32)
    for k in range(2):
        wi[:, k * 2:(k + 1) * 2] = Wi[k * 128:(k + 1) * 128, :]
    wi = wi.astype(F32)
    bia = np.zeros((128, 16), F32)
    bia[:, 0] = np.asarray(I["bd1"], F32)
    bia[0:64, 1] = np.asarray(I["bd2"], F32)
    bia[0:64, 2] = np.asarray(I["bd3"], F32)
    bia[:, 3] = np.asarray(I["bp1"], F32)
    bia[0:64, 4] = np.asarray(I["bp2"], F32)
    bia[64:128, 5] = np.asarray(I["bp3"], F32)
    bo1 = np.asarray(I["bo1"], F32); bia[:, 6] = bo1[0:128]; bia[:, 7] = bo1[128:256]
    bo2 = np.asarray(I["bo2"], F32); bia[:, 8] = bo2[0:128]; bia[:, 9] = bo2[128:256]
    bo3 = np.asarray(I["bo3"], F32); bia[:, 10] = bo3[0:128]; bia[:, 11] = bo3[128:256]
    bia[0:2, 12] = np.asarray(I["b_int"], F32)

    in_maps = []
    for c in range(NCORES):
        ic = idx_c[c * BPC:(c + 1) * BPC]
        ip = idx_p[c * BPC:(c + 1) * BPC]
        ent = np.concatenate([comp_intT[:, ic], Xc2T[:, ic],
                              prot_intT[:, ip], Xp2T[:, ip]], axis=0).astype(F32)
        dfT = np.ascontiguousarray(df[ic].T).reshape(8, 128, BPC)
        pfT = np.ascontiguousarray(pf[ip].T).reshape(8, 128, BPC)
        m = dict(ent=ent, df=dfT, pf=pfT, wd1=wd1, wd23=wd23, wp1=wp1,
                 wp23=wp23, wo1=wo1, wo2=wo2, wo3=wo3, wi=wi, bia=bia)
        in_maps.append(m)
    return in_maps


_CACHE = {}


def _get_kernels():
    if "p1" not in _CACHE:
        _CACHE["p1"] = build_phase1()
        _CACHE["p2"] = build_phase2()
    return _CACHE["p1"], _CACHE["p2"]


def run(inputs, trace=False):
    """Full pipeline. Returns (output [4096, 2] f32, info dict)."""
    I = inputs
    nc1, nc2 = _get_kernels()
    info = {}

    in1 = _prep_phase1_inputs(I)
    r1 = run_bass_kernel_spmd(nc1, in1, core_ids=list(range(NCORES)), trace=trace)
    res1 = r1.results
    if trace:
        info["p1_exec_ns"] = r1.exec_time_ns

    comp_sumT = np.zeros((64, N_C), F32)
    Xc2T = np.zeros((64, N_C), F32)
    prot_sumT = np.zeros((64, N_P_PAD), F32)
    Xp2T = np.zeros((64, N_P_PAD), F32)
    for c in range(NCORES):
        cs = np.asarray(res1[c]["csum"], F32)       # [128, 126]
        comp_sumT[:, c * CPC:c * CPC + G] = cs[0:64, 0:G]
        comp_sumT[:, c * CPC + G:(c + 1) * CPC] = cs[64:128, 0:G]
        Xc2T[:, c * CPC:(c + 1) * CPC] = np.asarray(res1[c]["xc2"], F32)
        pa = np.asarray(res1[c]["pacc"], F32)       # [128, 188]
        ev = np.arange(0, PPC, 2)
        prot_sumT[:, c * PPC + ev] = pa[0:64, ev]
        prot_sumT[:, c * PPC + ev + 1] = pa[64:128, ev + 1]
        Xp2T[:, c * PPC:(c + 1) * PPC] = np.asarray(res1[c]["xp2"], F32)
    comp_intT = comp_sumT / N_ATOMS
    prot_intT = prot_sumT / L

    in2 = _prep_phase2_inputs(I, comp_intT, Xc2T, prot_intT, Xp2T)
    r2 = run_bass_kernel_spmd(nc2, in2, core_ids=list(range(NCORES)), trace=trace)
    res2 = r2.results
    if trace:
        info["p2_exec_ns"] = r2.exec_time_ns

    out = np.zeros((B, 2), F32)
    for c in range(NCORES):
        out[c * BPC:(c + 1) * BPC] = np.asarray(res2[c]["out2"], F32).T
    return out, info


def kernel(**inputs):
    out, _ = run(inputs)
    return out


# revision 14
# speedup vs baseline: 2.4410x; 1.0266x over previous
"""Trainium2 Bass kernel for nn_DeepERA (GNN + CNN + GCN + MLP head), 8-core SPMD.

Self-contained: hardcodes shapes/sharding. Host does index gathers, weight
packing and layout prep; all dense compute runs on the 8 NeuronCores in two
SPMD launches (phase 1: entity embeddings; phase 2: pair MLPs + head).

Layouts (phase 1, per core):
  GNN: xs kept in "d-layout" [128, 5040] bf16 — partition p<64 -> (group 0,
  din=p), p>=64 -> (group 1, din=p-64); column = compound-in-group*40 + atom.
  Layer updates are never materialized: xs_{i+1} = xs0 + sum(deltas) is kept
  as separate delta tensors and accumulated in PSUM by the next matmul.
  CNN: per-protein doubled storage [128, 522] bf16 — partitions 0:64 hold
  x^T with left-pad 5, partitions 64:128 hold x^T with left-pad 4, so one
  [128, 512] rhs read at col offset 2p yields shifts (2p, 2p+1) stacked on
  the contraction dim. 11x11 conv = 6 such K=128 matmuls vs banded-pair
  weight matrices; two proteins run concurrently on PE col-groups via
  tile_position (0,0)/(0,64).
"""
import numpy as np
import ml_dtypes

import concourse.bass as bass
import concourse.bacc as bacc
import concourse.tile as tile
import concourse.mybir as mybir
from concourse.bass_utils import run_bass_kernel_spmd

BF16 = ml_dtypes.bfloat16
F32 = np.float32

DIM = 64
N_C = 2000
N_P = 1500
N_P_PAD = 1504           # 8 * 188
N_ATOMS = 40
L = 512
WIN = 5
B = 4096
NCORES = 8
CPC = N_C // NCORES      # 250 compounds / core
PPC = N_P_PAD // NCORES  # 188 proteins / core
BPC = B // NCORES        # 512 pairs / core
G = CPC // 2             # 125 compounds per partition-group
CHUNK = 42               # 3-compound chunks per group
GCP = CHUNK * 3 * N_ATOMS  # 5040 padded cols per group (5000 real)
NJ = 10
JW = GCP // NJ           # 504
NPAIR = PPC // 2         # 94 protein pairs / core

dt = mybir.dt
AFT = mybir.ActivationFunctionType


def _bands(K):
    """11 banded matrices Band_a[din, dout] = K[a, din - dout + 5]."""
    i, j = np.indices((DIM, DIM))
    bsel = i - j + WIN
    mask = (bsel >= 0) & (bsel < 11)
    out = np.zeros((11, DIM, DIM), np.float32)
    for a in range(11):
        out[a][mask] = K[a][bsel[mask]]
    return out


# ---------------------------------------------------------------- phase 1 ----
def build_phase1():
    nc = bacc.Bacc()
    bf, f32 = dt.bfloat16, dt.float32

    xw0_d = nc.dram_tensor("xw0", [128, GCP], bf, kind="ExternalInput")
    adjb_d = nc.dram_tensor("adjb", [120, 2 * CHUNK * 120], bf, kind="ExternalInput")
    wg_d = nc.dram_tensor("wg", [128, 3 * 128], bf, kind="ExternalInput")
    bg_d = nc.dram_tensor("bg", [128, 3], f32, kind="ExternalInput")
    idn_d = nc.dram_tensor("idn", [128, 128], bf, kind="ExternalInput")
    xp_d = nc.dram_tensor("xp", [PPC, 128, 522], bf, kind="ExternalInput")
    wc_d = nc.dram_tensor("wc", [128, 18 * 64], bf, kind="ExternalInput")
    bc_d = nc.dram_tensor("bc", [128, 3], f32, kind="ExternalInput")
    acf_d = nc.dram_tensor("acf", [16, 128, N_C], bf, kind="ExternalInput")
    acs_d = nc.dram_tensor("acs", [16, 128, CPC], bf, kind="ExternalInput")
    xsc_d = nc.dram_tensor("xsc", [128, 16 * 64], bf, kind="ExternalInput")
    wgd_d = nc.dram_tensor("wgd", [64, 128], bf, kind="ExternalInput")
    bgd_d = nc.dram_tensor("bgd", [64, 2], f32, kind="ExternalInput")
    apf_d = nc.dram_tensor("apf", [12, 128, N_P_PAD], bf, kind="ExternalInput")
    aps_d = nc.dram_tensor("aps", [12, 128, PPC], bf, kind="ExternalInput")
    xsp_d = nc.dram_tensor("xsp", [128, 12 * 64], bf, kind="ExternalInput")
    wgp_d = nc.dram_tensor("wgp", [64, 128], bf, kind="ExternalInput")
    bgp_d = nc.dram_tensor("bgp", [64, 2], f32, kind="ExternalInput")

    csum_d = nc.dram_tensor("csum", [128, 3 * CHUNK], f32, kind="ExternalOutput")
    pacc_d = nc.dram_tensor("pacc", [128, PPC], f32, kind="ExternalOutput")
    xc2_d = nc.dram_tensor("xc2", [64, CPC], bf, kind="ExternalOutput")
    xp2_d = nc.dram_tensor("xp2", [64, PPC], bf, kind="ExternalOutput")

    with tile.TileContext(nc) as tc:
        with tc.tile_pool(name="data", bufs=1) as data:
            # ---- persistent tiles + loads
            xw0 = data.tile([128, GCP], bf, name="xw0", tag="xw0")
            adjb = data.tile([120, 2 * CHUNK * 120], bf, name="adjb", tag="adjb")
            wg = data.tile([128, 3 * 128], bf, name="wg", tag="wg")
            bg = data.tile([128, 3], f32, name="bg", tag="bg")
            idn = data.tile([128, 128], bf, name="idn", tag="idn")
            wc = data.tile([128, 18 * 64], bf, name="wc", tag="wc")
            bc = data.tile([128, 3], f32, name="bc", tag="bc")
            for t, d in [(wc, wc_d), (bc, bc_d)]:
                nc.sync.dma_start(t[:], d[:])

            hsT = data.tile([128, GCP], bf, name="hsT", tag="hsT")
            dx = [data.tile([128, GCP], bf, name=f"dx{i}", tag=f"dx{i}") for i in range(3)]
            pracc = data.tile([128, PPC], f32, name="pracc", tag="pracc")

            # =================== CNN ===================
            with (
                tc.tile_pool(name="xb", bufs=1) as xb_pool,
                tc.tile_pool(name="ps_c", bufs=6, space=bass.MemorySpace.PSUM) as ps_c,
                tc.tile_pool(name="scr", bufs=4) as scr_p,
            ):
                xb = [xb_pool.tile([128, 522], bf, name=f"xb{i}", tag=f"xb{i}") for i in range(64)]
                for i, t in enumerate(xb):
                    if i % 2 == 1:  # 'nxt' tiles: zero the halo pads once
                        nc.gpsimd.memset(t[:, 0:5], 0.0)
                        nc.gpsimd.memset(t[:, 516:522], 0.0)
                BLK = 8
                for pr in range(min(BLK, NPAIR)):
                    s4 = (pr % 16) * 4
                    nc.sync.dma_start(xb[s4][:], xp_d[2 * pr])
                    nc.sync.dma_start(xb[s4 + 2][:], xp_d[2 * pr + 1])
                for t, d in [(idn, idn_d), (xw0, xw0_d), (wg, wg_d), (bg, bg_d),
                             (adjb, adjb_d)]:
                    nc.sync.dma_start(t[:], d[:])
                for b0 in range(0, NPAIR, BLK):
                    prs = list(range(b0, min(b0 + BLK, NPAIR)))
                    for pr in prs:
                        if b0 == 0:
                            break
                        s4 = (pr % 16) * 4
                        nc.sync.dma_start(xb[s4][:], xp_d[2 * pr])
                        nc.sync.dma_start(xb[s4 + 2][:], xp_d[2 * pr + 1])
                    for ly in range(3):
                        for pr in prs:
                            s4 = (pr % 16) * 4
                            cur0 = xb[s4 + (ly % 2)]
                            cur1 = xb[s4 + 2 + (ly % 2)]
                            nxt0 = xb[s4 + 1 - (ly % 2)]
                            nxt1 = xb[s4 + 3 - (ly % 2)]
                            pc = ps_c.tile([128, 512], f32, name="pc", tag="pc")
                            for p in range(6):
                                w = wc[:, (ly * 6 + p) * 64:(ly * 6 + p + 1) * 64]
                                nc.tensor.matmul(pc[0:64, :], w,
                                                 cur0[:, 2 * p:2 * p + 512],
                                                 start=(p == 0), stop=(p == 5),
                                                 tile_position=(0, 0))
                                nc.tensor.matmul(pc[64:128, :], w,
                                                 cur1[:, 2 * p:2 * p + 512],
                                                 start=(p == 0), stop=(p == 5),
                                                 tile_position=(0, 64))
                            if ly < 2:
                                nc.scalar.activation(nxt0[0:64, 5:517], pc[0:64, :],
                                                     AFT.Relu, bias=bc[0:64, ly:ly + 1])
                                nc.vector.tensor_scalar(
                                    nxt1[64:128, 4:516], pc[64:128, :],
                                    bc[64:128, ly:ly + 1], 0.0,
                                    op0=mybir.AluOpType.add, op1=mybir.AluOpType.max)
                                nc.gpsimd.dma_start(nxt0[64:128, 4:516], nxt0[0:64, 5:517])
                                nc.gpsimd.dma_start(nxt1[0:64, 5:517], nxt1[64:128, 4:516])
                            else:
                                s = scr_p.tile([128, 512], bf, name="scr", tag="scr")
                                nc.scalar.activation(
                                    s[0:64, :], pc[0:64, :], AFT.Relu,
                                    bias=bc[0:64, 2:3],
                                    accum_out=pracc[0:64, 2 * pr:2 * pr + 1])
                                nc.scalar.activation(
                                    s[64:128, :], pc[64:128, :], AFT.Relu,
                                    bias=bc[64:128, 2:3],
                                    accum_out=pracc[64:128, 2 * pr + 1:2 * pr + 2])
                nc.sync.dma_start(pacc_d[:], pracc[:])

            # =================== GCN ===================
            def gcn(nk, nfull, jn, jw, nshard, af_d, as_d, xs_d, w_d, b_d, out_d):
                with (
                    tc.tile_pool(name="gd", bufs=1) as gd,
                    tc.tile_pool(name="ga", bufs=16) as ga,
                    tc.tile_pool(name="gt", bufs=3) as gt,
                    tc.tile_pool(name="ps_g", bufs=2, space=bass.MemorySpace.PSUM) as ps_g,
                    tc.tile_pool(name="ps_g2", bufs=2, space=bass.MemorySpace.PSUM) as ps_g2,
                    tc.tile_pool(name="ps_t2", bufs=2, space=bass.MemorySpace.PSUM) as ps_t2,
                ):
                    xs = gd.tile([128, nk * 64], dt.bfloat16, name="xs", tag="xs")
                    w = gd.tile([64, 128], dt.bfloat16, name="w", tag="w")
                    b = gd.tile([64, 2], dt.float32, name="b", tag="b")
                    x1T = gd.tile([64, nk * 128], dt.bfloat16, name="x1T", tag="x1T")
                    x1n = gd.tile([128, nk * 64], dt.bfloat16, name="x1n", tag="x1n")
                    nc.sync.dma_start(xs[:], xs_d[:])
                    nc.sync.dma_start(w[:], w_d[:])
                    nc.sync.dma_start(b[:], b_d[:])
                    if nk * 128 > nfull:
                        nc.gpsimd.memset(x1T[:, nfull:nk * 128], 0.0)
                    # layer 1 (full, redundant on all cores)
                    for j in range(jn):
                        pg = ps_g.tile([64, jw], dt.float32, name="pg", tag="pg")
                        for k in range(nk):
                            at = ga.tile([128, jw], dt.bfloat16, name="at", tag="at")
                            nc.sync.dma_start(at[:], af_d[k][:, j * jw:(j + 1) * jw])
                            nc.tensor.matmul(pg[:], xs[:, k * 64:(k + 1) * 64], at[:],
                                             start=(k == 0), stop=(k == nk - 1))
                        p1 = gt.tile([64, jw], dt.bfloat16, name="p1", tag="p1")
                        nc.scalar.copy(p1[:], pg[:])
                        pg2 = ps_g2.tile([64, jw], dt.float32, name="pg2", tag="pg2")
                        nc.tensor.matmul(pg2[:], w[:, 0:64], p1[:],
                                         start=True, stop=True)
                        nc.scalar.activation(x1T[:, j * jw:(j + 1) * jw], pg2[:],
                                             AFT.Relu, bias=b[:, 0:1])
                    # transpose x1T -> x1n (normal layout chunks)
                    for k in range(nk):
                        pt2 = ps_t2.tile([128, 64], dt.float32, name="pt2", tag="pt2")
                        nc.tensor.matmul(pt2[:], x1T[:, k * 128:(k + 1) * 128],
                                         idn[0:64, 0:64], start=True, stop=True)
                        nc.scalar.copy(x1n[:, k * 64:(k + 1) * 64], pt2[:])
                    # layer 2 (only this core's shard of rows)
                    pg = ps_g.tile([64, nshard], dt.float32, name="pgs", tag="pg")
                    for k in range(nk):
                        at = ga.tile([128, nshard], dt.bfloat16, name="ats", tag="ats")
                        nc.sync.dma_start(at[:], as_d[k][:])
                        nc.tensor.matmul(pg[:], x1n[:, k * 64:(k + 1) * 64], at[:],
                                         start=(k == 0), stop=(k == nk - 1))
                    p2 = gt.tile([64, nshard], dt.bfloat16, name="p2", tag="p2")
                    nc.scalar.copy(p2[:], pg[:])
                    pg2 = ps_g2.tile([64, nshard], dt.float32, name="pg2s", tag="pg2")
                    nc.tensor.matmul(pg2[:], w[:, 64:128], p2[:],
                                     start=True, stop=True)
                    x2T = gt.tile([64, nshard], dt.bfloat16, name="x2T", tag="x2T")
                    nc.scalar.activation(x2T[:], pg2[:], AFT.Relu, bias=b[:, 1:2])
                    nc.sync.dma_start(out_d[:], x2T[:])

            gcn(16, N_C, 4, 500, CPC, acf_d, acs_d, xsc_d, wgd_d, bgd_d, xc2_d)
            gcn(12, N_P_PAD, 4, 376, PPC, apf_d, aps_d, xsp_d, wgp_d, bgp_d, xp2_d)

            # =================== GNN ===================
            with (
                tc.tile_pool(name="ps_h", bufs=2, space=bass.MemorySpace.PSUM) as ps_h,
                tc.tile_pool(name="ps_t", bufs=3, space=bass.MemorySpace.PSUM) as ps_t,
                tc.tile_pool(name="ps_d", bufs=3, space=bass.MemorySpace.PSUM) as ps_d,
                tc.tile_pool(name="ha_p", bufs=4) as ha_p,
            ):
                for ly in range(3):
                    srcs = [xw0] + dx[:ly]
                    for j in range(NJ):
                        ph = ps_h.tile([128, JW], f32, name="ph", tag="ph")
                        for si, s in enumerate(srcs):
                            nc.tensor.matmul(
                                ph[:], wg[:, ly * 128:(ly + 1) * 128],
                                s[:, j * JW:(j + 1) * JW],
                                start=(si == 0), stop=(si == len(srcs) - 1))
                        nc.scalar.activation(hsT[:, j * JW:(j + 1) * JW], ph[:],
                                             AFT.Relu, bias=bg[:, ly:ly + 1])
                    for c in range(CHUNK):
                        pt = ps_t.tile([120, 128], f32, name="pt", tag="pt")
                        nc.tensor.matmul(pt[:], hsT[:, c * 120:(c + 1) * 120],
                                         idn[:], start=True, stop=True)
                        ha = ha_p.tile([120, 128], bf, name="ha", tag="ha")
                        nc.scalar.copy(ha[:], pt[:])
                        pd = ps_d.tile([128, 120], f32, name="pd", tag="pd")
                        nc.tensor.matmul(
                            pd[0:64, :], ha[:, 0:64],
                            adjb[:, c * 120:(c + 1) * 120],
                            start=True, stop=True, tile_position=(0, 0))
                        nc.tensor.matmul(
                            pd[64:128, :], ha[:, 64:128],
                            adjb[:, (CHUNK + c) * 120:(CHUNK + c + 1) * 120],
                            start=True, stop=True, tile_position=(0, 64))
                        nc.vector.tensor_copy(dx[ly][:, c * 120:(c + 1) * 120], pd[:])

                # comp_int raw sums: reduce over atoms for xs0 + d1 + d2 + d3
                crs = []
                for si, s in enumerate([xw0] + dx):
                    cr = ha_p.tile([128, 3 * CHUNK], f32, name=f"cr{si}", tag=f"cr{si}")
                    nc.vector.reduce_sum(
                        cr[:], s[:].rearrange("p (c a) -> p c a", a=N_ATOMS),
                        axis=mybir.AxisListType.X)
                    crs.append(cr)
                nc.vector.tensor_add(crs[0][:], crs[0][:], crs[1][:])
                nc.vector.tensor_add(crs[2][:], crs[2][:], crs[3][:])
                nc.vector.tensor_add(crs[0][:], crs[0][:], crs[2][:])
                nc.sync.dma_start(csum_d[:], crs[0][:])

    nc.compile()
    return nc


# ---------------------------------------------------------------- phase 2 ----
def build_phase2():
    nc = bacc.Bacc()
    bf, f32 = dt.bfloat16, dt.float32

    ent_d = nc.dram_tensor("ent", [256, BPC], f32, kind="ExternalInput")
    df_d = nc.dram_tensor("df", [8, 128, BPC], f32, kind="ExternalInput")
    pf_d = nc.dram_tensor("pf", [8, 128, BPC], f32, kind="ExternalInput")
    wd1_d = nc.dram_tensor("wd1", [128, 1024], f32, kind="ExternalInput")
    wd23_d = nc.dram_tensor("wd23", [128, 128], f32, kind="ExternalInput")
    wp1_d = nc.dram_tensor("wp1", [128, 1024], f32, kind="ExternalInput")
    wp23_d = nc.dram_tensor("wp23", [128, 128], f32, kind="ExternalInput")
    wo1_d = nc.dram_tensor("wo1", [128, 768], f32, kind="ExternalInput")
    wo2_d = nc.dram_tensor("wo2", [128, 512], f32, kind="ExternalInput")
    wo3_d = nc.dram_tensor("wo3", [128, 512], f32, kind="ExternalInput")
    wi_d = nc.dram_tensor("wi", [128, 4], f32, kind="ExternalInput")
    bia_d = nc.dram_tensor("bia", [128, 16], f32, kind="ExternalInput")
    # bia columns: 0 bd1, 1 bd2(0:64), 2 bd3(0:64), 3 bp1, 4 bp2, 5 bp3,
    #              6 bo1_m0, 7 bo1_m1, 8 bo2_m0, 9 bo2_m1, 10 bo3_m0, 11 bo3_m1,
    #              12 bint(0:2)
    out_d = nc.dram_tensor("out2", [2, BPC], f32, kind="ExternalOutput")

    with tile.TileContext(nc) as tc:
        with (
            tc.tile_pool(name="data", bufs=1) as data,
            tc.tile_pool(name="ps_a", bufs=2, space=bass.MemorySpace.PSUM) as ps_a,
            tc.tile_pool(name="ps_b", bufs=2, space=bass.MemorySpace.PSUM) as ps_b,
        ):
            wd1 = data.tile([128, 1024], f32, name="wd1", tag="wd1")
            wd23 = data.tile([128, 128], f32, name="wd23", tag="wd23")
            wp1 = data.tile([128, 1024], f32, name="wp1", tag="wp1")
            wp23 = data.tile([128, 128], f32, name="wp23", tag="wp23")
            wo1 = data.tile([128, 768], f32, name="wo1", tag="wo1")
            wo2 = data.tile([128, 512], f32, name="wo2", tag="wo2")
            wo3 = data.tile([128, 512], f32, name="wo3", tag="wo3")
            wi = data.tile([128, 4], f32, name="wi", tag="wi")
            bia = data.tile([128, 16], f32, name="bia", tag="bia")
            for t, d in [(wd1, wd1_d), (wd23, wd23_d), (wp1, wp1_d),
                         (wp23, wp23_d), (wo1, wo1_d), (wo2, wo2_d),
                         (wo3, wo3_d), (wi, wi_d), (bia, bia_d)]:
                nc.sync.dma_start(t[:], d[:])
            C = [data.tile([128, BPC], f32, name=f"C{k}", tag=f"C{k}") for k in range(3)]
            nc.sync.dma_start(C[0][:], ent_d[0:128, :])
            nc.sync.dma_start(C[1][64:128, :], ent_d[128:192, :])
            nc.sync.dma_start(C[2][0:64, :], ent_d[192:256, :])

            def mlp(src_d, w1, w23, bcol, outtile, outslice, tpos):
                f1p = ps_a.tile([128, BPC], f32, name="f1p", tag="psa")
                for k in range(8):
                    xt = data.tile([128, BPC], f32, name=f"xt{bcol}_{k}", tag=f"xt{bcol}_{k}")
                    nc.sync.dma_start(xt[:], src_d[k])
                    nc.tensor.matmul(f1p[:], w1[:, k * 128:(k + 1) * 128], xt[:],
                                     start=(k == 0), stop=(k == 7))
                f1 = data.tile([128, BPC], f32, name=f"f1_{bcol}", tag=f"f1_{bcol}")
                nc.scalar.activation(f1[:], f1p[:], AFT.Relu, bias=bia[:, bcol:bcol + 1])
                f2p = ps_b.tile([128, BPC], f32, name="f2p", tag="psb")
                nc.tensor.matmul(f2p[0:64, :], w23[:, 0:64], f1[:],
                                 start=True, stop=True)
                f2 = data.tile([64, BPC], f32, name=f"f2_{bcol}", tag=f"f2_{bcol}")
                nc.scalar.activation(f2[:], f2p[0:64, :], AFT.Relu,
                                     bias=bia[0:64, bcol + 1:bcol + 2])
                f3p = ps_b.tile([128, BPC], f32, name="f3p", tag="psb")
                lo, hi = (0, 64) if tpos == (0, 0) else (64, 128)
                nc.tensor.matmul(f3p[lo:hi, :], w23[0:64, 64:128], f2[:],
                                 start=True, stop=True, tile_position=tpos)
                nc.scalar.activation(outtile[outslice, :], f3p[lo:hi, :], AFT.Relu,
                                     bias=bia[lo:hi, bcol + 2:bcol + 3])

            mlp(df_d, wd1, wd23, 0, C[1], slice(0, 64), (0, 0))
            mlp(pf_d, wp1, wp23, 3, C[2], slice(64, 128), (0, 64))

            h = C
            for li, (wt, nk, bcol) in enumerate([(wo1, 3, 6), (wo2, 2, 8),
                                                 (wo3, 2, 10)]):
                hn = []
                for m in range(2):
                    hp = ps_a.tile([128, BPC], f32, name=f"hp{li}{m}", tag="psa")
                    for k in range(nk):
                        nc.tensor.matmul(hp[:], wt[:, (k * 2 + m) * 128:
                                                    (k * 2 + m + 1) * 128],
                                         h[k][:], start=(k == 0), stop=(k == nk - 1))
                    ht = data.tile([128, BPC], f32, name=f"h{li}{m}", tag=f"h{li}{m}")
                    nc.scalar.activation(ht[:], hp[:], AFT.Relu,
                                         bias=bia[:, bcol + m:bcol + m + 1])
                    hn.append(ht)
                h = hn
            zp = ps_b.tile([2, BPC], f32, name="zp", tag="psb")
            for k in range(2):
                nc.tensor.matmul(zp[:], wi[:, k * 2:(k + 1) * 2], h[k][:],
                                 start=(k == 0), stop=(k == 1))
            zs = data.tile([2, BPC], f32, name="zs", tag="zs")
            nc.scalar.activation(zs[:], zp[:], AFT.Sigmoid, bias=bia[0:2, 12:13])
            nc.sync.dma_start(out_d[:], zs[:])

    nc.compile()
    return nc


# ------------------------------------------------------------- host prep ----
def _prep_phase1_inputs(I):
    """Returns list of 8 per-core input dicts for phase 1."""
    bf = BF16
    emb_fp = np.asarray(I["embed_fp"], F32)
    compounds = np.asarray(I["compounds"])
    adj = np.asarray(I["adjacencies"], F32)
    W_gnn = np.asarray(I["W_gnn"], F32)
    b_gnn = np.asarray(I["b_gnn"], F32)
    emb_w = np.asarray(I["embed_word"], F32).astype(bf)
    proteins = np.asarray(I["proteins"])
    K_cnn = np.asarray(I["K_cnn"], F32)
    b_cnn = np.asarray(I["b_cnn"], F32)

    # GNN packing
    xg = emb_fp[compounds]                       # [N_C, 40, 64] f32
    wg = np.zeros((128, 3 * 128), F32)
    bg = np.zeros((128, 3), F32)
    for i in range(3):
        wg[0:64, i * 128:i * 128 + 64] = W_gnn[i]
        wg[64:128, i * 128 + 64:i * 128 + 128] = W_gnn[i]
        bg[0:64, i] = b_gnn[i]
        bg[64:128, i] = b_gnn[i]
    wg = wg.astype(bf)
    idn = np.eye(128, dtype=F32).astype(bf)

    # CNN packing
    bands = np.stack([_bands(K_cnn[i]) for i in range(3)])   # [3, 11, 64, 64]
    wcp = np.zeros((128, 18 * 64), F32)
    for i in range(3):
        for p in range(6):
            cb = (i * 6 + p) * 64
            wcp[0:64, cb:cb + 64] = bands[i, 2 * p]
            if 2 * p + 1 < 11:
                wcp[64:128, cb:cb + 64] = bands[i, 2 * p + 1]
    wcp = wcp.astype(bf)
    bcp = np.zeros((128, 3), F32)
    bcp[:, :] = b_cnn[None, :]

    prot_pad = np.zeros((N_P_PAD,) + proteins.shape[1:], proteins.dtype)
    prot_pad[:N_P] = proteins
    xw_all = emb_w[prot_pad]                     # [1504, 512, 64] bf16
    xT = np.ascontiguousarray(xw_all.transpose(0, 2, 1))  # [1504, 64, 512]
    xp_all = np.zeros((N_P_PAD, 128, 522), bf)
    xp_all[:, 0:64, 5:517] = xT
    xp_all[:, 64:128, 4:516] = xT

    # GCN packing
    def gcn_pack(A, Xs, Wl, bl, nk, nfull, npad, shard_lo, shard_n):
        Ap = np.zeros((nk * 128, npad), F32)
        Ap[:nfull, :nfull] = A[:nfull, :nfull]
        Xp = np.zeros((nk * 128, 64), F32)
        Xp[:nfull] = Xs[:nfull]
        af = np.ascontiguousarray(
            Ap.reshape(nk, 128, npad)).astype(bf)
        asd = np.ascontiguousarray(
            Ap[:, shard_lo:shard_lo + shard_n].reshape(nk, 128, shard_n)).astype(bf)
        xs = np.ascontiguousarray(
            Xp.reshape(nk, 128, 64).transpose(1, 0, 2).reshape(128, nk * 64)).astype(bf)
        w = np.concatenate([Wl[0], Wl[1]], axis=1).astype(bf)   # [64, 128]
        b = np.stack([bl[0], bl[1]], axis=1).astype(F32)        # [64, 2]
        return af, asd, xs, w, b

    A_c = np.asarray(I["A_c"], F32)
    A_p = np.asarray(I["A_p"], F32)
    Ap_pad = np.zeros((N_P_PAD, N_P_PAD), F32)
    Ap_pad[:N_P, :N_P] = A_p
    Xs_c = np.asarray(I["Xs_c"], F32)
    Xs_p = np.asarray(I["Xs_p"], F32)
    Xsp_pad = np.zeros((N_P_PAD, 64), F32)
    Xsp_pad[:N_P] = Xs_p

    in_maps = []
    for c in range(NCORES):
        m = {}
        # GNN per-core
        xs_c = xg[c * CPC:(c + 1) * CPC]          # [250, 40, 64]
        xw0 = np.zeros((128, GCP), F32)
        for g in range(2):
            blk = xs_c[g * G:(g + 1) * G].reshape(G * N_ATOMS, 64).T  # [64, 5000]
            xw0[g * 64:(g + 1) * 64, :G * N_ATOMS] = blk
        m["xw0"] = xw0.astype(bf)
        adjb = np.zeros((120, 2 * CHUNK * 120), F32)
        for g in range(2):
            for ch in range(CHUNK):
                for k3 in range(3):
                    ci = g * G + ch * 3 + k3
                    if ch * 3 + k3 < G:
                        cb = (g * CHUNK + ch) * 120 + k3 * 40
                        adjb[k3 * 40:(k3 + 1) * 40, cb:cb + 40] = \
                            adj[c * CPC + ci]
        m["adjb"] = adjb.astype(bf)
        m["wg"] = wg
        m["bg"] = bg
        m["idn"] = idn
        # CNN per-core
        m["xp"] = xp_all[c * PPC:(c + 1) * PPC]
        m["wc"] = wcp
        m["bc"] = bcp
        # GCN per-core
        acf, acs, xsc, wgd, bgd = gcn_pack(
            A_c, Xs_c, np.asarray(I["W_gcn_d"], F32), np.asarray(I["b_gcn_d"], F32),
            16, N_C, N_C, c * CPC, CPC)
        m["acf"], m["acs"], m["xsc"], m["wgd"], m["bgd"] = acf, acs, xsc, wgd, bgd
        apf, aps, xsp, wgp, bgp = gcn_pack(
            Ap_pad, Xsp_pad, np.asarray(I["W_gcn_p"], F32), np.asarray(I["b_gcn_p"], F32),
            12, N_P_PAD, N_P_PAD, c * PPC, PPC)
        m["apf"], m["aps"], m["xsp"], m["wgp"], m["bgp"] = apf, aps, xsp, wgp, bgp
        in_maps.append(m)
    return in_maps


def _prep_phase2_inputs(I, comp_intT, Xc2T, prot_intT, Xp2T):
    bf = BF16
    idx_c = np.asarray(I["idx_c"])
    idx_p = np.asarray(I["idx_p"])
    df = np.asarray(I["drug_feat"], F32)
    pf = np.asarray(I["protein_feat"], F32).astype(F32)

    def pack_w1(W):   # [1024, 128] -> [128, 1024] lhsT chunks
        out = np.zeros((128, 1024), F32)
        for k in range(8):
            out[:, k * 128:(k + 1) * 128] = W[k * 128:(k + 1) * 128, :]
        return out.astype(F32)

    def pack_w23(W2, W3):  # [128, 64], [64, 64] -> [128, 128]
        out = np.zeros((128, 128), F32)
        out[:, 0:64] = W2
        out[0:64, 64:128] = W3
        return out.astype(F32)

    def pack_head(W, nk):  # [nk*128, 256] -> [128, nk*256]
        out = np.zeros((128, nk * 256), F32)
        for k in range(nk):
            for mh in range(2):
                out[:, (k * 2 + mh) * 128:(k * 2 + mh + 1) * 128] = \
                    W[k * 128:(k + 1) * 128, mh * 128:(mh + 1) * 128]
        return out.astype(F32)

    wd1 = pack_w1(np.asarray(I["Wd1"], F32))
    wd23 = pack_w23(np.asarray(I["Wd2"], F32), np.asarray(I["Wd3"], F32))
    wp1 = pack_w1(np.asarray(I["Wp1"], F32))
    wp23 = pack_w23(np.asarray(I["Wp2"], F32), np.asarray(I["Wp3"], F32))
    wo1 = pack_head(np.asarray(I["Wo1"], F32), 3)
    wo2 = pack_head(np.asarray(I["Wo2"], F32), 2)
    wo3 = pack_head(np.asarray(I["Wo3"], F32), 2)
    wi = np.zeros((128, 4), F32)
    Wi = np.asarray(I["W_int"], F32)
    for k in range(2):
        wi[:, k * 2:(k + 1) * 2] = Wi[k * 128:(k + 1) * 128, :]
    wi = wi.astype(F32)
    bia = np.zeros((128, 16), F32)
    bia[:, 0] = np.asarray(I["bd1"], F32)
    bia[0:64, 1] = np.asarray(I["bd2"], F32)
    bia[0:64, 2] = np.asarray(I["bd3"], F32)
    bia[:, 3] = np.asarray(I["bp1"], F32)
    bia[0:64, 4] = np.asarray(I["bp2"], F32)
    bia[64:128, 5] = np.asarray(I["bp3"], F32)
    bo1 = np.asarray(I["bo1"], F32); bia[:, 6] = bo1[0:128]; bia[:, 7] = bo1[128:256]
    bo2 = np.asarray(I["bo2"], F32); bia[:, 8] = bo2[0:128]; bia[:, 9] = bo2[128:256]
    bo3 = np.asarray(I["bo3"], F32); bia[:, 10] = bo3[0:128]; bia[:, 11] = bo3[128:256]
    bia[0:2, 12] = np.asarray(I["b_int"], F32)

    in_maps = []
    for c in range(NCORES):
        ic = idx_c[c * BPC:(c + 1) * BPC]
        ip = idx_p[c * BPC:(c + 1) * BPC]
        ent = np.concatenate([comp_intT[:, ic], Xc2T[:, ic],
                              prot_intT[:, ip], Xp2T[:, ip]], axis=0).astype(F32)
        dfT = np.ascontiguousarray(df[ic].T).reshape(8, 128, BPC)
        pfT = np.ascontiguousarray(pf[ip].T).reshape(8, 128, BPC)
        m = dict(ent=ent, df=dfT, pf=pfT, wd1=wd1, wd23=wd23, wp1=wp1,
                 wp23=wp23, wo1=wo1, wo2=wo2, wo3=wo3, wi=wi, bia=bia)
        in_maps.append(m)
    return in_maps


_CACHE = {}


def _get_kernels():
    if "p1" not in _CACHE:
        _CACHE["p1"] = build_phase1()
        _CACHE["p2"] = build_phase2()
    return _CACHE["p1"], _CACHE["p2"]


def run(inputs, trace=False):
    """Full pipeline. Returns (output [4096, 2] f32, info dict)."""
    I = inputs
    nc1, nc2 = _get_kernels()
    info = {}

    in1 = _prep_phase1_inputs(I)
    r1 = run_bass_kernel_spmd(nc1, in1, core_ids=list(range(NCORES)), trace=trace)
    res1 = r1.results
    if trace:
        info["p1_exec_ns"] = r1.exec_time_ns

    comp_sumT = np.zeros((64, N_C), F32)
    Xc2T = np.zeros((64, N_C), F32)
    prot_sumT = np.zeros((64, N_P_PAD), F32)
    Xp2T = np.zeros((64, N_P_PAD), F32)
    for c in range(NCORES):
        cs = np.asarray(res1[c]["csum"], F32)       # [128, 126]
        comp_sumT[:, c * CPC:c * CPC + G] = cs[0:64, 0:G]
        comp_sumT[:, c * CPC + G:(c + 1) * CPC] = cs[64:128, 0:G]
        Xc2T[:, c * CPC:(c + 1) * CPC] = np.asarray(res1[c]["xc2"], F32)
        pa = np.asarray(res1[c]["pacc"], F32)       # [128, 188]
        ev = np.arange(0, PPC, 2)
        prot_sumT[:, c * PPC + ev] = pa[0:64, ev]
        prot_sumT[:, c * PPC + ev + 1] = pa[64:128, ev + 1]
        Xp2T[:, c * PPC:(c + 1) * PPC] = np.asarray(res1[c]["xp2"], F32)
    comp_intT = comp_sumT / N_ATOMS
    prot_intT = prot_sumT / L

    in2 = _prep_phase2_inputs(I, comp_intT, Xc2T, prot_intT, Xp2T)
    r2 = run_bass_kernel_spmd(nc2, in2, core_ids=list(range(NCORES)), trace=trace)
    res2 = r2.results
    if trace:
        info["p2_exec_ns"] = r2.exec_time_ns

    out = np.zeros((B, 2), F32)
    for c in range(NCORES):
        out[c * BPC:(c + 1) * BPC] = np.asarray(res2[c]["out2"], F32).T
    return out, info


def kernel(**inputs):
    out, _ = run(inputs)
    return out


# revision 15
# speedup vs baseline: 2.4526x; 1.0048x over previous
"""Trainium2 Bass kernel for nn_DeepERA (GNN + CNN + GCN + MLP head), 8-core SPMD.

Self-contained: hardcodes shapes/sharding. Host does index gathers, weight
packing and layout prep; all dense compute runs on the 8 NeuronCores in two
SPMD launches (phase 1: entity embeddings; phase 2: pair MLPs + head).

Layouts (phase 1, per core):
  GNN: xs kept in "d-layout" [128, 5040] bf16 — partition p<64 -> (group 0,
  din=p), p>=64 -> (group 1, din=p-64); column = compound-in-group*40 + atom.
  Layer updates are never materialized: xs_{i+1} = xs0 + sum(deltas) is kept
  as separate delta tensors and accumulated in PSUM by the next matmul.
  CNN: per-protein doubled storage [128, 522] bf16 — partitions 0:64 hold
  x^T with left-pad 5, partitions 64:128 hold x^T with left-pad 4, so one
  [128, 512] rhs read at col offset 2p yields shifts (2p, 2p+1) stacked on
  the contraction dim. 11x11 conv = 6 such K=128 matmuls vs banded-pair
  weight matrices; two proteins run concurrently on PE col-groups via
  tile_position (0,0)/(0,64).
"""
import numpy as np
import ml_dtypes

import concourse.bass as bass
import concourse.bacc as bacc
import concourse.tile as tile
import concourse.mybir as mybir
from concourse.bass_utils import run_bass_kernel_spmd

BF16 = ml_dtypes.bfloat16
F32 = np.float32

DIM = 64
N_C = 2000
N_P = 1500
N_P_PAD = 1504           # 8 * 188
N_ATOMS = 40
L = 512
WIN = 5
B = 4096
NCORES = 8
CPC = N_C // NCORES      # 250 compounds / core
PPC = N_P_PAD // NCORES  # 188 proteins / core
BPC = B // NCORES        # 512 pairs / core
G = CPC // 2             # 125 compounds per partition-group
CHUNK = 42               # 3-compound chunks per group
GCP = CHUNK * 3 * N_ATOMS  # 5040 padded cols per group (5000 real)
NJ = 10
JW = GCP // NJ           # 504
NPAIR = PPC // 2         # 94 protein pairs / core

dt = mybir.dt
AFT = mybir.ActivationFunctionType


def _bands(K):
    """11 banded matrices Band_a[din, dout] = K[a, din - dout + 5]."""
    i, j = np.indices((DIM, DIM))
    bsel = i - j + WIN
    mask = (bsel >= 0) & (bsel < 11)
    out = np.zeros((11, DIM, DIM), np.float32)
    for a in range(11):
        out[a][mask] = K[a][bsel[mask]]
    return out


# ---------------------------------------------------------------- phase 1 ----
def build_phase1():
    nc = bacc.Bacc()
    bf, f32 = dt.bfloat16, dt.float32

    xw0_d = nc.dram_tensor("xw0", [128, GCP], bf, kind="ExternalInput")
    adjb_d = nc.dram_tensor("adjb", [120, 2 * CHUNK * 120], bf, kind="ExternalInput")
    wg_d = nc.dram_tensor("wg", [128, 3 * 128], bf, kind="ExternalInput")
    bg_d = nc.dram_tensor("bg", [128, 3], f32, kind="ExternalInput")
    idn_d = nc.dram_tensor("idn", [128, 128], bf, kind="ExternalInput")
    xp_d = nc.dram_tensor("xp", [PPC, 128, 522], bf, kind="ExternalInput")
    wc_d = nc.dram_tensor("wc", [128, 18 * 64], bf, kind="ExternalInput")
    bc_d = nc.dram_tensor("bc", [128, 3], f32, kind="ExternalInput")
    acf_d = nc.dram_tensor("acf", [16, 128, N_C], bf, kind="ExternalInput")
    acs_d = nc.dram_tensor("acs", [16, 128, CPC], bf, kind="ExternalInput")
    xsc_d = nc.dram_tensor("xsc", [128, 16 * 64], bf, kind="ExternalInput")
    wgd_d = nc.dram_tensor("wgd", [64, 128], bf, kind="ExternalInput")
    bgd_d = nc.dram_tensor("bgd", [64, 2], f32, kind="ExternalInput")
    apf_d = nc.dram_tensor("apf", [12, 128, N_P_PAD], bf, kind="ExternalInput")
    aps_d = nc.dram_tensor("aps", [12, 128, PPC], bf, kind="ExternalInput")
    xsp_d = nc.dram_tensor("xsp", [128, 12 * 64], bf, kind="ExternalInput")
    wgp_d = nc.dram_tensor("wgp", [64, 128], bf, kind="ExternalInput")
    bgp_d = nc.dram_tensor("bgp", [64, 2], f32, kind="ExternalInput")

    csum_d = nc.dram_tensor("csum", [128, 3 * CHUNK], f32, kind="ExternalOutput")
    pacc_d = nc.dram_tensor("pacc", [128, PPC], f32, kind="ExternalOutput")
    xc2_d = nc.dram_tensor("xc2", [64, CPC], bf, kind="ExternalOutput")
    xp2_d = nc.dram_tensor("xp2", [64, PPC], bf, kind="ExternalOutput")

    with tile.TileContext(nc) as tc:
        with tc.tile_pool(name="data", bufs=1) as data:
            # ---- persistent tiles + loads
            xw0 = data.tile([128, GCP], bf, name="xw0", tag="xw0")
            adjb = data.tile([120, 2 * CHUNK * 120], bf, name="adjb", tag="adjb")
            wg = data.tile([128, 3 * 128], bf, name="wg", tag="wg")
            bg = data.tile([128, 3], f32, name="bg", tag="bg")
            idn = data.tile([128, 128], bf, name="idn", tag="idn")
            wc = data.tile([128, 18 * 64], bf, name="wc", tag="wc")
            bc = data.tile([128, 3], f32, name="bc", tag="bc")
            for t, d in [(wc, wc_d), (bc, bc_d)]:
                nc.sync.dma_start(t[:], d[:])

            hsT = data.tile([128, GCP], bf, name="hsT", tag="hsT")
            dx = [data.tile([128, GCP], bf, name=f"dx{i}", tag=f"dx{i}") for i in range(3)]
            pracc = data.tile([128, PPC], f32, name="pracc", tag="pracc")

            # =================== CNN ===================
            with (
                tc.tile_pool(name="xb", bufs=1) as xb_pool,
                tc.tile_pool(name="ps_c", bufs=6, space=bass.MemorySpace.PSUM) as ps_c,
                tc.tile_pool(name="scr", bufs=4) as scr_p,
            ):
                xb = [xb_pool.tile([128, 522], bf, name=f"xb{i}", tag=f"xb{i}") for i in range(64)]
                for i, t in enumerate(xb):
                    if i % 2 == 1:  # 'nxt' tiles: zero the halo pads once
                        nc.gpsimd.memset(t[:, 0:5], 0.0)
                        nc.gpsimd.memset(t[:, 516:522], 0.0)
                BLK = 8
                for pr in range(min(BLK, NPAIR)):
                    s4 = (pr % 16) * 4
                    nc.sync.dma_start(xb[s4][:], xp_d[2 * pr])
                    nc.sync.dma_start(xb[s4 + 2][:], xp_d[2 * pr + 1])
                for t, d in [(idn, idn_d), (xw0, xw0_d), (wg, wg_d), (bg, bg_d),
                             (adjb, adjb_d)]:
                    nc.sync.dma_start(t[:], d[:])
                for b0 in range(0, NPAIR, BLK):
                    prs = list(range(b0, min(b0 + BLK, NPAIR)))
                    for pr in prs:
                        if b0 == 0:
                            break
                        s4 = (pr % 16) * 4
                        nc.sync.dma_start(xb[s4][:], xp_d[2 * pr])
                        nc.sync.dma_start(xb[s4 + 2][:], xp_d[2 * pr + 1])
                    for ly in range(3):
                        for pr in prs:
                            s4 = (pr % 16) * 4
                            cur0 = xb[s4 + (ly % 2)]
                            cur1 = xb[s4 + 2 + (ly % 2)]
                            nxt0 = xb[s4 + 1 - (ly % 2)]
                            nxt1 = xb[s4 + 3 - (ly % 2)]
                            pc = ps_c.tile([128, 512], f32, name="pc", tag="pc")
                            for p in range(6):
                                w = wc[:, (ly * 6 + p) * 64:(ly * 6 + p + 1) * 64]
                                nc.tensor.matmul(pc[0:64, :], w,
                                                 cur0[:, 2 * p:2 * p + 512],
                                                 start=(p == 0), stop=(p == 5),
                                                 tile_position=(0, 0))
                                nc.tensor.matmul(pc[64:128, :], w,
                                                 cur1[:, 2 * p:2 * p + 512],
                                                 start=(p == 0), stop=(p == 5),
                                                 tile_position=(0, 64))
                            if ly < 2:
                                nc.scalar.activation(nxt0[0:64, 5:517], pc[0:64, :],
                                                     AFT.Relu, bias=bc[0:64, ly:ly + 1])
                                nc.vector.tensor_scalar(
                                    nxt1[64:128, 4:516], pc[64:128, :],
                                    bc[64:128, ly:ly + 1], 0.0,
                                    op0=mybir.AluOpType.add, op1=mybir.AluOpType.max)
                                nc.gpsimd.dma_start(nxt0[64:128, 4:516], nxt0[0:64, 5:517])
                                nc.gpsimd.dma_start(nxt1[0:64, 5:517], nxt1[64:128, 4:516])
                            else:
                                s = scr_p.tile([128, 512], bf, name="scr", tag="scr")
                                nc.scalar.activation(
                                    s[0:64, :], pc[0:64, :], AFT.Relu,
                                    bias=bc[0:64, 2:3],
                                    accum_out=pracc[0:64, 2 * pr:2 * pr + 1])
                                nc.scalar.activation(
                                    s[64:128, :], pc[64:128, :], AFT.Relu,
                                    bias=bc[64:128, 2:3],
                                    accum_out=pracc[64:128, 2 * pr + 1:2 * pr + 2])
                nc.sync.dma_start(pacc_d[:], pracc[:])

            # =================== GCN ===================
            def gcn(nk, nfull, jn, jw, nshard, af_d, as_d, xs_d, w_d, b_d, out_d):
                with (
                    tc.tile_pool(name="gd", bufs=1) as gd,
                    tc.tile_pool(name="ga", bufs=16) as ga,
                    tc.tile_pool(name="gt", bufs=3) as gt,
                    tc.tile_pool(name="ps_g", bufs=2, space=bass.MemorySpace.PSUM) as ps_g,
                    tc.tile_pool(name="ps_g2", bufs=2, space=bass.MemorySpace.PSUM) as ps_g2,
                    tc.tile_pool(name="ps_t2", bufs=2, space=bass.MemorySpace.PSUM) as ps_t2,
                ):
                    xs = gd.tile([128, nk * 64], dt.bfloat16, name="xs", tag="xs")
                    w = gd.tile([64, 128], dt.bfloat16, name="w", tag="w")
                    b = gd.tile([64, 2], dt.float32, name="b", tag="b")
                    x1T = gd.tile([64, nk * 128], dt.bfloat16, name="x1T", tag="x1T")
                    x1n = gd.tile([128, nk * 64], dt.bfloat16, name="x1n", tag="x1n")
                    nc.sync.dma_start(xs[:], xs_d[:])
                    nc.sync.dma_start(w[:], w_d[:])
                    nc.sync.dma_start(b[:], b_d[:])
                    if nk * 128 > nfull:
                        nc.gpsimd.memset(x1T[:, nfull:nk * 128], 0.0)
                    # layer 1 (full, redundant on all cores)
                    for j in range(jn):
                        pg = ps_g.tile([64, jw], dt.float32, name="pg", tag="pg")
                        for k in range(nk):
                            at = ga.tile([128, jw], dt.bfloat16, name="at", tag="at")
                            nc.sync.dma_start(at[:], af_d[k][:, j * jw:(j + 1) * jw])
                            nc.tensor.matmul(pg[:], xs[:, k * 64:(k + 1) * 64], at[:],
                                             start=(k == 0), stop=(k == nk - 1))
                        p1 = gt.tile([64, jw], dt.bfloat16, name="p1", tag="p1")
                        nc.scalar.copy(p1[:], pg[:])
                        pg2 = ps_g2.tile([64, jw], dt.float32, name="pg2", tag="pg2")
                        nc.tensor.matmul(pg2[:], w[:, 0:64], p1[:],
                                         start=True, stop=True)
                        nc.scalar.activation(x1T[:, j * jw:(j + 1) * jw], pg2[:],
                                             AFT.Relu, bias=b[:, 0:1])
                    # transpose x1T -> x1n (normal layout chunks)
                    for k in range(nk):
                        pt2 = ps_t2.tile([128, 64], dt.float32, name="pt2", tag="pt2")
                        nc.tensor.matmul(pt2[:], x1T[:, k * 128:(k + 1) * 128],
                                         idn[0:64, 0:64], start=True, stop=True)
                        nc.scalar.copy(x1n[:, k * 64:(k + 1) * 64], pt2[:])
                    # layer 2 (only this core's shard of rows)
                    pg = ps_g.tile([64, nshard], dt.float32, name="pgs", tag="pg")
                    for k in range(nk):
                        at = ga.tile([128, nshard], dt.bfloat16, name="ats", tag="ats")
                        nc.sync.dma_start(at[:], as_d[k][:])
                        nc.tensor.matmul(pg[:], x1n[:, k * 64:(k + 1) * 64], at[:],
                                         start=(k == 0), stop=(k == nk - 1))
                    p2 = gt.tile([64, nshard], dt.bfloat16, name="p2", tag="p2")
                    nc.scalar.copy(p2[:], pg[:])
                    pg2 = ps_g2.tile([64, nshard], dt.float32, name="pg2s", tag="pg2")
                    nc.tensor.matmul(pg2[:], w[:, 64:128], p2[:],
                                     start=True, stop=True)
                    x2T = gt.tile([64, nshard], dt.bfloat16, name="x2T", tag="x2T")
                    nc.scalar.activation(x2T[:], pg2[:], AFT.Relu, bias=b[:, 1:2])
                    nc.sync.dma_start(out_d[:], x2T[:])

            gcn(16, N_C, 4, 500, CPC, acf_d, acs_d, xsc_d, wgd_d, bgd_d, xc2_d)
            gcn(12, N_P_PAD, 4, 376, PPC, apf_d, aps_d, xsp_d, wgp_d, bgp_d, xp2_d)

            # =================== GNN ===================
            with (
                tc.tile_pool(name="ps_h", bufs=2, space=bass.MemorySpace.PSUM) as ps_h,
                tc.tile_pool(name="ps_t", bufs=3, space=bass.MemorySpace.PSUM) as ps_t,
                tc.tile_pool(name="ps_d", bufs=3, space=bass.MemorySpace.PSUM) as ps_d,
                tc.tile_pool(name="ha_p", bufs=4) as ha_p,
            ):
                for ly in range(3):
                    srcs = [xw0] + dx[:ly]
                    for j in range(NJ):
                        ph = ps_h.tile([128, JW], f32, name="ph", tag="ph")
                        for si, s in enumerate(srcs):
                            nc.tensor.matmul(
                                ph[:], wg[:, ly * 128:(ly + 1) * 128],
                                s[:, j * JW:(j + 1) * JW],
                                start=(si == 0), stop=(si == len(srcs) - 1))
                        nc.scalar.activation(hsT[:, j * JW:(j + 1) * JW], ph[:],
                                             AFT.Relu, bias=bg[:, ly:ly + 1])
                    has = {}
                    for c in range(CHUNK + 1):
                        if c < CHUNK:
                            pt = ps_t.tile([120, 128], f32, name="pt", tag="pt")
                            nc.tensor.matmul(pt[:], hsT[:, c * 120:(c + 1) * 120],
                                             idn[:], start=True, stop=True)
                            ha = ha_p.tile([120, 128], bf, name="ha", tag="ha")
                            nc.scalar.copy(ha[:], pt[:])
                            has[c] = ha
                        if c >= 1:
                            cc = c - 1
                            ha = has.pop(cc)
                            pd = ps_d.tile([128, 120], f32, name="pd", tag="pd")
                            nc.tensor.matmul(
                                pd[0:64, :], ha[:, 0:64],
                                adjb[:, cc * 120:(cc + 1) * 120],
                                start=True, stop=True, tile_position=(0, 0))
                            nc.tensor.matmul(
                                pd[64:128, :], ha[:, 64:128],
                                adjb[:, (CHUNK + cc) * 120:(CHUNK + cc + 1) * 120],
                                start=True, stop=True, tile_position=(0, 64))
                            nc.vector.tensor_copy(dx[ly][:, cc * 120:(cc + 1) * 120], pd[:])

                # comp_int raw sums: reduce over atoms for xs0 + d1 + d2 + d3
                crs = []
                for si, s in enumerate([xw0] + dx):
                    cr = ha_p.tile([128, 3 * CHUNK], f32, name=f"cr{si}", tag=f"cr{si}")
                    nc.vector.reduce_sum(
                        cr[:], s[:].rearrange("p (c a) -> p c a", a=N_ATOMS),
                        axis=mybir.AxisListType.X)
                    crs.append(cr)
                nc.vector.tensor_add(crs[0][:], crs[0][:], crs[1][:])
                nc.vector.tensor_add(crs[2][:], crs[2][:], crs[3][:])
                nc.vector.tensor_add(crs[0][:], crs[0][:], crs[2][:])
                nc.sync.dma_start(csum_d[:], crs[0][:])

    nc.compile()
    return nc


# ---------------------------------------------------------------- phase 2 ----
def build_phase2():
    nc = bacc.Bacc()
    bf, f32 = dt.bfloat16, dt.float32

    ent_d = nc.dram_tensor("ent", [256, BPC], f32, kind="ExternalInput")
    df_d = nc.dram_tensor("df", [8, 128, BPC], bf, kind="ExternalInput")
    pf_d = nc.dram_tensor("pf", [8, 128, BPC], bf, kind="ExternalInput")
    wd1_d = nc.dram_tensor("wd1", [128, 1024], bf, kind="ExternalInput")
    wd23_d = nc.dram_tensor("wd23", [128, 128], f32, kind="ExternalInput")
    wp1_d = nc.dram_tensor("wp1", [128, 1024], bf, kind="ExternalInput")
    wp23_d = nc.dram_tensor("wp23", [128, 128], f32, kind="ExternalInput")
    wo1_d = nc.dram_tensor("wo1", [128, 768], f32, kind="ExternalInput")
    wo2_d = nc.dram_tensor("wo2", [128, 512], f32, kind="ExternalInput")
    wo3_d = nc.dram_tensor("wo3", [128, 512], f32, kind="ExternalInput")
    wi_d = nc.dram_tensor("wi", [128, 4], f32, kind="ExternalInput")
    bia_d = nc.dram_tensor("bia", [128, 16], f32, kind="ExternalInput")
    out_d = nc.dram_tensor("out2", [2, BPC], f32, kind="ExternalOutput")

    with tile.TileContext(nc) as tc:
        with (
            tc.tile_pool(name="data", bufs=1) as data,
            tc.tile_pool(name="ps_a", bufs=2, space=bass.MemorySpace.PSUM) as ps_a,
            tc.tile_pool(name="ps_b", bufs=4, space=bass.MemorySpace.PSUM) as ps_b,
        ):
            wd1 = data.tile([128, 1024], bf, name="wd1", tag="wd1")
            wp1 = data.tile([128, 1024], bf, name="wp1", tag="wp1")
            wd23 = data.tile([128, 128], f32, name="wd23", tag="wd23")
            wp23 = data.tile([128, 128], f32, name="wp23", tag="wp23")
            wo1 = data.tile([128, 768], f32, name="wo1", tag="wo1")
            wo2 = data.tile([128, 512], f32, name="wo2", tag="wo2")
            wo3 = data.tile([128, 512], f32, name="wo3", tag="wo3")
            wi = data.tile([128, 4], f32, name="wi", tag="wi")
            bia = data.tile([128, 16], f32, name="bia", tag="bia")
            nc.sync.dma_start(wd1[:], wd1_d[:])
            nc.sync.dma_start(wp1[:], wp1_d[:])
            xtd = [data.tile([128, BPC], bf, name=f"xtd{k}", tag=f"xtd{k}")
                   for k in range(8)]
            xtp = [data.tile([128, BPC], bf, name=f"xtp{k}", tag=f"xtp{k}")
                   for k in range(8)]
            for k in range(8):
                nc.sync.dma_start(xtd[k][:], df_d[k])
                nc.sync.dma_start(xtp[k][:], pf_d[k])
            for t, d in [(wd23, wd23_d), (wp23, wp23_d), (wo1, wo1_d),
                         (wo2, wo2_d), (wo3, wo3_d), (wi, wi_d), (bia, bia_d)]:
                nc.sync.dma_start(t[:], d[:])
            C = [data.tile([128, BPC], f32, name=f"C{k}", tag=f"C{k}")
                 for k in range(3)]
            nc.sync.dma_start(C[0][:], ent_d[0:128, :])
            nc.sync.dma_start(C[1][64:128, :], ent_d[128:192, :])
            nc.sync.dma_start(C[2][0:64, :], ent_d[192:256, :])

            # preload the sigmoid LUT off the critical path
            warm = data.tile([2, 4], f32, name="warm", tag="warm")
            nc.gpsimd.memset(warm[:], 0.0)
            nc.scalar.activation(warm[0:2, 2:4], warm[0:2, 0:2], AFT.Sigmoid)

            # ---- fd/fp layer-1 (bf16), interleaved
            f1 = {}
            for nm, w1, xt in [("d", wd1, xtd), ("p", wp1, xtp)]:
                f1p = ps_a.tile([128, BPC], f32, name=f"f1p{nm}", tag="psa")
                for k in range(8):
                    nc.tensor.matmul(f1p[:], w1[:, k * 128:(k + 1) * 128], xt[k][:],
                                     start=(k == 0), stop=(k == 7))
                f1[nm] = (f1p, data.tile([128, BPC], f32, name=f"f1{nm}", tag=f"f1{nm}"))
            nc.scalar.activation(f1["d"][1][:], f1["d"][0][:], AFT.Relu, bias=bia[:, 0:1])
            nc.scalar.activation(f1["p"][1][:], f1["p"][0][:], AFT.Relu, bias=bia[:, 3:4])
            # ---- layer-2 (f32)
            f2 = {}
            for nm, w23, bcol in [("d", wd23, 1), ("p", wp23, 4)]:
                f2p = ps_b.tile([128, BPC], f32, name=f"f2p{nm}", tag="psb")
                nc.tensor.matmul(f2p[0:64, :], w23[:, 0:64], f1[nm][1][:],
                                 start=True, stop=True)
                f2[nm] = (f2p, data.tile([64, BPC], f32, name=f"f2{nm}", tag=f"f2{nm}"))
            nc.scalar.activation(f2["d"][1][:], f2["d"][0][0:64, :], AFT.Relu,
                                 bias=bia[0:64, 1:2])
            nc.scalar.activation(f2["p"][1][:], f2["p"][0][0:64, :], AFT.Relu,
                                 bias=bia[0:64, 4:5])
            # ---- layer-3 -> C tiles
            f3pd = ps_b.tile([128, BPC], f32, name="f3pd", tag="psb")
            nc.tensor.matmul(f3pd[0:64, :], wd23[0:64, 64:128], f2["d"][1][:],
                             start=True, stop=True, tile_position=(0, 0))
            f3pp = ps_b.tile([128, BPC], f32, name="f3pp", tag="psb")
            nc.tensor.matmul(f3pp[64:128, :], wp23[0:64, 64:128], f2["p"][1][:],
                             start=True, stop=True, tile_position=(0, 64))
            nc.scalar.activation(C[1][0:64, :], f3pd[0:64, :], AFT.Relu,
                                 bias=bia[0:64, 2:3])
            nc.scalar.activation(C[2][64:128, :], f3pp[64:128, :], AFT.Relu,
                                 bias=bia[64:128, 5:6])

            # ---- head
            h = C
            for li, (wt, nk, bcol) in enumerate([(wo1, 3, 6), (wo2, 2, 8),
                                                 (wo3, 2, 10)]):
                hn = []
                for m in range(2):
                    hp = ps_a.tile([128, BPC], f32, name=f"hp{li}{m}", tag="psa")
                    for k in range(nk):
                        nc.tensor.matmul(hp[:], wt[:, (k * 2 + m) * 128:
                                                    (k * 2 + m + 1) * 128],
                                         h[k][:], start=(k == 0), stop=(k == nk - 1))
                    ht = data.tile([128, BPC], f32, name=f"h{li}{m}", tag=f"h{li}{m}")
                    nc.scalar.activation(ht[:], hp[:], AFT.Relu,
                                         bias=bia[:, bcol + m:bcol + m + 1])
                    hn.append(ht)
                h = hn
            zp = ps_b.tile([2, BPC], f32, name="zp", tag="psb")
            for k in range(2):
                nc.tensor.matmul(zp[:], wi[:, k * 2:(k + 1) * 2], h[k][:],
                                 start=(k == 0), stop=(k == 1))
            zs = data.tile([2, BPC], f32, name="zs", tag="zs")
            nc.scalar.activation(zs[:], zp[:], AFT.Sigmoid, bias=bia[0:2, 12:13])
            nc.sync.dma_start(out_d[:], zs[:])

    nc.compile()
    return nc


# ------------------------------------------------------------- host prep ----
def _prep_phase1_inputs(I):
    """Returns list of 8 per-core input dicts for phase 1."""
    bf = BF16
    emb_fp = np.asarray(I["embed_fp"], F32)
    compounds = np.asarray(I["compounds"])
    adj = np.asarray(I["adjacencies"], F32)
    W_gnn = np.asarray(I["W_gnn"], F32)
    b_gnn = np.asarray(I["b_gnn"], F32)
    emb_w = np.asarray(I["embed_word"], F32).astype(bf)
    proteins = np.asarray(I["proteins"])
    K_cnn = np.asarray(I["K_cnn"], F32)
    b_cnn = np.asarray(I["b_cnn"], F32)

    # GNN packing
    xg = emb_fp[compounds]                       # [N_C, 40, 64] f32
    wg = np.zeros((128, 3 * 128), F32)
    bg = np.zeros((128, 3), F32)
    for i in range(3):
        wg[0:64, i * 128:i * 128 + 64] = W_gnn[i]
        wg[64:128, i * 128 + 64:i * 128 + 128] = W_gnn[i]
        bg[0:64, i] = b_gnn[i]
        bg[64:128, i] = b_gnn[i]
    wg = wg.astype(bf)
    idn = np.eye(128, dtype=F32).astype(bf)

    # CNN packing
    bands = np.stack([_bands(K_cnn[i]) for i in range(3)])   # [3, 11, 64, 64]
    wcp = np.zeros((128, 18 * 64), F32)
    for i in range(3):
        for p in range(6):
            cb = (i * 6 + p) * 64
            wcp[0:64, cb:cb + 64] = bands[i, 2 * p]
            if 2 * p + 1 < 11:
                wcp[64:128, cb:cb + 64] = bands[i, 2 * p + 1]
    wcp = wcp.astype(bf)
    bcp = np.zeros((128, 3), F32)
    bcp[:, :] = b_cnn[None, :]

    prot_pad = np.zeros((N_P_PAD,) + proteins.shape[1:], proteins.dtype)
    prot_pad[:N_P] = proteins
    xw_all = emb_w[prot_pad]                     # [1504, 512, 64] bf16
    xT = np.ascontiguousarray(xw_all.transpose(0, 2, 1))  # [1504, 64, 512]
    xp_all = np.zeros((N_P_PAD, 128, 522), bf)
    xp_all[:, 0:64, 5:517] = xT
    xp_all[:, 64:128, 4:516] = xT

    # GCN packing
    def gcn_pack(A, Xs, Wl, bl, nk, nfull, npad, shard_lo, shard_n):
        Ap = np.zeros((nk * 128, npad), F32)
        Ap[:nfull, :nfull] = A[:nfull, :nfull]
        Xp = np.zeros((nk * 128, 64), F32)
        Xp[:nfull] = Xs[:nfull]
        af = np.ascontiguousarray(
            Ap.reshape(nk, 128, npad)).astype(bf)
        asd = np.ascontiguousarray(
            Ap[:, shard_lo:shard_lo + shard_n].reshape(nk, 128, shard_n)).astype(bf)
        xs = np.ascontiguousarray(
            Xp.reshape(nk, 128, 64).transpose(1, 0, 2).reshape(128, nk * 64)).astype(bf)
        w = np.concatenate([Wl[0], Wl[1]], axis=1).astype(bf)   # [64, 128]
        b = np.stack([bl[0], bl[1]], axis=1).astype(F32)        # [64, 2]
        return af, asd, xs, w, b

    A_c = np.asarray(I["A_c"], F32)
    A_p = np.asarray(I["A_p"], F32)
    Ap_pad = np.zeros((N_P_PAD, N_P_PAD), F32)
    Ap_pad[:N_P, :N_P] = A_p
    Xs_c = np.asarray(I["Xs_c"], F32)
    Xs_p = np.asarray(I["Xs_p"], F32)
    Xsp_pad = np.zeros((N_P_PAD, 64), F32)
    Xsp_pad[:N_P] = Xs_p

    in_maps = []
    for c in range(NCORES):
        m = {}
        # GNN per-core
        xs_c = xg[c * CPC:(c + 1) * CPC]          # [250, 40, 64]
        xw0 = np.zeros((128, GCP), F32)
        for g in range(2):
            blk = xs_c[g * G:(g + 1) * G].reshape(G * N_ATOMS, 64).T  # [64, 5000]
            xw0[g * 64:(g + 1) * 64, :G * N_ATOMS] = blk
        m["xw0"] = xw0.astype(bf)
        adjb = np.zeros((120, 2 * CHUNK * 120), F32)
        for g in range(2):
            for ch in range(CHUNK):
                for k3 in range(3):
                    ci = g * G + ch * 3 + k3
                    if ch * 3 + k3 < G:
                        cb = (g * CHUNK + ch) * 120 + k3 * 40
                        adjb[k3 * 40:(k3 + 1) * 40, cb:cb + 40] = \
                            adj[c * CPC + ci]
        m["adjb"] = adjb.astype(bf)
        m["wg"] = wg
        m["bg"] = bg
        m["idn"] = idn
        # CNN per-core
        m["xp"] = xp_all[c * PPC:(c + 1) * PPC]
        m["wc"] = wcp
        m["bc"] = bcp
        # GCN per-core
        acf, acs, xsc, wgd, bgd = gcn_pack(
            A_c, Xs_c, np.asarray(I["W_gcn_d"], F32), np.asarray(I["b_gcn_d"], F32),
            16, N_C, N_C, c * CPC, CPC)
        m["acf"], m["acs"], m["xsc"], m["wgd"], m["bgd"] = acf, acs, xsc, wgd, bgd
        apf, aps, xsp, wgp, bgp = gcn_pack(
            Ap_pad, Xsp_pad, np.asarray(I["W_gcn_p"], F32), np.asarray(I["b_gcn_p"], F32),
            12, N_P_PAD, N_P_PAD, c * PPC, PPC)
        m["apf"], m["aps"], m["xsp"], m["wgp"], m["bgp"] = apf, aps, xsp, wgp, bgp
        in_maps.append(m)
    return in_maps


def _prep_phase2_inputs(I, comp_intT, Xc2T, prot_intT, Xp2T):
    bf = BF16
    idx_c = np.asarray(I["idx_c"])
    idx_p = np.asarray(I["idx_p"])
    df = np.asarray(I["drug_feat"], F32)
    pf = np.asarray(I["protein_feat"], F32).astype(F32)

    def pack_w1(W):   # [1024, 128] -> [128, 1024] lhsT chunks
        out = np.zeros((128, 1024), F32)
        for k in range(8):
            out[:, k * 128:(k + 1) * 128] = W[k * 128:(k + 1) * 128, :]
        return out.astype(F32)

    def pack_w23(W2, W3):  # [128, 64], [64, 64] -> [128, 128]
        out = np.zeros((128, 128), F32)
        out[:, 0:64] = W2
        out[0:64, 64:128] = W3
        return out.astype(F32)

    def pack_head(W, nk):  # [nk*128, 256] -> [128, nk*256]
        out = np.zeros((128, nk * 256), F32)
        for k in range(nk):
            for mh in range(2):
                out[:, (k * 2 + mh) * 128:(k * 2 + mh + 1) * 128] = \
                    W[k * 128:(k + 1) * 128, mh * 128:(mh + 1) * 128]
        return out.astype(F32)

    wd1 = pack_w1(np.asarray(I["Wd1"], F32)).astype(BF16)
    wd23 = pack_w23(np.asarray(I["Wd2"], F32), np.asarray(I["Wd3"], F32))
    wp1 = pack_w1(np.asarray(I["Wp1"], F32)).astype(BF16)
    wp23 = pack_w23(np.asarray(I["Wp2"], F32), np.asarray(I["Wp3"], F32))
    wo1 = pack_head(np.asarray(I["Wo1"], F32), 3)
    wo2 = pack_head(np.asarray(I["Wo2"], F32), 2)
    wo3 = pack_head(np.asarray(I["Wo3"], F32), 2)
    wi = np.zeros((128, 4), F32)
    Wi = np.asarray(I["W_int"], F32)
    for k in range(2):
        wi[:, k * 2:(k + 1) * 2] = Wi[k * 128:(k + 1) * 128, :]
    wi = wi.astype(F32)
    bia = np.zeros((128, 16), F32)
    bia[:, 0] = np.asarray(I["bd1"], F32)
    bia[0:64, 1] = np.asarray(I["bd2"], F32)
    bia[0:64, 2] = np.asarray(I["bd3"], F32)
    bia[:, 3] = np.asarray(I["bp1"], F32)
    bia[0:64, 4] = np.asarray(I["bp2"], F32)
    bia[64:128, 5] = np.asarray(I["bp3"], F32)
    bo1 = np.asarray(I["bo1"], F32); bia[:, 6] = bo1[0:128]; bia[:, 7] = bo1[128:256]
    bo2 = np.asarray(I["bo2"], F32); bia[:, 8] = bo2[0:128]; bia[:, 9] = bo2[128:256]
    bo3 = np.asarray(I["bo3"], F32); bia[:, 10] = bo3[0:128]; bia[:, 11] = bo3[128:256]
    bia[0:2, 12] = np.asarray(I["b_int"], F32)

    in_maps = []
    for c in range(NCORES):
        ic = idx_c[c * BPC:(c + 1) * BPC]
        ip = idx_p[c * BPC:(c + 1) * BPC]
        ent = np.concatenate([comp_intT[:, ic], Xc2T[:, ic],
                              prot_intT[:, ip], Xp2T[:, ip]], axis=0).astype(F32)
        dfT = np.ascontiguousarray(df[ic].T.astype(BF16)).reshape(8, 128, BPC)
        pfT = np.ascontiguousarray(pf[ip].T.astype(BF16)).reshape(8, 128, BPC)
        m = dict(ent=ent, df=dfT, pf=pfT, wd1=wd1, wd23=wd23, wp1=wp1,
                 wp23=wp23, wo1=wo1, wo2=wo2, wo3=wo3, wi=wi, bia=bia)
        in_maps.append(m)
    return in_maps


_CACHE = {}


def _get_kernels():
    if "p1" not in _CACHE:
        _CACHE["p1"] = build_phase1()
        _CACHE["p2"] = build_phase2()
    return _CACHE["p1"], _CACHE["p2"]


def run(inputs, trace=False):
    """Full pipeline. Returns (output [4096, 2] f32, info dict)."""
    I = inputs
    nc1, nc2 = _get_kernels()
    info = {}

    in1 = _prep_phase1_inputs(I)
    r1 = run_bass_kernel_spmd(nc1, in1, core_ids=list(range(NCORES)), trace=trace)
    res1 = r1.results
    if trace:
        info["p1_exec_ns"] = r1.exec_time_ns

    comp_sumT = np.zeros((64, N_C), F32)
    Xc2T = np.zeros((64, N_C), F32)
    prot_sumT = np.zeros((64, N_P_PAD), F32)
    Xp2T = np.zeros((64, N_P_PAD), F32)
    for c in range(NCORES):
        cs = np.asarray(res1[c]["csum"], F32)       # [128, 126]
        comp_sumT[:, c * CPC:c * CPC + G] = cs[0:64, 0:G]
        comp_sumT[:, c * CPC + G:(c + 1) * CPC] = cs[64:128, 0:G]
        Xc2T[:, c * CPC:(c + 1) * CPC] = np.asarray(res1[c]["xc2"], F32)
        pa = np.asarray(res1[c]["pacc"], F32)       # [128, 188]
        ev = np.arange(0, PPC, 2)
        prot_sumT[:, c * PPC + ev] = pa[0:64, ev]
        prot_sumT[:, c * PPC + ev + 1] = pa[64:128, ev + 1]
        Xp2T[:, c * PPC:(c + 1) * PPC] = np.asarray(res1[c]["xp2"], F32)
    comp_intT = comp_sumT / N_ATOMS
    prot_intT = prot_sumT / L

    in2 = _prep_phase2_inputs(I, comp_intT, Xc2T, prot_intT, Xp2T)
    r2 = run_bass_kernel_spmd(nc2, in2, core_ids=list(range(NCORES)), trace=trace)
    res2 = r2.results
    if trace:
        info["p2_exec_ns"] = r2.exec_time_ns

    out = np.zeros((B, 2), F32)
    for c in range(NCORES):
        out[c * BPC:(c + 1) * BPC] = np.asarray(res2[c]["out2"], F32).T
    return out, info


def kernel(**inputs):
    out, _ = run(inputs)
    return out
